# revision 1
# baseline (speedup 1.0000x reference)
"""GCN (2-layer, PyG gcn_norm) on 8 Trainium2 NeuronCores via Bass.

Strategy (dst-range sharding, no collectives):
  * Host sorts edges by dst and lays them out into per-node-tile slots
    (128-node tiles, padded to S chunks of 128 edge-lanes per tile).
  * Device sweep = for each node tile: one-hot dst masks (built on DVE from
    relative dst values) are the PE moving tensor; per-edge messages are the
    stationary tensor, split into bf16 hi/lo columns so the fp32 value is
    reconstructed exactly in PSUM accumulation (error ~2^-16 relative).
  * Three sequential NEFF launches: (1) deg -> dinv, (2) layer-1 aggregation
    -> h -> v, (3) layer-2 aggregation -> output. Between launches the host
    only performs index-space stream building (gather of returned per-node
    arrays into per-edge streams).
"""

import os
import sys

sys.path.insert(0, "/opt/trn_rl_repo")

import numpy as np
import ml_dtypes

import concourse.bass as bass
import concourse.tile as tile
from concourse import mybir
from concourse.bass_utils import run_bass_kernel_spmd

BF16 = ml_dtypes.bfloat16

N = 100000
E = 3200000
D = 2
HID = 16
NCORE = 8
TPC = 98                      # node tiles per core
TOTAL_TILES = NCORE * TPC     # 784
NPAD = TOTAL_TILES * 128      # 100352
NSH = TPC * 128               # 12544 nodes per core
MASK_BATCH = 8                # chunks per mask-build instruction
G = 8                         # same-dst edges pre-reduced per lane (DVE)


def _split_multi_waits(nc):
    """This toolchain's walrus encodes at most one sync-wait per instruction.
    Hoist extra waits onto fresh single-wait NoOps placed just before."""
    ctr = 0
    for fn in nc.m.functions:
        for bb in fn.blocks:
            insts = list(bb.instructions)
            if not any(
                i.sync_info is not None and len(i.sync_info.on_wait or []) > 1
                for i in insts
            ):
                continue
            new = []
            for inst in insts:
                si = inst.sync_info
                if si is not None and len(si.on_wait or []) > 1:
                    waits = list(si.on_wait)
                    for w in waits[:-1]:
                        ctr += 1
                        new.append(
                            mybir.InstNoOp(
                                name=f"wsplit-{ctr}",
                                engine=inst.engine,
                                sync_info=mybir.SyncInfo(on_wait=[w], on_update=[]),
                                bass_nofuse=True,
                            )
                        )
                    si.on_wait = [waits[-1]]
                new.append(inst)
            bb.instructions = new
    return ctr


def _preprocess(edge_index, edge_weight):
    """Sort edges by dst; group each node's edges into lanes of G (device
    pre-reduces the G slots of a lane on DVE before the PE scatter).
    Edge slot layout: (tile, chunk, partition-lane, g); lane layout for the
    per-lane dst-relative id: (tile, chunk, partition-lane)."""
    src = edge_index[0].astype(np.int64)
    dst = edge_index[1].astype(np.int64)
    perm = np.argsort(dst, kind="stable")
    src_s = src[perm]
    dst_s = dst[perm]
    ew_s = edge_weight[perm].astype(np.float32)

    c = np.bincount(dst_s, minlength=NPAD)            # per-node degree
    L = (c + G - 1) // G                              # lanes per node
    Lt = L.reshape(TOTAL_TILES, 128)
    S = int(np.ceil(Lt.sum(axis=1).max() / 128))      # lane chunks per tile
    lane_off = np.zeros_like(Lt)
    lane_off[:, 1:] = np.cumsum(Lt[:, :-1], axis=1)
    lane_base = lane_off.reshape(NPAD)                # node's first lane in tile

    node_start = np.zeros(NPAD + 1, np.int64)
    np.cumsum(c, out=node_start[1:])
    r = np.arange(len(dst_s)) - node_start[dst_s]
    lane = lane_base[dst_s] + r // G                  # lane within tile
    g = r % G
    t = dst_s >> 7
    lane_slot = (t * S + lane // 128) * 128 + (lane % 128)
    slot = lane_slot * G + g

    tot = TOTAL_TILES * S * 128
    ew_pad = np.zeros(tot * G, np.float32)
    src_pad = np.zeros(tot * G, np.int64)
    dstrel_pad = np.zeros(tot, np.float32)
    ew_pad[slot] = ew_s
    src_pad[slot] = src_s
    dstrel_pad[lane_slot] = (dst_s & 127).astype(np.float32)
    return dict(S=S, ew=ew_pad, src=src_pad, dstrel=dstrel_pad)


def _to_core_stream(arr, S, dtype, inner=1):
    """[TOTAL_TILES*S*128*inner] slot array -> per-core [128, TPC*S*inner]."""
    a = arr.reshape(TOTAL_TILES, S, 128, inner)
    out = []
    for c in range(NCORE):
        blk = a[c * TPC:(c + 1) * TPC]            # [TPC, S, 128, inner]
        out.append(np.ascontiguousarray(
            blk.transpose(2, 0, 1, 3).reshape(128, TPC * S * inner)).astype(dtype))
    return out


def _to_core_nodes(arr, dtype):
    """[NPAD] node array -> per-core [128, TPC] (node = c*NSH + t*128 + p)."""
    a = arr.reshape(TOTAL_TILES, 128)
    out = []
    for c in range(NCORE):
        blk = a[c * TPC:(c + 1) * TPC]            # [TPC, 128]
        out.append(np.ascontiguousarray(blk.T).astype(dtype))
    return out


def _from_core_nodes(parts):
    """inverse of _to_core_nodes -> [NPAD] float32."""
    full = np.empty((TOTAL_TILES, 128), np.float32)
    for c in range(NCORE):
        full[c * TPC:(c + 1) * TPC] = np.asarray(parts[c], np.float32).T
    return full.reshape(NPAD)


def _mask_build(nc, mask_t, dstrel_t, iota_sb, s0, nb):
    """mask_t[:, 0:nb*128] (bf16) = one-hot of dstrel_t[:, s0:s0+nb] vs iota."""
    in0 = dstrel_t[:, s0:s0 + nb].unsqueeze(2).broadcast_to([128, nb, 128])
    in1 = iota_sb[:, :].unsqueeze(1).broadcast_to([128, nb, 128])
    out = mask_t[:, 0:nb * 128].rearrange("p (s n) -> p s n", n=128)
    nc.vector.tensor_tensor(out, in0, in1, mybir.AluOpType.is_equal)


def _hilo(nc, pool, S, msrc, mt_view_hi, mt_view_lo):
    """Write bf16 hi/lo split of f32 msrc into (strided) bf16 views."""
    nc.vector.tensor_copy(mt_view_hi, msrc)                      # f32 -> bf16
    hif = pool.tile([128, S], mybir.dt.float32, tag="hif")
    nc.vector.tensor_copy(hif, mt_view_hi)                       # bf16 -> f32
    lof = pool.tile([128, S], mybir.dt.float32, tag="lof")
    nc.vector.tensor_sub(lof, msrc, hif)
    nc.vector.tensor_copy(mt_view_lo, lof)                       # f32 -> bf16


def _build_sweep(mode, S, tpc=TPC, reps=1, skip_masks=False, skip_mm=False):
    """Build the Bass program for one sweep. mode in {deg, layer1, layer2}.
    reps>1 wraps the main tile loop in a hardware For_i loop repeating the
    identical (idempotent) work — used only for timing measurements."""
    from contextlib import ExitStack

    F = D if mode == "layer1" else 1
    M = 2 * F  # stationary columns (hi.., lo..)
    CS = tpc * S
    CSG = tpc * S * G
    f32 = mybir.dt.float32
    bf = mybir.dt.bfloat16

    nc = bass.Bass("TRN2", target_bir_lowering=False, debug=False,
                   num_devices=NCORE)
    dram = {}

    def din(name, shape, dtype=f32):
        dram[name] = nc.dram_tensor(name, shape, dtype, kind="ExternalInput").ap()
        return dram[name]

    def dout(name, shape, dtype=f32):
        dram[name] = nc.dram_tensor(name, shape, dtype, kind="ExternalOutput").ap()
        return dram[name]

    iota_d = din("iota", [128, 128], bf)
    ident_d = din("ident", [128, 128])
    dstrel_d = din("dstrel", [128, CS], bf)
    ew_d = din("ew", [128, CSG])
    if mode == "layer1":
        dinvsrc_d = din("dinvsrc", [128, CSG])
        xs_d = [din(f"xsrc{f}", [128, CSG]) for f in range(D)]
        dinv_d = din("dinvn", [128, tpc])
        xn_d = [din(f"xn{f}", [128, tpc]) for f in range(D)]
        w1_d = din("w1b", [128, 2 * HID])
        b1_d = din("b1b", [128, HID])
        w2_d = din("w2b", [128, HID])
        v_out = dout("vout", [128, tpc])
    elif mode == "layer2":
        vsrc_d = din("vsrc", [128, CSG])
        dinv_d = din("dinvn", [128, tpc])
        vn_d = din("vn", [128, tpc])
        b2_d = din("b2b", [128, 1])
        y_out = dout("yout", [128, tpc])
    else:
        dinv_out = dout("dinvout", [128, tpc])

    with tile.TileContext(nc) as tc, ExitStack() as ctx:
        const = ctx.enter_context(tc.tile_pool(name="const", bufs=1))
        sp = ctx.enter_context(tc.tile_pool(name="streams", bufs=3))
        wp = ctx.enter_context(tc.tile_pool(name="work", bufs=3))
        mp = ctx.enter_context(tc.tile_pool(name="masks", bufs=3))
        accp = ctx.enter_context(tc.tile_pool(name="acc", bufs=1))
        psA = ctx.enter_context(tc.tile_pool(name="psA", bufs=4, space="PSUM"))
        psB = ctx.enter_context(tc.tile_pool(name="psB", bufs=2, space="PSUM"))

        iota_sb = const.tile([128, 128], bf)
        nc.sync.dma_start(iota_sb[:], iota_d[:])
        ident_sb = const.tile([128, 128], f32)
        nc.sync.dma_start(ident_sb[:], ident_d[:])

        aggN = accp.tile([128, tpc * F], f32)  # col = t*F + f

        if mode == "layer1":
            dinvN = const.tile([128, tpc], f32)
            nc.sync.dma_start(dinvN[:], dinv_d[:])
            xN = []
            for f in range(D):
                t_ = const.tile([128, tpc], f32, tag=f"xn{f}")
                nc.sync.dma_start(t_[:], xn_d[f][:])
                xN.append(t_)
            w1_sb = const.tile([128, 2 * HID], f32)
            nc.sync.dma_start(w1_sb[:], w1_d[:])
            b1_sb = const.tile([128, HID], f32)
            nc.sync.dma_start(b1_sb[:], b1_d[:])
            w2_sb = const.tile([128, HID], f32)
            nc.sync.dma_start(w2_sb[:], w2_d[:])
        elif mode == "layer2":
            dinvN = const.tile([128, tpc], f32)
            nc.sync.dma_start(dinvN[:], dinv_d[:])
            vN = const.tile([128, tpc], f32)
            nc.sync.dma_start(vN[:], vn_d[:])
            b2_sb = const.tile([128, 1], f32)
            nc.sync.dma_start(b2_sb[:], b2_d[:])

        def _reduce_g(dst, src_f32):
            nc.vector.tensor_reduce(
                dst.unsqueeze(2), src_f32.rearrange("p (s g) -> p s g", g=G),
                mybir.AxisListType.X, mybir.AluOpType.add)

        def _tile_loop():
          for t in range(tpc):
            c0 = t * S
            cg0 = t * S * G
            ew_t = sp.tile([128, S * G], f32, tag="ew")
            nc.sync.dma_start(ew_t[:], ew_d[:, cg0:cg0 + S * G])
            dstrel_t = sp.tile([128, S], bf, tag="dstrel")
            nc.sync.dma_start(dstrel_t[:], dstrel_d[:, c0:c0 + S])

            # stationary tensor [128, M*S]; chunk s occupies cols M*s..M*s+M
            # with column order (hi_0..hi_{F-1}, lo_0..lo_{F-1})
            mt = wp.tile([128, M * S], bf, tag="mt")
            mtv = mt.rearrange("p (s k) -> p k s", k=M)

            if mode == "deg":
                red = wp.tile([128, S], f32, tag="red0")
                _reduce_g(red, ew_t)
                _hilo(nc, wp, S, red, mtv[:, 0], mtv[:, 1])
            elif mode == "layer1":
                dsrc_t = sp.tile([128, S * G], f32, tag="dsrc")
                nc.sync.dma_start(dsrc_t[:], dinvsrc_d[:, cg0:cg0 + S * G])
                t1 = wp.tile([128, S * G], f32, tag="t1")
                nc.vector.tensor_mul(t1, ew_t, dsrc_t)
                for f in range(D):
                    xs_t = sp.tile([128, S * G], f32, tag=f"xs{f}")
                    nc.sync.dma_start(xs_t[:], xs_d[f][:, cg0:cg0 + S * G])
                    mf = wp.tile([128, S * G], f32, tag=f"mf{f}")
                    nc.vector.tensor_mul(mf, t1, xs_t)
                    red = wp.tile([128, S], f32, tag=f"red{f}")
                    _reduce_g(red, mf)
                    _hilo(nc, wp, S, red, mtv[:, f], mtv[:, F + f])
            else:
                vs_t = sp.tile([128, S * G], f32, tag="vs")
                nc.sync.dma_start(vs_t[:], vsrc_d[:, cg0:cg0 + S * G])
                mf = wp.tile([128, S * G], f32, tag="mf0")
                nc.vector.tensor_mul(mf, ew_t, vs_t)
                red = wp.tile([128, S], f32, tag="red0")
                _reduce_g(red, mf)
                _hilo(nc, wp, S, red, mtv[:, 0], mtv[:, 1])

            psum_t = psA.tile([M, 128], f32, tag="psum")
            s = 0
            while s < S:
                nb = min(MASK_BATCH, S - s)
                mask_t = mp.tile([128, MASK_BATCH * 128], bf, tag="mask")
                if not skip_masks or (t == 0 and s == 0):
                    _mask_build(nc, mask_t, dstrel_t, iota_sb, s, nb)
                for k in range(nb):
                    if skip_mm and not (s + k == 0 or s + k == S - 1):
                        continue
                    nc.tensor.matmul(
                        psum_t[:, :],
                        lhsT=mt[:, M * (s + k):M * (s + k) + M],
                        rhs=mask_t[:, k * 128:(k + 1) * 128],
                        start=(s + k == 0),
                        stop=(s + k == S - 1),
                    )
                s += nb

            # transpose [M,128] -> [128,M] and combine hi+lo into aggN
            zt = wp.tile([M, 128], f32, tag="zt")
            nc.vector.tensor_copy(zt, psum_t)
            pt2 = psB.tile([128, M], f32, tag="pt2")
            nc.tensor.matmul(pt2[:, :], lhsT=zt[:, :],
                             rhs=ident_sb[0:M, 0:M], is_transpose=True)
            ptsb = wp.tile([128, M], f32, tag="ptsb")
            nc.vector.tensor_copy(ptsb, pt2)
            nc.vector.tensor_add(aggN[:, t * F:(t + 1) * F],
                                 ptsb[:, 0:F], ptsb[:, F:M])

        if reps > 1:
            with tc.For_i(0, reps, 1):
                _tile_loop()
        else:
            _tile_loop()

        # ---- epilogue ----
        agf = aggN.rearrange("p (t f) -> p f t", f=F)
        if mode == "deg":
            deg = wp.tile([128, tpc], f32, tag="deg")
            nc.vector.tensor_scalar(deg, agf[:, 0], 1.0, None,
                                    mybir.AluOpType.add)
            sq = wp.tile([128, tpc], f32, tag="sq")
            nc.scalar.activation(sq, deg, mybir.ActivationFunctionType.Sqrt)
            dinv = wp.tile([128, tpc], f32, tag="dinvr")
            nc.vector.reciprocal(dinv, sq)
            nc.sync.dma_start(dinv_out[:], dinv[:])
        elif mode == "layer1":
            dsq = wp.tile([128, tpc], f32, tag="dsq")
            nc.vector.tensor_mul(dsq, dinvN, dinvN)
            zf = []
            for f in range(D):
                a = wp.tile([128, tpc], f32, tag=f"za{f}")
                nc.vector.tensor_mul(a, agf[:, f], dinvN)
                b = wp.tile([128, tpc], f32, tag=f"zb{f}")
                nc.vector.tensor_mul(b, xN[f], dsq)
                z = accp.tile([128, tpc], f32, tag=f"z{f}")
                nc.vector.tensor_add(z, a, b)
                zf.append(z)
            h_sb = accp.tile([128, HID * tpc], f32)
            for j in range(HID):
                hj = h_sb[:, j * tpc:(j + 1) * tpc]
                a = wp.tile([128, tpc], f32, tag="ha")
                nc.vector.tensor_scalar(a, zf[0], w1_sb[:, j:j + 1], None,
                                        mybir.AluOpType.mult)
                b = wp.tile([128, tpc], f32, tag="hb")
                nc.vector.tensor_scalar(b, zf[1], w1_sb[:, HID + j:HID + j + 1],
                                        None, mybir.AluOpType.mult)
                nc.vector.tensor_add(hj, a, b)
                nc.vector.tensor_scalar(hj, hj, b1_sb[:, j:j + 1], None,
                                        mybir.AluOpType.add)
            nc.scalar.activation(h_sb, h_sb, mybir.ActivationFunctionType.Relu)
            vacc = wp.tile([128, tpc], f32, tag="vacc")
            nc.vector.tensor_scalar(vacc, h_sb[:, 0:tpc], w2_sb[:, 0:1], None,
                                    mybir.AluOpType.mult)
            for j in range(1, HID):
                tmp = wp.tile([128, tpc], f32, tag="vtmp")
                nc.vector.tensor_scalar(tmp, h_sb[:, j * tpc:(j + 1) * tpc],
                                        w2_sb[:, j:j + 1], None,
                                        mybir.AluOpType.mult)
                nc.vector.tensor_add(vacc, vacc, tmp)
            vres = wp.tile([128, tpc], f32, tag="vres")
            nc.vector.tensor_mul(vres, vacc, dinvN)
            nc.sync.dma_start(v_out[:], vres[:])
        else:
            a = wp.tile([128, tpc], f32, tag="ya")
            nc.vector.tensor_mul(a, agf[:, 0], dinvN)
            b = wp.tile([128, tpc], f32, tag="yb")
            nc.vector.tensor_mul(b, vN, dinvN)
            y = wp.tile([128, tpc], f32, tag="y")
            nc.vector.tensor_add(y, a, b)
            nc.vector.tensor_scalar(y, y, b2_sb[:, 0:1], None,
                                    mybir.AluOpType.add)
            nc.sync.dma_start(y_out[:], y[:])

    _split_multi_waits(nc)
    return nc


_IOTA = np.tile(np.arange(128, dtype=np.float32).astype(BF16), (128, 1))
_IDENT = np.eye(128, dtype=np.float32)


def kernel(x, edge_index, edge_weight, W1, b1, W2, b2, _timing=None):
    x = np.asarray(x, np.float32)
    edge_index = np.asarray(edge_index)
    edge_weight = np.asarray(edge_weight, np.float32)
    W1 = np.asarray(W1, np.float32)
    b1 = np.asarray(b1, np.float32)
    W2 = np.asarray(W2, np.float32)
    b2 = np.asarray(b2, np.float32)

    pp = _preprocess(edge_index, edge_weight)
    S = pp["S"]

    xp = np.zeros((NPAD, D), np.float32)
    xp[:N] = x

    ew_cs = _to_core_stream(pp["ew"], S, np.float32, inner=G)
    dstrel_cs = _to_core_stream(pp["dstrel"], S, BF16)

    common = {"iota": np.ascontiguousarray(_IOTA),
              "ident": np.ascontiguousarray(_IDENT)}

    # ---- NEFF 1: deg -> dinv ----
    nc1 = _build_sweep("deg", S)
    in1 = [dict(common, dstrel=dstrel_cs[c], ew=ew_cs[c]) for c in range(NCORE)]
    r1 = run_bass_kernel_spmd(nc1, in1, core_ids=list(range(NCORE)))
    dinv = _from_core_nodes([r1.results[c]["dinvout"] for c in range(NCORE)])

    # ---- host glue: per-edge dinv[src], x[src] streams ----
    dinvsrc_cs = _to_core_stream(dinv[pp["src"]], S, np.float32, inner=G)
    xs_cs = [_to_core_stream(xp[pp["src"], f], S, np.float32, inner=G)
             for f in range(D)]
    dinv_n = _to_core_nodes(dinv, np.float32)
    xn = [_to_core_nodes(xp[:, f], np.float32) for f in range(D)]
    w1b = np.tile(W1.reshape(1, 2 * HID), (128, 1)).astype(np.float32)
    b1b = np.tile(b1.reshape(1, HID), (128, 1)).astype(np.float32)
    w2b = np.tile(W2.reshape(1, HID), (128, 1)).astype(np.float32)
    b2b = np.full((128, 1), float(b2[0]), np.float32)

    # ---- NEFF 2: layer 1 -> v ----
    nc2 = _build_sweep("layer1", S)
    in2 = [dict(common, dstrel=dstrel_cs[c], ew=ew_cs[c],
                dinvsrc=dinvsrc_cs[c], xsrc0=xs_cs[0][c], xsrc1=xs_cs[1][c],
                dinvn=dinv_n[c], xn0=xn[0][c], xn1=xn[1][c],
                w1b=w1b, b1b=b1b, w2b=w2b) for c in range(NCORE)]
    r2 = run_bass_kernel_spmd(nc2, in2, core_ids=list(range(NCORE)))
    v = _from_core_nodes([r2.results[c]["vout"] for c in range(NCORE)])

    # ---- host glue: v[src] stream ----
    vsrc_cs = _to_core_stream(v[pp["src"]], S, np.float32, inner=G)
    vn = _to_core_nodes(v, np.float32)

    # ---- NEFF 3: layer 2 -> output ----
    nc3 = _build_sweep("layer2", S)
    in3 = [dict(common, dstrel=dstrel_cs[c], ew=ew_cs[c], vsrc=vsrc_cs[c],
                dinvn=dinv_n[c], vn=vn[c], b2b=b2b) for c in range(NCORE)]
    r3 = run_bass_kernel_spmd(nc3, in3, core_ids=list(range(NCORE)))
    y = _from_core_nodes([r3.results[c]["yout"] for c in range(NCORE)])

    return y[:N, None].astype(np.float32)



# revision 3
# speedup vs baseline: 1.9273x; 1.9273x over previous
"""GCN (2-layer, PyG gcn_norm) on 8 Trainium2 NeuronCores via Bass.

Strategy (dst-partition-row sharding, no collectives, no PE):
  * Host sorts nodes by in-degree and assigns each node one SBUF
    partition-row of G slots (G = per-stripe max degree, rounded up), so
    the per-node segment-sum is a plain free-axis tensor_reduce — no
    one-hot masks, no matmuls.  Stripes of 1024 nodes (one 128-node tile
    per core) share a G schedule so all 8 SPMD cores run one program.
  * Per-edge streams are bf16 (tolerance is 2e-2); node-level arrays and
    accumulations stay f32.
  * Three sequential NEFF launches: (1) deg -> dinv, x*dinv, (2) layer-1
    aggregation -> h -> v*dinv, (3) layer-2 aggregation -> output.
    Between launches the host only gathers returned per-node arrays into
    per-edge streams (index-space data movement, no float math).
"""

import sys

sys.path.insert(0, "/opt/trn_rl_repo")

import numpy as np
import ml_dtypes

import concourse.bass as bass
import concourse.tile as tile
from concourse import mybir
from concourse.bass_utils import run_bass_kernel_spmd

BF16 = ml_dtypes.bfloat16

N = 100000
E = 3200000
D = 2
HID = 16
NCORE = 8
TPC = 98                      # stripes == node tiles per core
NPAD = TPC * 1024             # 100352
GMULT = 4                     # stripe slot width rounded up to this
BLK_COLS = 1024               # target stream columns per DMA block


def _split_multi_waits(nc):
    """This toolchain's walrus encodes at most one sync-wait per instruction.
    Hoist extra waits onto fresh single-wait NoOps placed just before."""
    ctr = 0
    for fn in nc.m.functions:
        for bb in fn.blocks:
            insts = list(bb.instructions)
            if not any(
                i.sync_info is not None and len(i.sync_info.on_wait or []) > 1
                for i in insts
            ):
                continue
            new = []
            for inst in insts:
                si = inst.sync_info
                if si is not None and len(si.on_wait or []) > 1:
                    waits = list(si.on_wait)
                    for w in waits[:-1]:
                        ctr += 1
                        new.append(
                            mybir.InstNoOp(
                                name=f"wsplit-{ctr}",
                                engine=inst.engine,
                                sync_info=mybir.SyncInfo(on_wait=[w], on_update=[]),
                                bass_nofuse=True,
                            )
                        )
                    si.on_wait = [waits[-1]]
                new.append(inst)
            bb.instructions = new
    return ctr


def _preprocess(edge_index, edge_weight):
    """Degree-sort nodes, assign each node a partition-row slot range, and
    scatter edge weight / src index into the per-core slot streams."""
    dst = edge_index[1].astype(np.int64)
    src = edge_index[0].astype(np.int64)
    ew = edge_weight.astype(np.float32)

    deg = np.bincount(dst, minlength=NPAD)
    order = np.argsort(deg, kind="stable")       # newpos -> orig id
    newpos = np.empty(NPAD, np.int64)
    newpos[order] = np.arange(NPAD)

    counts_new = deg[order]                      # per-newpos degree
    smax = counts_new.reshape(TPC, 1024).max(axis=1)
    G = np.maximum(GMULT, ((smax + GMULT - 1) // GMULT) * GMULT).astype(np.int64)
    offs = np.zeros(TPC + 1, np.int64)
    np.cumsum(G, out=offs[1:])
    CS = int(offs[-1])

    nd = newpos[dst]
    start = np.zeros(NPAD + 1, np.int64)
    np.cumsum(counts_new, out=start[1:])
    perm = np.argsort(nd, kind="stable")
    r = np.empty(E, np.int64)
    r[perm] = np.arange(E) - start[nd[perm]]     # rank of edge within its dst

    s = nd >> 10
    w = nd & 1023
    c = w >> 7
    p = w & 127
    flat = (c * 128 + p) * CS + offs[s] + r

    ew_flat = np.zeros(NCORE * 128 * CS, np.float32)
    src_flat = np.zeros(NCORE * 128 * CS, np.int64)
    ew_flat[flat] = ew
    src_flat[flat] = src

    # DMA blocks: consecutive stripes until >= BLK_COLS columns; per-block
    # runs of stripes sharing G (one tensor_reduce instruction per run).
    blocks = []
    t0, cols = 0, 0
    for t in range(TPC):
        cols += int(G[t])
        if cols >= BLK_COLS or t == TPC - 1:
            runs = []
            ro = 0
            for tt in range(t0, t + 1):
                g = int(G[tt])
                if runs and runs[-1][2] == g:
                    runs[-1] = (runs[-1][0], runs[-1][1] + 1, g, runs[-1][3])
                else:
                    runs.append((tt, 1, g, ro))
                ro += g
            blocks.append((t0, t + 1 - t0, int(offs[t0]), cols, runs))
            t0, cols = t + 1, 0

    return dict(G=G, offs=offs, CS=CS, blocks=blocks, order=order,
                ew=ew_flat, src=src_flat)


def _to_core_stream(arrflat, CS, dtype):
    a = arrflat.reshape(NCORE, 128, CS)
    return [np.ascontiguousarray(a[c]).astype(dtype) for c in range(NCORE)]


def _to_core_nodes(val_new, dtype):
    """[NPAD] array in newpos space -> per-core [128, TPC]
    (newpos = s*1024 + c*128 + p)."""
    a = val_new.reshape(TPC, NCORE, 128)
    return [np.ascontiguousarray(a[:, c, :].T).astype(dtype) for c in range(NCORE)]


def _from_core_nodes(parts):
    full = np.empty((TPC, NCORE, 128), np.float32)
    for c in range(NCORE):
        full[:, c, :] = np.asarray(parts[c], np.float32).T
    return full.reshape(NPAD)


def _build_sweep(mode, sched, reps=1):
    """Build the Bass program for one sweep. mode in {deg, layer1, layer2}.
    reps>1 wraps the body in a hardware For_i loop repeating the identical
    (idempotent) work — used only for timing measurements."""
    from contextlib import ExitStack

    CS = sched["CS"]
    blocks = sched["blocks"]
    BCMAX = max(b[3] for b in blocks)
    f32 = mybir.dt.float32
    bf = mybir.dt.bfloat16

    nc = bass.Bass("TRN2", target_bir_lowering=False, debug=False,
                   num_devices=NCORE)

    def din(name, shape, dtype=f32):
        return nc.dram_tensor(name, shape, dtype, kind="ExternalInput").ap()

    def dout(name, shape, dtype=f32):
        return nc.dram_tensor(name, shape, dtype, kind="ExternalOutput").ap()

    ew_d = din("ew", [128, CS], bf)
    if mode == "deg":
        xn_d = [din(f"xn{f}", [128, TPC]) for f in range(D)]
        dinv_out = dout("dinvout", [128, TPC])
        xt_out = [dout(f"xtout{f}", [128, TPC]) for f in range(D)]
    elif mode == "layer1":
        xs_d = [din(f"xs{f}", [128, CS], bf) for f in range(D)]
        dinv_d = din("dinvn", [128, TPC])
        xt_d = [din(f"xtn{f}", [128, TPC]) for f in range(D)]
        w1r_d = [din(f"w1r{f}b", [128, HID], bf) for f in range(D)]
        b1_d = din("b1b", [128, HID], bf)
        w2_d = din("w2b", [128, HID], bf)
        vt_out = dout("vtout", [128, TPC])
    else:
        vs_d = din("vs", [128, CS], bf)
        dinv_d = din("dinvn", [128, TPC])
        vt_d = din("vtn", [128, TPC])
        b2_d = din("b2b", [128, 1])
        y_out = dout("yout", [128, TPC])

    with tile.TileContext(nc) as tc, ExitStack() as ctx:
        const = ctx.enter_context(tc.tile_pool(name="const", bufs=1))
        sp = ctx.enter_context(tc.tile_pool(name="streams", bufs=3))
        wp = ctx.enter_context(tc.tile_pool(name="work", bufs=3))
        accp = ctx.enter_context(tc.tile_pool(name="acc", bufs=1))

        if mode == "deg":
            xN = []
            for f in range(D):
                t_ = const.tile([128, TPC], f32, tag=f"xn{f}")
                nc.sync.dma_start(t_[:], xn_d[f][:])
                xN.append(t_)
        elif mode == "layer1":
            dinvN = const.tile([128, TPC], f32)
            nc.sync.dma_start(dinvN[:], dinv_d[:])
            xtN = []
            for f in range(D):
                t_ = const.tile([128, TPC], f32, tag=f"xtn{f}")
                nc.sync.dma_start(t_[:], xt_d[f][:])
                xtN.append(t_)
            w1r = []
            for f in range(D):
                t_ = const.tile([128, HID], bf, tag=f"w1r{f}")
                nc.sync.dma_start(t_[:], w1r_d[f][:])
                w1r.append(t_)
            b1_sb = const.tile([128, HID], bf)
            nc.sync.dma_start(b1_sb[:], b1_d[:])
            w2_sb = const.tile([128, HID], bf)
            nc.sync.dma_start(w2_sb[:], w2_d[:])
        else:
            dinvN = const.tile([128, TPC], f32)
            nc.sync.dma_start(dinvN[:], dinv_d[:])
            vtN = const.tile([128, TPC], f32)
            nc.sync.dma_start(vtN[:], vt_d[:])
            b2_sb = const.tile([128, 1], f32)
            nc.sync.dma_start(b2_sb[:], b2_d[:])

        nF = D if mode == "layer1" else 1
        agg = [accp.tile([128, TPC], f32, tag=f"agg{f}", name=f"agg{f}")
               for f in range(nF)]

        def _reduce_runs(m_t, runs, dst_agg):
            for (tt, nt, g, ro) in runs:
                nc.vector.tensor_reduce(
                    dst_agg[:, tt:tt + nt],
                    m_t[:, ro:ro + nt * g].rearrange("p (t g) -> p t g", g=g),
                    mybir.AxisListType.X, mybir.AluOpType.add)

        def body():
            for (t0, ntb, c0, bc, runs) in blocks:
                ew_t = sp.tile([128, BCMAX], bf, tag="ew")
                nc.sync.dma_start(ew_t[:, 0:bc], ew_d[:, c0:c0 + bc])
                if mode == "deg":
                    _reduce_runs(ew_t, runs, agg[0])
                elif mode == "layer1":
                    xs_t = []
                    for f in range(D):
                        t_ = sp.tile([128, BCMAX], bf, tag=f"xs{f}")
                        nc.sync.dma_start(t_[:, 0:bc], xs_d[f][:, c0:c0 + bc])
                        xs_t.append(t_)
                    m0 = wp.tile([128, BCMAX], bf, tag="m0")
                    nc.gpsimd.tensor_mul(m0[:, 0:bc], ew_t[:, 0:bc],
                                         xs_t[0][:, 0:bc])
                    m1 = wp.tile([128, BCMAX], bf, tag="m1")
                    nc.vector.tensor_mul(m1[:, 0:bc], ew_t[:, 0:bc],
                                         xs_t[1][:, 0:bc])
                    _reduce_runs(m0, runs, agg[0])
                    _reduce_runs(m1, runs, agg[1])
                else:
                    vs_t = sp.tile([128, BCMAX], bf, tag="vs")
                    nc.sync.dma_start(vs_t[:, 0:bc], vs_d[:, c0:c0 + bc])
                    m0 = wp.tile([128, BCMAX], bf, tag="m0")
                    nc.gpsimd.tensor_mul(m0[:, 0:bc], ew_t[:, 0:bc],
                                         vs_t[:, 0:bc])
                    _reduce_runs(m0, runs, agg[0])

            # ---- epilogue ----
            if mode == "deg":
                degp = wp.tile([128, TPC], f32, tag="degp")
                nc.vector.tensor_scalar(degp, agg[0], 1.0, None,
                                        mybir.AluOpType.add)
                sq = wp.tile([128, TPC], f32, tag="sq")
                nc.scalar.activation(sq, degp, mybir.ActivationFunctionType.Sqrt)
                dinv = wp.tile([128, TPC], f32, tag="dinv")
                nc.vector.reciprocal(dinv, sq)
                nc.sync.dma_start(dinv_out[:], dinv[:])
                for f in range(D):
                    xt = wp.tile([128, TPC], f32, tag=f"xt{f}")
                    nc.vector.tensor_mul(xt, xN[f], dinv)
                    nc.sync.dma_start(xt_out[f][:], xt[:])
            elif mode == "layer1":
                zb = []
                for f in range(D):
                    z = wp.tile([128, TPC], f32, tag=f"z{f}")
                    nc.vector.tensor_add(z, agg[f], xtN[f])
                    nc.vector.tensor_mul(z, z, dinvN)
                    zb_ = wp.tile([128, TPC], bf, tag=f"zb{f}")
                    nc.scalar.copy(zb_, z)
                    zb.append(zb_)
                # h layout [128, (t j)]: node-tile major, hidden-unit minor
                hA = accp.tile([128, TPC * HID], bf, tag="hA")
                hAv = hA.rearrange("p (t j) -> p t j", j=HID)
                nc.vector.tensor_tensor(
                    hAv,
                    zb[0].unsqueeze(2).broadcast_to([128, TPC, HID]),
                    w1r[0].unsqueeze(1).broadcast_to([128, TPC, HID]),
                    mybir.AluOpType.mult)
                hB = accp.tile([128, TPC * HID], bf, tag="hB")
                hBv = hB.rearrange("p (t j) -> p t j", j=HID)
                nc.gpsimd.tensor_tensor(
                    hBv,
                    zb[1].unsqueeze(2).broadcast_to([128, TPC, HID]),
                    w1r[1].unsqueeze(1).broadcast_to([128, TPC, HID]),
                    mybir.AluOpType.mult)
                nc.vector.tensor_add(hA, hA, hB)
                nc.gpsimd.tensor_tensor(
                    hAv, hAv,
                    b1_sb.unsqueeze(1).broadcast_to([128, TPC, HID]),
                    mybir.AluOpType.add)
                nc.scalar.activation(hA, hA, mybir.ActivationFunctionType.Relu)
                hv = accp.tile([128, TPC * HID], bf, tag="hv")
                hvv = hv.rearrange("p (t j) -> p t j", j=HID)
                nc.vector.tensor_tensor(
                    hvv, hAv,
                    w2_sb.unsqueeze(1).broadcast_to([128, TPC, HID]),
                    mybir.AluOpType.mult)
                v = wp.tile([128, TPC], f32, tag="v")
                nc.vector.tensor_reduce(v, hvv, mybir.AxisListType.X,
                                        mybir.AluOpType.add)
                vt = wp.tile([128, TPC], f32, tag="vt")
                nc.vector.tensor_mul(vt, v, dinvN)
                nc.sync.dma_start(vt_out[:], vt[:])
            else:
                y = wp.tile([128, TPC], f32, tag="y")
                nc.vector.tensor_add(y, agg[0], vtN)
                nc.vector.tensor_mul(y, y, dinvN)
                nc.vector.tensor_scalar(y, y, b2_sb[:, 0:1], None,
                                        mybir.AluOpType.add)
                nc.sync.dma_start(y_out[:], y[:])

        if reps > 1:
            with tc.For_i(0, reps, 1):
                body()
        else:
            body()

    _split_multi_waits(nc)
    return nc


def _rep_bf16(vec):
    return np.ascontiguousarray(
        np.tile(np.asarray(vec, np.float32).reshape(1, -1), (128, 1))
    ).astype(BF16)


def kernel(x, edge_index, edge_weight, W1, b1, W2, b2):
    x = np.asarray(x, np.float32)
    edge_index = np.asarray(edge_index)
    edge_weight = np.asarray(edge_weight, np.float32)
    W1 = np.asarray(W1, np.float32)
    b1 = np.asarray(b1, np.float32)
    W2 = np.asarray(W2, np.float32)
    b2 = np.asarray(b2, np.float32)

    pp = _preprocess(edge_index, edge_weight)
    CS = pp["CS"]
    order = pp["order"]

    ew_cs = _to_core_stream(pp["ew"], CS, BF16)

    xfull = np.zeros((NPAD, D), np.float32)
    xfull[:N] = x
    xnew = xfull[order]                          # newpos layout
    xn_cs = [_to_core_nodes(xnew[:, f], np.float32) for f in range(D)]

    # ---- NEFF 1: deg -> dinv, x*dinv ----
    nc1 = _build_sweep("deg", pp)
    in1 = [dict(ew=ew_cs[c], xn0=xn_cs[0][c], xn1=xn_cs[1][c])
           for c in range(NCORE)]
    r1 = run_bass_kernel_spmd(nc1, in1, core_ids=list(range(NCORE)))
    dinv_new = _from_core_nodes([r1.results[c]["dinvout"] for c in range(NCORE)])
    xt_new = [_from_core_nodes([r1.results[c][f"xtout{f}"] for c in range(NCORE)])
              for f in range(D)]

    # ---- host glue: per-edge (x*dinv)[src] streams ----
    xt_orig = np.empty((NPAD, D), np.float32)
    for f in range(D):
        xt_orig[order, f] = xt_new[f]
    xs_cs = [_to_core_stream(xt_orig[pp["src"], f], CS, BF16) for f in range(D)]
    dinv_n = _to_core_nodes(dinv_new, np.float32)
    xt_n = [_to_core_nodes(xt_new[f], np.float32) for f in range(D)]

    w1r = [_rep_bf16(W1[f]) for f in range(D)]
    b1b = _rep_bf16(b1)
    w2b = _rep_bf16(W2[:, 0])
    b2b = np.full((128, 1), float(b2[0]), np.float32)

    # ---- NEFF 2: layer 1 -> v*dinv ----
    nc2 = _build_sweep("layer1", pp)
    in2 = [dict(ew=ew_cs[c], xs0=xs_cs[0][c], xs1=xs_cs[1][c],
                dinvn=dinv_n[c], xtn0=xt_n[0][c], xtn1=xt_n[1][c],
                w1r0b=w1r[0], w1r1b=w1r[1], b1b=b1b, w2b=w2b)
           for c in range(NCORE)]
    r2 = run_bass_kernel_spmd(nc2, in2, core_ids=list(range(NCORE)))
    vt_new = _from_core_nodes([r2.results[c]["vtout"] for c in range(NCORE)])

    # ---- host glue: (v*dinv)[src] stream ----
    vt_orig = np.empty(NPAD, np.float32)
    vt_orig[order] = vt_new
    vs_cs = _to_core_stream(vt_orig[pp["src"]], CS, BF16)
    vt_n = _to_core_nodes(vt_new, np.float32)

    # ---- NEFF 3: layer 2 -> output ----
    nc3 = _build_sweep("layer2", pp)
    in3 = [dict(ew=ew_cs[c], vs=vs_cs[c], dinvn=dinv_n[c], vtn=vt_n[c],
                b2b=b2b) for c in range(NCORE)]
    r3 = run_bass_kernel_spmd(nc3, in3, core_ids=list(range(NCORE)))
    y_new = _from_core_nodes([r3.results[c]["yout"] for c in range(NCORE)])

    y_orig = np.empty(NPAD, np.float32)
    y_orig[order] = y_new
    return y_orig[:N, None].astype(np.float32)


# revision 4
# speedup vs baseline: 4.2084x; 2.1836x over previous
"""GCN (2-layer, PyG gcn_norm) on 8 Trainium2 NeuronCores via Bass.

Strategy (dst-partition-row sharding, no collectives, no PE):
  * Host sorts nodes by in-degree and assigns each node one SBUF
    partition-row of G slots (G = per-stripe max degree, rounded up), so
    the per-node segment-sum is a plain free-axis tensor_reduce — no
    one-hot masks, no matmuls.  Stripes of 1024 nodes (one 128-node tile
    per core) share a G schedule so all 8 SPMD cores run one program.
  * Per-edge streams are bf16 (tolerance is 2e-2); node-level arrays and
    accumulations stay f32.
  * Three sequential NEFF launches: (1) deg -> dinv, x*dinv, (2) layer-1
    aggregation -> h -> v*dinv, (3) layer-2 aggregation -> output.
    Between launches the host only gathers returned per-node arrays into
    per-edge streams (index-space data movement, no float math).
"""

import sys

sys.path.insert(0, "/opt/trn_rl_repo")

import numpy as np
import ml_dtypes

import concourse.bass as bass
import concourse.tile as tile
from concourse import mybir
from concourse.bass_utils import run_bass_kernel_spmd

BF16 = ml_dtypes.bfloat16

N = 100000
E = 3200000
D = 2
HID = 16
NCORE = 8
TPC = 98                      # stripes == node tiles per core
NPAD = TPC * 1024             # 100352
GMULT = 4                     # stripe slot width rounded up to this
BLK_COLS = 1024               # target stream columns per DMA block


def _split_multi_waits(nc):
    """This toolchain's walrus encodes at most one sync-wait per instruction.
    Hoist extra waits onto fresh single-wait NoOps placed just before."""
    ctr = 0
    for fn in nc.m.functions:
        for bb in fn.blocks:
            insts = list(bb.instructions)
            if not any(
                i.sync_info is not None and len(i.sync_info.on_wait or []) > 1
                for i in insts
            ):
                continue
            new = []
            for inst in insts:
                si = inst.sync_info
                if si is not None and len(si.on_wait or []) > 1:
                    waits = list(si.on_wait)
                    for w in waits[:-1]:
                        ctr += 1
                        new.append(
                            mybir.InstNoOp(
                                name=f"wsplit-{ctr}",
                                engine=inst.engine,
                                sync_info=mybir.SyncInfo(on_wait=[w], on_update=[]),
                                bass_nofuse=True,
                            )
                        )
                    si.on_wait = [waits[-1]]
                new.append(inst)
            bb.instructions = new
    return ctr


def _preprocess(edge_index, edge_weight):
    """Degree-sort nodes, assign each node a partition-row slot range, and
    scatter edge weight / src index into the per-core slot streams."""
    dst = edge_index[1].astype(np.int64)
    src = edge_index[0].astype(np.int64)
    ew = edge_weight.astype(np.float32)

    deg = np.bincount(dst, minlength=NPAD)
    order = np.argsort(deg, kind="stable")       # newpos -> orig id
    newpos = np.empty(NPAD, np.int64)
    newpos[order] = np.arange(NPAD)

    counts_new = deg[order]                      # per-newpos degree
    smax = counts_new.reshape(TPC, 1024).max(axis=1)
    G = np.maximum(GMULT, ((smax + GMULT - 1) // GMULT) * GMULT).astype(np.int64)
    offs = np.zeros(TPC + 1, np.int64)
    np.cumsum(G, out=offs[1:])
    CS = int(offs[-1])

    nd = newpos[dst]
    start = np.zeros(NPAD + 1, np.int64)
    np.cumsum(counts_new, out=start[1:])
    perm = np.argsort(nd, kind="stable")
    r = np.empty(E, np.int64)
    r[perm] = np.arange(E) - start[nd[perm]]     # rank of edge within its dst

    s = nd >> 10
    w = nd & 1023
    c = w >> 7
    p = w & 127
    flat = (c * 128 + p) * CS + offs[s] + r

    ew_flat = np.zeros(NCORE * 128 * CS, np.float32)
    src_flat = np.zeros(NCORE * 128 * CS, np.int64)
    ew_flat[flat] = ew
    src_flat[flat] = src

    # DMA blocks: consecutive stripes until >= BLK_COLS columns; per-block
    # runs of stripes sharing G (one tensor_reduce instruction per run).
    blocks = []
    t0, cols = 0, 0
    for t in range(TPC):
        cols += int(G[t])
        if cols >= BLK_COLS or t == TPC - 1:
            runs = []
            ro = 0
            for tt in range(t0, t + 1):
                g = int(G[tt])
                if runs and runs[-1][2] == g:
                    runs[-1] = (runs[-1][0], runs[-1][1] + 1, g, runs[-1][3])
                else:
                    runs.append((tt, 1, g, ro))
                ro += g
            blocks.append((t0, t + 1 - t0, int(offs[t0]), cols, runs))
            t0, cols = t + 1, 0

    return dict(G=G, offs=offs, CS=CS, blocks=blocks, order=order,
                ew=ew_flat, src=src_flat)


def _to_core_stream(arrflat, CS, dtype):
    a = arrflat.reshape(NCORE, 128, CS)
    return [np.ascontiguousarray(a[c]).astype(dtype) for c in range(NCORE)]


def _to_core_nodes(val_new, dtype):
    """[NPAD] array in newpos space -> per-core [128, TPC]
    (newpos = s*1024 + c*128 + p)."""
    a = val_new.reshape(TPC, NCORE, 128)
    return [np.ascontiguousarray(a[:, c, :].T).astype(dtype) for c in range(NCORE)]


def _from_core_nodes(parts):
    full = np.empty((TPC, NCORE, 128), np.float32)
    for c in range(NCORE):
        full[:, c, :] = np.asarray(parts[c], np.float32).T
    return full.reshape(NPAD)


def _build_sweep(mode, sched, reps=1):
    """Build the Bass program for one sweep. mode in {deg, layer1, layer2}.
    reps>1 wraps the body in a hardware For_i loop repeating the identical
    (idempotent) work — used only for timing measurements."""
    from contextlib import ExitStack

    CS = sched["CS"]
    blocks = sched["blocks"]
    BCMAX = max(b[3] for b in blocks)
    f32 = mybir.dt.float32
    bf = mybir.dt.bfloat16

    nc = bass.Bass("TRN2", target_bir_lowering=False, debug=False,
                   num_devices=NCORE)

    def din(name, shape, dtype=f32):
        return nc.dram_tensor(name, shape, dtype, kind="ExternalInput").ap()

    def dout(name, shape, dtype=f32):
        return nc.dram_tensor(name, shape, dtype, kind="ExternalOutput").ap()

    ew_d = din("ew", [128, CS], bf)
    if mode == "deg":
        xn_d = [din(f"xn{f}", [128, TPC]) for f in range(D)]
        dinv_out = dout("dinvout", [128, TPC])
        xt_out = [dout(f"xtout{f}", [128, TPC]) for f in range(D)]
    elif mode == "layer1":
        xs_d = [din(f"xs{f}", [128, CS], bf) for f in range(D)]
        dinv_d = din("dinvn", [128, TPC])
        xt_d = [din(f"xtn{f}", [128, TPC]) for f in range(D)]
        w1r_d = [din(f"w1r{f}b", [128, HID], bf) for f in range(D)]
        b1_d = din("b1b", [128, HID], bf)
        w2_d = din("w2b", [128, HID], bf)
        vt_out = dout("vtout", [128, TPC])
    else:
        vs_d = din("vs", [128, CS], bf)
        dinv_d = din("dinvn", [128, TPC])
        vt_d = din("vtn", [128, TPC])
        b2_d = din("b2b", [128, 1])
        y_out = dout("yout", [128, TPC])

    with tile.TileContext(nc) as tc, ExitStack() as ctx:
        const = ctx.enter_context(tc.tile_pool(name="const", bufs=1))
        sp = ctx.enter_context(tc.tile_pool(name="streams", bufs=3))
        wp = ctx.enter_context(tc.tile_pool(name="work", bufs=3))
        accp = ctx.enter_context(tc.tile_pool(name="acc", bufs=1))

        if mode == "deg":
            xN = []
            for f in range(D):
                t_ = const.tile([128, TPC], f32, tag=f"xn{f}")
                nc.sync.dma_start(t_[:], xn_d[f][:])
                xN.append(t_)
        elif mode == "layer1":
            dinvN = const.tile([128, TPC], f32)
            nc.sync.dma_start(dinvN[:], dinv_d[:])
            xtN = []
            for f in range(D):
                t_ = const.tile([128, TPC], f32, tag=f"xtn{f}")
                nc.sync.dma_start(t_[:], xt_d[f][:])
                xtN.append(t_)
            w1r = []
            for f in range(D):
                t_ = const.tile([128, HID], bf, tag=f"w1r{f}")
                nc.sync.dma_start(t_[:], w1r_d[f][:])
                w1r.append(t_)
            b1_sb = const.tile([128, HID], bf)
            nc.sync.dma_start(b1_sb[:], b1_d[:])
            w2_sb = const.tile([128, HID], bf)
            nc.sync.dma_start(w2_sb[:], w2_d[:])
        else:
            dinvN = const.tile([128, TPC], f32)
            nc.sync.dma_start(dinvN[:], dinv_d[:])
            vtN = const.tile([128, TPC], f32)
            nc.sync.dma_start(vtN[:], vt_d[:])
            b2_sb = const.tile([128, 1], f32)
            nc.sync.dma_start(b2_sb[:], b2_d[:])

        nF = D if mode == "layer1" else 1
        agg = [accp.tile([128, TPC], f32, tag=f"agg{f}", name=f"agg{f}")
               for f in range(nF)]

        def _reduce_runs(m_t, runs, dst_agg):
            for (tt, nt, g, ro) in runs:
                nc.vector.tensor_reduce(
                    dst_agg[:, tt:tt + nt],
                    m_t[:, ro:ro + nt * g].rearrange("p (t g) -> p t g", g=g),
                    mybir.AxisListType.X, mybir.AluOpType.add)

        def body():
            for (t0, ntb, c0, bc, runs) in blocks:
                ew_t = sp.tile([128, BCMAX], bf, tag="ew")
                nc.sync.dma_start(ew_t[:, 0:bc], ew_d[:, c0:c0 + bc])
                if mode == "deg":
                    _reduce_runs(ew_t, runs, agg[0])
                elif mode == "layer1":
                    xs_t = []
                    for f in range(D):
                        t_ = sp.tile([128, BCMAX], bf, tag=f"xs{f}")
                        nc.sync.dma_start(t_[:, 0:bc], xs_d[f][:, c0:c0 + bc])
                        xs_t.append(t_)
                    m0 = wp.tile([128, BCMAX], bf, tag="m0")
                    nc.vector.tensor_mul(m0[:, 0:bc], ew_t[:, 0:bc],
                                         xs_t[0][:, 0:bc])
                    m1 = wp.tile([128, BCMAX], bf, tag="m1")
                    nc.vector.tensor_mul(m1[:, 0:bc], ew_t[:, 0:bc],
                                         xs_t[1][:, 0:bc])
                    _reduce_runs(m0, runs, agg[0])
                    _reduce_runs(m1, runs, agg[1])
                else:
                    vs_t = sp.tile([128, BCMAX], bf, tag="vs")
                    nc.sync.dma_start(vs_t[:, 0:bc], vs_d[:, c0:c0 + bc])
                    m0 = wp.tile([128, BCMAX], bf, tag="m0")
                    nc.vector.tensor_mul(m0[:, 0:bc], ew_t[:, 0:bc],
                                         vs_t[:, 0:bc])
                    _reduce_runs(m0, runs, agg[0])

            # ---- epilogue ----
            if mode == "deg":
                degp = wp.tile([128, TPC], f32, tag="degp")
                nc.vector.tensor_scalar(degp, agg[0], 1.0, None,
                                        mybir.AluOpType.add)
                sq = wp.tile([128, TPC], f32, tag="sq")
                nc.scalar.activation(sq, degp, mybir.ActivationFunctionType.Sqrt)
                dinv = wp.tile([128, TPC], f32, tag="dinv")
                nc.vector.reciprocal(dinv, sq)
                nc.sync.dma_start(dinv_out[:], dinv[:])
                for f in range(D):
                    xt = wp.tile([128, TPC], f32, tag=f"xt{f}")
                    nc.vector.tensor_mul(xt, xN[f], dinv)
                    nc.sync.dma_start(xt_out[f][:], xt[:])
            elif mode == "layer1":
                zb = []
                for f in range(D):
                    z = wp.tile([128, TPC], f32, tag=f"z{f}")
                    nc.vector.tensor_add(z, agg[f], xtN[f])
                    nc.vector.tensor_mul(z, z, dinvN)
                    zb_ = wp.tile([128, TPC], bf, tag=f"zb{f}")
                    nc.scalar.copy(zb_, z)
                    zb.append(zb_)
                # h layout [128, (t j)]: node-tile major, hidden-unit minor
                hA = accp.tile([128, TPC * HID], bf, tag="hA")
                hAv = hA.rearrange("p (t j) -> p t j", j=HID)
                nc.vector.tensor_tensor(
                    hAv,
                    zb[0].unsqueeze(2).broadcast_to([128, TPC, HID]),
                    w1r[0].unsqueeze(1).broadcast_to([128, TPC, HID]),
                    mybir.AluOpType.mult)
                hB = accp.tile([128, TPC * HID], bf, tag="hB")
                hBv = hB.rearrange("p (t j) -> p t j", j=HID)
                nc.vector.tensor_tensor(
                    hBv,
                    zb[1].unsqueeze(2).broadcast_to([128, TPC, HID]),
                    w1r[1].unsqueeze(1).broadcast_to([128, TPC, HID]),
                    mybir.AluOpType.mult)
                nc.vector.tensor_add(hA, hA, hB)
                nc.vector.tensor_tensor(
                    hAv, hAv,
                    b1_sb.unsqueeze(1).broadcast_to([128, TPC, HID]),
                    mybir.AluOpType.add)
                nc.scalar.activation(hA, hA, mybir.ActivationFunctionType.Relu)
                hv = accp.tile([128, TPC * HID], bf, tag="hv")
                hvv = hv.rearrange("p (t j) -> p t j", j=HID)
                nc.vector.tensor_tensor(
                    hvv, hAv,
                    w2_sb.unsqueeze(1).broadcast_to([128, TPC, HID]),
                    mybir.AluOpType.mult)
                v = wp.tile([128, TPC], f32, tag="v")
                nc.vector.tensor_reduce(v, hvv, mybir.AxisListType.X,
                                        mybir.AluOpType.add)
                vt = wp.tile([128, TPC], f32, tag="vt")
                nc.vector.tensor_mul(vt, v, dinvN)
                nc.sync.dma_start(vt_out[:], vt[:])
            else:
                y = wp.tile([128, TPC], f32, tag="y")
                nc.vector.tensor_add(y, agg[0], vtN)
                nc.vector.tensor_mul(y, y, dinvN)
                nc.vector.tensor_scalar(y, y, b2_sb[:, 0:1], None,
                                        mybir.AluOpType.add)
                nc.sync.dma_start(y_out[:], y[:])

        if reps > 1:
            with tc.For_i(0, reps, 1):
                body()
        else:
            body()

    _split_multi_waits(nc)
    return nc


def _rep_bf16(vec):
    return np.ascontiguousarray(
        np.tile(np.asarray(vec, np.float32).reshape(1, -1), (128, 1))
    ).astype(BF16)


def kernel(x, edge_index, edge_weight, W1, b1, W2, b2):
    x = np.asarray(x, np.float32)
    edge_index = np.asarray(edge_index)
    edge_weight = np.asarray(edge_weight, np.float32)
    W1 = np.asarray(W1, np.float32)
    b1 = np.asarray(b1, np.float32)
    W2 = np.asarray(W2, np.float32)
    b2 = np.asarray(b2, np.float32)

    pp = _preprocess(edge_index, edge_weight)
    CS = pp["CS"]
    order = pp["order"]

    ew_cs = _to_core_stream(pp["ew"], CS, BF16)

    xfull = np.zeros((NPAD, D), np.float32)
    xfull[:N] = x
    xnew = xfull[order]                          # newpos layout
    xn_cs = [_to_core_nodes(xnew[:, f], np.float32) for f in range(D)]

    # ---- NEFF 1: deg -> dinv, x*dinv ----
    nc1 = _build_sweep("deg", pp)
    in1 = [dict(ew=ew_cs[c], xn0=xn_cs[0][c], xn1=xn_cs[1][c])
           for c in range(NCORE)]
    r1 = run_bass_kernel_spmd(nc1, in1, core_ids=list(range(NCORE)))
    dinv_new = _from_core_nodes([r1.results[c]["dinvout"] for c in range(NCORE)])
    xt_new = [_from_core_nodes([r1.results[c][f"xtout{f}"] for c in range(NCORE)])
              for f in range(D)]

    # ---- host glue: per-edge (x*dinv)[src] streams ----
    xt_orig = np.empty((NPAD, D), np.float32)
    for f in range(D):
        xt_orig[order, f] = xt_new[f]
    xs_cs = [_to_core_stream(xt_orig[pp["src"], f], CS, BF16) for f in range(D)]
    dinv_n = _to_core_nodes(dinv_new, np.float32)
    xt_n = [_to_core_nodes(xt_new[f], np.float32) for f in range(D)]

    w1r = [_rep_bf16(W1[f]) for f in range(D)]
    b1b = _rep_bf16(b1)
    w2b = _rep_bf16(W2[:, 0])
    b2b = np.full((128, 1), float(b2[0]), np.float32)

    # ---- NEFF 2: layer 1 -> v*dinv ----
    nc2 = _build_sweep("layer1", pp)
    in2 = [dict(ew=ew_cs[c], xs0=xs_cs[0][c], xs1=xs_cs[1][c],
                dinvn=dinv_n[c], xtn0=xt_n[0][c], xtn1=xt_n[1][c],
                w1r0b=w1r[0], w1r1b=w1r[1], b1b=b1b, w2b=w2b)
           for c in range(NCORE)]
    r2 = run_bass_kernel_spmd(nc2, in2, core_ids=list(range(NCORE)))
    vt_new = _from_core_nodes([r2.results[c]["vtout"] for c in range(NCORE)])

    # ---- host glue: (v*dinv)[src] stream ----
    vt_orig = np.empty(NPAD, np.float32)
    vt_orig[order] = vt_new
    vs_cs = _to_core_stream(vt_orig[pp["src"]], CS, BF16)
    vt_n = _to_core_nodes(vt_new, np.float32)

    # ---- NEFF 3: layer 2 -> output ----
    nc3 = _build_sweep("layer2", pp)
    in3 = [dict(ew=ew_cs[c], vs=vs_cs[c], dinvn=dinv_n[c], vtn=vt_n[c],
                b2b=b2b) for c in range(NCORE)]
    r3 = run_bass_kernel_spmd(nc3, in3, core_ids=list(range(NCORE)))
    y_new = _from_core_nodes([r3.results[c]["yout"] for c in range(NCORE)])

    y_orig = np.empty(NPAD, np.float32)
    y_orig[order] = y_new
    return y_orig[:N, None].astype(np.float32)


# revision 11
# speedup vs baseline: 43.3364x; 10.2977x over previous
"""GCN (2-layer, PyG gcn_norm) on 8 Trainium2 NeuronCores via Bass.

Strategy (dst-partition-row sharding, no collectives, no PE):
  * Host appends self-loop edges (weight 1, as in gcn_norm), sorts nodes
    by in-degree and assigns each node one SBUF partition-row of G slots
    (G = per-stripe max degree, rounded up), so the per-node segment-sum
    is a plain free-axis tensor_reduce — no one-hot masks, no matmuls.
    Stripes of 1024 nodes (one 128-node tile per core) share a G schedule
    so all 8 SPMD cores run one program.
  * Per-edge streams are bf16 (tolerance is 2e-2); accumulations stay
    f32.  Streams are stored per DMA-block so every transfer is fully
    contiguous, issued alternately on the SP and Activation DGE queues.
  * Three sequential NEFF launches: (1) deg -> dinv, x*dinv, (2) layer-1
    aggregation -> h -> v*dinv, (3) layer-2 aggregation -> output.
    Between launches the host only gathers returned per-node arrays into
    per-edge streams (index-space data movement, no float math).
"""

import sys

sys.path.insert(0, "/opt/trn_rl_repo")

import numpy as np
import ml_dtypes

import concourse.bass as bass
import concourse.tile as tile
from concourse import mybir
from concourse.bass_utils import run_bass_kernel_spmd

BF16 = ml_dtypes.bfloat16

N = 100000
E = 3200000
D = 2
HID = 16
NCORE = 8
TPC = 98                      # stripes == node tiles per core
NPAD = TPC * 1024             # 100352
GMULT = 4                     # stripe slot width rounded up to this
BLK_COLS = 1024               # target stream columns per DMA block


def _split_multi_waits(nc):
    """This toolchain's walrus encodes at most one sync-wait per instruction.
    Hoist extra waits onto fresh single-wait NoOps placed just before."""
    ctr = 0
    for fn in nc.m.functions:
        for bb in fn.blocks:
            insts = list(bb.instructions)
            if not any(
                i.sync_info is not None and len(i.sync_info.on_wait or []) > 1
                for i in insts
            ):
                continue
            new = []
            for inst in insts:
                si = inst.sync_info
                if si is not None and len(si.on_wait or []) > 1:
                    waits = list(si.on_wait)
                    for w in waits[:-1]:
                        ctr += 1
                        new.append(
                            mybir.InstNoOp(
                                name=f"wsplit-{ctr}",
                                engine=inst.engine,
                                sync_info=mybir.SyncInfo(on_wait=[w], on_update=[]),
                                bass_nofuse=True,
                            )
                        )
                    si.on_wait = [waits[-1]]
                new.append(inst)
            bb.instructions = new
    return ctr


def _preprocess(edge_index, edge_weight):
    """Append self-loops, degree-sort nodes, assign each node a
    partition-row slot range, and scatter edge weight / src index into the
    per-core slot streams."""
    loop = np.arange(N, dtype=np.int64)
    dst = np.concatenate([edge_index[1].astype(np.int64), loop])
    src = np.concatenate([edge_index[0].astype(np.int64), loop])
    ew = np.concatenate([edge_weight.astype(np.float32),
                         np.ones(N, np.float32)])
    ne = len(dst)

    deg = np.bincount(dst, minlength=NPAD)
    order = np.argsort(deg, kind="stable")       # newpos -> orig id
    newpos = np.empty(NPAD, np.int64)
    newpos[order] = np.arange(NPAD)

    counts_new = deg[order]                      # per-newpos degree
    smax = counts_new.reshape(TPC, 1024).max(axis=1)
    G = np.maximum(GMULT, ((smax + GMULT - 1) // GMULT) * GMULT).astype(np.int64)
    offs = np.zeros(TPC + 1, np.int64)
    np.cumsum(G, out=offs[1:])
    CS = int(offs[-1])

    nd = newpos[dst]
    start = np.zeros(NPAD + 1, np.int64)
    np.cumsum(counts_new, out=start[1:])
    perm = np.argsort(nd, kind="stable")
    r = np.empty(ne, np.int64)
    r[perm] = np.arange(ne) - start[nd[perm]]    # rank of edge within its dst

    s = nd >> 10
    w = nd & 1023
    c = w >> 7
    p = w & 127
    flat = (c * 128 + p) * CS + offs[s] + r

    ew_flat = np.zeros(NCORE * 128 * CS, np.float32)
    src_flat = np.zeros(NCORE * 128 * CS, np.int64)
    ew_flat[flat] = ew
    src_flat[flat] = src

    # DMA blocks: consecutive stripes until >= BLK_COLS columns; per-block
    # runs of stripes sharing G (one tensor_reduce instruction per run).
    blocks = []
    t0, cols = 0, 0
    for t in range(TPC):
        cols += int(G[t])
        if cols >= BLK_COLS or t == TPC - 1:
            runs = []
            ro = 0
            for tt in range(t0, t + 1):
                g = int(G[tt])
                if runs and runs[-1][2] == g:
                    runs[-1] = (runs[-1][0], runs[-1][1] + 1, g, runs[-1][3])
                else:
                    runs.append((tt, 1, g, ro))
                ro += g
            blocks.append((t0, t + 1 - t0, int(offs[t0]), cols, runs))
            t0, cols = t + 1, 0

    return dict(G=G, offs=offs, CS=CS, blocks=blocks, order=order,
                ew=ew_flat, src=src_flat)


def _stream_blocks(sched, arrflat, prefix, dtype):
    """Per-core dicts of per-DMA-block contiguous stream arrays."""
    CS = sched["CS"]
    a = arrflat.reshape(NCORE, 128, CS)
    out = []
    for c in range(NCORE):
        d = {}
        for bi, (t0, ntb, c0, bc, runs) in enumerate(sched["blocks"]):
            d[f"{prefix}{bi}"] = np.ascontiguousarray(
                a[c, :, c0:c0 + bc]).astype(dtype)
        out.append(d)
    return out


def _to_core_nodes(val_new, dtype):
    """[NPAD] array in newpos space -> per-core [128, TPC]
    (newpos = s*1024 + c*128 + p)."""
    a = val_new.reshape(TPC, NCORE, 128)
    return [np.ascontiguousarray(a[:, c, :].T).astype(dtype) for c in range(NCORE)]


def _from_core_nodes(parts):
    full = np.empty((TPC, NCORE, 128), np.float32)
    for c in range(NCORE):
        full[:, c, :] = np.asarray(parts[c], np.float32).T
    return full.reshape(NPAD)


def _build_sweep(mode, sched, reps=1, variant=None, unroll=16,
                 skip_b1=True, skip_b2=True):
    """Build the Bass program for one sweep. mode in {deg, layer1, layer2}.
    reps>1 wraps `reps` copies of the (idempotent) body in a hardware For_i
    loop, `unroll` bodies per trip — used only for timing measurements.
    variant (timing experiments only): 'dmaonly' = stream DMA without
    compute, 'reduceonly' = compute without stream DMA."""
    from contextlib import ExitStack

    CS = sched["CS"]
    blocks = sched["blocks"]
    BCMAX = max(b[3] for b in blocks)
    f32 = mybir.dt.float32
    bf = mybir.dt.bfloat16

    nc = bass.Bass("TRN2", target_bir_lowering=False, debug=False,
                   num_devices=NCORE)

    def din(name, shape, dtype=f32):
        return nc.dram_tensor(name, shape, dtype, kind="ExternalInput").ap()

    def dout(name, shape, dtype=f32):
        return nc.dram_tensor(name, shape, dtype, kind="ExternalOutput").ap()

    def din_blocks(prefix):
        return [din(f"{prefix}{bi}", [128, b[3]], bf)
                for bi, b in enumerate(blocks)]

    ew_d = din_blocks("ew")
    if mode == "deg":
        xn_d = [din(f"xn{f}", [128, TPC]) for f in range(D)]
        dinv_out = dout("dinvout", [128, TPC])
        xt_out = [dout(f"xtout{f}", [128, TPC]) for f in range(D)]
    elif mode == "layer1":
        xs_d = [din_blocks(f"xs{f}_") for f in range(D)]
        dinv_d = din("dinvn", [128, TPC])
        w1r_d = [din(f"w1r{f}b", [128, HID], bf) for f in range(D)]
        b1_d = din("b1b", [128, HID], bf)
        w2_d = din("w2b", [128, HID], bf)
        vt_out = dout("vtout", [128, TPC])
    else:
        vs_d = din_blocks("vs")
        dinv_d = din("dinvn", [128, TPC])
        b2_d = din("b2b", [128, 1])
        y_out = dout("yout", [128, TPC])

    with tile.TileContext(nc) as tc, ExitStack() as ctx:
        const = ctx.enter_context(tc.tile_pool(name="const", bufs=1))
        sp = ctx.enter_context(tc.tile_pool(name="streams", bufs=3))
        wp = ctx.enter_context(tc.tile_pool(name="work", bufs=3))
        accp = ctx.enter_context(tc.tile_pool(name="acc", bufs=2))

        if mode == "deg":
            xN = []
            for f in range(D):
                t_ = const.tile([128, TPC], f32, tag=f"xn{f}")
                nc.sync.dma_start(t_[:], xn_d[f][:])
                xN.append(t_)
        elif mode == "layer1":
            dinvN = const.tile([128, TPC], f32)
            nc.sync.dma_start(dinvN[:], dinv_d[:])
            w1r = []
            for f in range(D):
                t_ = const.tile([128, HID], bf, tag=f"w1r{f}")
                nc.sync.dma_start(t_[:], w1r_d[f][:])
                w1r.append(t_)
            b1_sb = const.tile([128, HID], bf)
            nc.sync.dma_start(b1_sb[:], b1_d[:])
            w2_sb = const.tile([128, HID], bf)
            nc.sync.dma_start(w2_sb[:], w2_d[:])
        else:
            dinvN = const.tile([128, TPC], f32)
            nc.sync.dma_start(dinvN[:], dinv_d[:])
            b2_sb = const.tile([128, 1], f32)
            nc.sync.dma_start(b2_sb[:], b2_d[:])

        nF = D if mode == "layer1" else 1
        agg = [accp.tile([128, TPC], f32, tag=f"agg{f}", name=f"agg{f}")
               for f in range(nF)]

        def _reduce_runs(m_t, runs, dst_agg):
            for (tt, nt, g, ro) in runs:
                nc.vector.tensor_reduce(
                    dst_agg[:, tt:tt + nt],
                    m_t[:, ro:ro + nt * g].rearrange("p (t g) -> p t g", g=g),
                    mybir.AxisListType.X, mybir.AluOpType.add)

        def body():
            for bi, (t0, ntb, c0, bc, runs) in enumerate(blocks):
                ew_t = sp.tile([128, BCMAX], bf, tag="ew")
                if variant != "reduceonly":
                    nc.sync.dma_start(ew_t[:, 0:bc], ew_d[bi][:])
                else:
                    nc.vector.memset(ew_t[:, 0:1], 0.0)
                if variant == "dmaonly":
                    continue
                if mode == "deg":
                    _reduce_runs(ew_t, runs, agg[0])
                elif mode == "layer1":
                    xs_t = []
                    for f in range(D):
                        t_ = sp.tile([128, BCMAX], bf, tag=f"xs{f}")
                        if variant != "reduceonly":
                            nc.scalar.dma_start(t_[:, 0:bc], xs_d[f][bi][:])
                        else:
                            nc.vector.memset(t_[:, 0:1], 0.0)
                        xs_t.append(t_)
                    m0 = wp.tile([128, BCMAX], bf, tag="m0")
                    nc.vector.tensor_mul(m0[:, 0:bc], ew_t[:, 0:bc],
                                         xs_t[0][:, 0:bc])
                    m1 = wp.tile([128, BCMAX], bf, tag="m1")
                    nc.vector.tensor_mul(m1[:, 0:bc], ew_t[:, 0:bc],
                                         xs_t[1][:, 0:bc])
                    _reduce_runs(m0, runs, agg[0])
                    _reduce_runs(m1, runs, agg[1])
                else:
                    vs_t = sp.tile([128, BCMAX], bf, tag="vs")
                    if variant != "reduceonly":
                        nc.scalar.dma_start(vs_t[:, 0:bc], vs_d[bi][:])
                    else:
                        nc.vector.memset(vs_t[:, 0:1], 0.0)
                    m0 = wp.tile([128, BCMAX], bf, tag="m0")
                    nc.vector.tensor_mul(m0[:, 0:bc], ew_t[:, 0:bc],
                                         vs_t[:, 0:bc])
                    _reduce_runs(m0, runs, agg[0])

            if variant in ("dmaonly", "reduceonly"):
                return
            # ---- epilogue (self-loop slots make agg complete: deg sweep
            # yields deg+1; layer sweeps include the dinv*val self term) ----
            if mode == "deg":
                sq = wp.tile([128, TPC], f32, tag="sq")
                nc.scalar.activation(sq, agg[0],
                                     mybir.ActivationFunctionType.Sqrt)
                dinv = wp.tile([128, TPC], f32, tag="dinv")
                nc.vector.reciprocal(dinv, sq)
                nc.sync.dma_start(dinv_out[:], dinv[:])
                for f in range(D):
                    xt = wp.tile([128, TPC], f32, tag=f"xt{f}")
                    nc.vector.tensor_mul(xt, xN[f], dinv)
                    nc.sync.dma_start(xt_out[f][:], xt[:])
            elif mode == "layer1":
                zb = []
                for f in range(D):
                    z = wp.tile([128, TPC], f32, tag=f"z{f}")
                    nc.vector.tensor_mul(z, agg[f], dinvN)
                    zb_ = wp.tile([128, TPC], bf, tag=f"zb{f}")
                    nc.scalar.copy(zb_, z)
                    zb.append(zb_)
                # h layout [128, (t j)]: node-tile major, hidden-unit minor
                hA = wp.tile([128, TPC * HID], bf, tag="hA")
                hAv = hA.rearrange("p (t j) -> p t j", j=HID)
                nc.vector.tensor_tensor(
                    hAv,
                    zb[0].unsqueeze(2).broadcast_to([128, TPC, HID]),
                    w1r[0].unsqueeze(1).broadcast_to([128, TPC, HID]),
                    mybir.AluOpType.mult)
                hB = wp.tile([128, TPC * HID], bf, tag="hB")
                hBv = hB.rearrange("p (t j) -> p t j", j=HID)
                nc.vector.tensor_tensor(
                    hBv,
                    zb[1].unsqueeze(2).broadcast_to([128, TPC, HID]),
                    w1r[1].unsqueeze(1).broadcast_to([128, TPC, HID]),
                    mybir.AluOpType.mult)
                nc.vector.tensor_add(hA, hA, hB)
                if not skip_b1:
                    nc.vector.tensor_tensor(
                        hAv, hAv,
                        b1_sb.unsqueeze(1).broadcast_to([128, TPC, HID]),
                        mybir.AluOpType.add)
                nc.scalar.activation(hA, hA, mybir.ActivationFunctionType.Relu)
                hv = wp.tile([128, TPC * HID], bf, tag="hv")
                hvv = hv.rearrange("p (t j) -> p t j", j=HID)
                nc.vector.tensor_tensor(
                    hvv, hAv,
                    w2_sb.unsqueeze(1).broadcast_to([128, TPC, HID]),
                    mybir.AluOpType.mult)
                v = wp.tile([128, TPC], f32, tag="v")
                nc.vector.tensor_reduce(v, hvv, mybir.AxisListType.X,
                                        mybir.AluOpType.add)
                vt = wp.tile([128, TPC], f32, tag="vt")
                nc.vector.tensor_mul(vt, v, dinvN)
                nc.sync.dma_start(vt_out[:], vt[:])
            else:
                y = wp.tile([128, TPC], f32, tag="y")
                nc.vector.tensor_mul(y, agg[0], dinvN)
                if not skip_b2:
                    nc.vector.tensor_scalar(y, y, b2_sb[:, 0:1], None,
                                            mybir.AluOpType.add)
                nc.sync.dma_start(y_out[:], y[:])

        if reps > 1:
            assert reps % unroll == 0
            with tc.For_i(0, reps // unroll, 1):
                for _ in range(unroll):
                    body()
        else:
            body()

    _split_multi_waits(nc)
    return nc


def _rep_bf16(vec):
    return np.ascontiguousarray(
        np.tile(np.asarray(vec, np.float32).reshape(1, -1), (128, 1))
    ).astype(BF16)


def kernel(x, edge_index, edge_weight, W1, b1, W2, b2):
    x = np.asarray(x, np.float32)
    edge_index = np.asarray(edge_index)
    edge_weight = np.asarray(edge_weight, np.float32)
    W1 = np.asarray(W1, np.float32)
    b1 = np.asarray(b1, np.float32)
    W2 = np.asarray(W2, np.float32)
    b2 = np.asarray(b2, np.float32)
    skip_b1 = not np.any(b1 != 0)
    skip_b2 = not np.any(b2 != 0)

    pp = _preprocess(edge_index, edge_weight)
    order = pp["order"]

    ew_cs = _stream_blocks(pp, pp["ew"], "ew", BF16)

    xfull = np.zeros((NPAD, D), np.float32)
    xfull[:N] = x
    xnew = xfull[order]                          # newpos layout
    xn_cs = [_to_core_nodes(xnew[:, f], np.float32) for f in range(D)]

    # ---- NEFF 1: deg+1 -> dinv, x*dinv ----
    nc1 = _build_sweep("deg", pp)
    in1 = [dict(ew_cs[c], xn0=xn_cs[0][c], xn1=xn_cs[1][c])
           for c in range(NCORE)]
    r1 = run_bass_kernel_spmd(nc1, in1, core_ids=list(range(NCORE)))
    dinv_new = _from_core_nodes([r1.results[c]["dinvout"] for c in range(NCORE)])
    xt_new = [_from_core_nodes([r1.results[c][f"xtout{f}"] for c in range(NCORE)])
              for f in range(D)]

    # ---- host glue: per-edge (x*dinv)[src] streams ----
    xt_orig = np.empty((NPAD, D), np.float32)
    for f in range(D):
        xt_orig[order, f] = xt_new[f]
    xs_cs = [_stream_blocks(pp, xt_orig[pp["src"], f], f"xs{f}_", BF16)
             for f in range(D)]
    dinv_n = _to_core_nodes(dinv_new, np.float32)

    w1r = [_rep_bf16(W1[f]) for f in range(D)]
    b1b = _rep_bf16(b1)
    w2b = _rep_bf16(W2[:, 0])
    b2b = np.full((128, 1), float(b2[0]), np.float32)

    # ---- NEFF 2: layer 1 -> v*dinv ----
    nc2 = _build_sweep("layer1", pp, skip_b1=skip_b1, skip_b2=skip_b2)
    in2 = [dict(ew_cs[c], **xs_cs[0][c], **xs_cs[1][c],
                dinvn=dinv_n[c], w1r0b=w1r[0], w1r1b=w1r[1], b1b=b1b,
                w2b=w2b) for c in range(NCORE)]
    r2 = run_bass_kernel_spmd(nc2, in2, core_ids=list(range(NCORE)))
    vt_new = _from_core_nodes([r2.results[c]["vtout"] for c in range(NCORE)])

    # ---- host glue: (v*dinv)[src] stream ----
    vt_orig = np.empty(NPAD, np.float32)
    vt_orig[order] = vt_new
    vs_cs = _stream_blocks(pp, vt_orig[pp["src"]], "vs", BF16)
    dinv_n2 = dinv_n

    # ---- NEFF 3: layer 2 -> output ----
    nc3 = _build_sweep("layer2", pp, skip_b1=skip_b1, skip_b2=skip_b2)
    in3 = [dict(ew_cs[c], **vs_cs[c], dinvn=dinv_n2[c], b2b=b2b)
           for c in range(NCORE)]
    r3 = run_bass_kernel_spmd(nc3, in3, core_ids=list(range(NCORE)))
    y_new = _from_core_nodes([r3.results[c]["yout"] for c in range(NCORE)])

    y_orig = np.empty(NPAD, np.float32)
    y_orig[order] = y_new
    return y_orig[:N, None].astype(np.float32)


# revision 17
# speedup vs baseline: 46.0246x; 1.0620x over previous
"""GCN (2-layer, PyG gcn_norm) on 8 Trainium2 NeuronCores via Bass.

Strategy (dst-partition-row sharding, no collectives, no PE):
  * Host appends self-loop edges (weight 1, as in gcn_norm), sorts nodes
    by in-degree and assigns each node one SBUF partition-row of G slots
    (G = per-stripe max degree, rounded up), so the per-node segment-sum
    is a plain free-axis tensor_reduce — no one-hot masks, no matmuls.
    Stripes of 1024 nodes (one 128-node tile per core) share a G schedule
    so all 8 SPMD cores run one program.
  * Per-edge streams are bf16 (tolerance is 2e-2); accumulations stay
    f32.  Streams are stored per DMA-block so every transfer is fully
    contiguous, issued alternately on the SP and Activation DGE queues.
  * Three sequential NEFF launches: (1) deg -> dinv, x*dinv, (2) layer-1
    aggregation -> h -> v*dinv, (3) layer-2 aggregation -> output.
    Between launches the host only gathers returned per-node arrays into
    per-edge streams (index-space data movement, no float math).
"""

import sys

sys.path.insert(0, "/opt/trn_rl_repo")

import numpy as np
import ml_dtypes

import concourse.bass as bass
import concourse.tile as tile
from concourse import mybir
from concourse.bass_utils import run_bass_kernel_spmd

BF16 = ml_dtypes.bfloat16

N = 100000
E = 3200000
D = 2
HID = 16
NCORE = 8
TPC = 98                      # stripes == node tiles per core
NPAD = TPC * 1024             # 100352
GMULT = 8                     # stripe slot width rounded up to this
BLK_COLS = 1024               # target stream columns per DMA block


def _split_multi_waits(nc):
    """This toolchain's walrus encodes at most one sync-wait per instruction.
    Hoist extra waits onto fresh single-wait NoOps placed just before."""
    ctr = 0
    for fn in nc.m.functions:
        for bb in fn.blocks:
            insts = list(bb.instructions)
            if not any(
                i.sync_info is not None and len(i.sync_info.on_wait or []) > 1
                for i in insts
            ):
                continue
            new = []
            for inst in insts:
                si = inst.sync_info
                if si is not None and len(si.on_wait or []) > 1:
                    waits = list(si.on_wait)
                    for w in waits[:-1]:
                        ctr += 1
                        new.append(
                            mybir.InstNoOp(
                                name=f"wsplit-{ctr}",
                                engine=inst.engine,
                                sync_info=mybir.SyncInfo(on_wait=[w], on_update=[]),
                                bass_nofuse=True,
                            )
                        )
                    si.on_wait = [waits[-1]]
                new.append(inst)
            bb.instructions = new
    return ctr


def _preprocess(edge_index, edge_weight):
    """Append self-loops, degree-sort nodes, assign each node a
    partition-row slot range, and scatter edge weight / src index into the
    per-core slot streams."""
    loop = np.arange(N, dtype=np.int64)
    dst = np.concatenate([edge_index[1].astype(np.int64), loop])
    src = np.concatenate([edge_index[0].astype(np.int64), loop])
    ew = np.concatenate([edge_weight.astype(np.float32),
                         np.ones(N, np.float32)])
    ne = len(dst)

    deg = np.bincount(dst, minlength=NPAD)
    order = np.argsort(deg, kind="stable")       # newpos -> orig id
    newpos = np.empty(NPAD, np.int64)
    newpos[order] = np.arange(NPAD)

    counts_new = deg[order]                      # per-newpos degree
    smax = counts_new.reshape(TPC, 1024).max(axis=1)
    G = np.maximum(GMULT, ((smax + GMULT - 1) // GMULT) * GMULT).astype(np.int64)
    offs = np.zeros(TPC + 1, np.int64)
    np.cumsum(G, out=offs[1:])
    CS = int(offs[-1])

    nd = newpos[dst]
    start = np.zeros(NPAD + 1, np.int64)
    np.cumsum(counts_new, out=start[1:])
    perm = np.argsort(nd, kind="stable")
    r = np.empty(ne, np.int64)
    r[perm] = np.arange(ne) - start[nd[perm]]    # rank of edge within its dst

    s = nd >> 10
    w = nd & 1023
    c = w >> 7
    p = w & 127
    flat = (c * 128 + p) * CS + offs[s] + r

    ew_flat = np.zeros(NCORE * 128 * CS, np.float32)
    src_flat = np.zeros(NCORE * 128 * CS, np.int64)
    ew_flat[flat] = ew
    src_flat[flat] = src

    # DMA blocks: consecutive stripes until >= BLK_COLS columns; per-block
    # runs of stripes sharing G (one tensor_reduce instruction per run).
    blocks = []
    t0, cols = 0, 0
    for t in range(TPC):
        cols += int(G[t])
        if cols >= BLK_COLS or t == TPC - 1:
            runs = []
            ro = 0
            for tt in range(t0, t + 1):
                g = int(G[tt])
                if runs and runs[-1][2] == g:
                    runs[-1] = (runs[-1][0], runs[-1][1] + 1, g, runs[-1][3])
                else:
                    runs.append((tt, 1, g, ro))
                ro += g
            blocks.append((t0, t + 1 - t0, int(offs[t0]), cols, runs))
            t0, cols = t + 1, 0

    return dict(G=G, offs=offs, CS=CS, blocks=blocks, order=order,
                ew=ew_flat, src=src_flat)


def _stream_blocks(sched, arrflat, prefix, dtype):
    """Per-core dicts of per-DMA-block contiguous stream arrays."""
    CS = sched["CS"]
    a = arrflat.reshape(NCORE, 128, CS)
    out = []
    for c in range(NCORE):
        d = {}
        for bi, (t0, ntb, c0, bc, runs) in enumerate(sched["blocks"]):
            d[f"{prefix}{bi}"] = np.ascontiguousarray(
                a[c, :, c0:c0 + bc]).astype(dtype)
        out.append(d)
    return out


def _to_core_nodes(val_new, dtype):
    """[NPAD] array in newpos space -> per-core [128, TPC]
    (newpos = s*1024 + c*128 + p)."""
    a = val_new.reshape(TPC, NCORE, 128)
    return [np.ascontiguousarray(a[:, c, :].T).astype(dtype) for c in range(NCORE)]


def _from_core_nodes(parts):
    full = np.empty((TPC, NCORE, 128), np.float32)
    for c in range(NCORE):
        full[:, c, :] = np.asarray(parts[c], np.float32).T
    return full.reshape(NPAD)


def _build_sweep(mode, sched, reps=1, variant=None, unroll=16,
                 skip_b1=True, skip_b2=True):
    """Build the Bass program for one sweep. mode in {deg, layer1, layer2}.
    reps>1 wraps `reps` copies of the (idempotent) body in a hardware For_i
    loop, `unroll` bodies per trip — used only for timing measurements.
    variant (timing experiments only): 'dmaonly' = stream DMA without
    compute, 'reduceonly' = compute without stream DMA."""
    from contextlib import ExitStack

    CS = sched["CS"]
    blocks = sched["blocks"]
    BCMAX = max(b[3] for b in blocks)
    f32 = mybir.dt.float32
    bf = mybir.dt.bfloat16

    nc = bass.Bass("TRN2", target_bir_lowering=False, debug=False,
                   num_devices=NCORE)

    def din(name, shape, dtype=f32):
        return nc.dram_tensor(name, shape, dtype, kind="ExternalInput").ap()

    def dout(name, shape, dtype=f32):
        return nc.dram_tensor(name, shape, dtype, kind="ExternalOutput").ap()

    def din_blocks(prefix):
        return [din(f"{prefix}{bi}", [128, b[3]], bf)
                for bi, b in enumerate(blocks)]

    ew_d = din_blocks("ew")
    if mode == "deg":
        xn_d = [din(f"xn{f}", [128, TPC]) for f in range(D)]
        dinv_out = dout("dinvout", [128, TPC])
        xt_out = [dout(f"xtout{f}", [128, TPC]) for f in range(D)]
    elif mode == "layer1":
        xs_d = [din_blocks(f"xs{f}_") for f in range(D)]
        dinv_d = din("dinvn", [128, TPC])
        w1x_d = [din(f"w1x{f}b", [128, HID * TPC], bf) for f in range(D)]
        b1_d = din("b1b", [128, HID], bf)
        w2x_d = din("w2xb", [128, HID * TPC], bf)
        vt_out = dout("vtout", [128, TPC])
    else:
        vs_d = din_blocks("vs")
        dinv_d = din("dinvn", [128, TPC])
        b2_d = din("b2b", [128, 1])
        y_out = dout("yout", [128, TPC])

    with tile.TileContext(nc) as tc, ExitStack() as ctx:
        const = ctx.enter_context(tc.tile_pool(name="const", bufs=1))
        sp = ctx.enter_context(tc.tile_pool(name="streams", bufs=3))
        wp = ctx.enter_context(tc.tile_pool(name="work", bufs=3))
        accp = ctx.enter_context(tc.tile_pool(name="acc", bufs=2))

        if mode == "deg":
            xN = []
            for f in range(D):
                t_ = const.tile([128, TPC], f32, tag=f"xn{f}")
                nc.sync.dma_start(t_[:], xn_d[f][:])
                xN.append(t_)
        elif mode == "layer1":
            dinvN = const.tile([128, TPC], f32)
            nc.sync.dma_start(dinvN[:], dinv_d[:])
            w1x = []
            for f in range(D):
                t_ = const.tile([128, HID * TPC], bf, tag=f"w1x{f}")
                nc.sync.dma_start(t_[:], w1x_d[f][:])
                w1x.append(t_)
            b1_sb = const.tile([128, HID], bf)
            nc.sync.dma_start(b1_sb[:], b1_d[:])
            w2x_sb = const.tile([128, HID * TPC], bf)
            nc.sync.dma_start(w2x_sb[:], w2x_d[:])
        else:
            dinvN = const.tile([128, TPC], f32)
            nc.sync.dma_start(dinvN[:], dinv_d[:])
            b2_sb = const.tile([128, 1], f32)
            nc.sync.dma_start(b2_sb[:], b2_d[:])

        nF = D if mode == "layer1" else 1

        def _reduce_2stage(m_t, bc, runs, dst_agg, s1tag):
            # stage 1: one bf16 2x-rate instruction per block summing 8-slot
            # sub-chunks; stage 2: f32 per-run reduce of the partials.
            s1 = wp.tile([128, BCMAX // 8], bf, tag=s1tag)
            with nc.allow_low_precision("stage-1 partial sums of 8 bf16 terms"):
                nc.vector.tensor_reduce(
                    s1[:, 0:bc // 8],
                    m_t[:, 0:bc].rearrange("p (q g) -> p q g", g=8),
                    mybir.AxisListType.X, mybir.AluOpType.add)
            for (tt, nt, g, ro) in runs:
                nc.vector.tensor_reduce(
                    dst_agg[:, tt:tt + nt],
                    s1[:, ro // 8:ro // 8 + nt * (g // 8)].rearrange(
                        "p (t q) -> p t q", q=g // 8),
                    mybir.AxisListType.X, mybir.AluOpType.add)

        def body():
            agg = [accp.tile([128, TPC], f32, tag=f"agg{f}", name=f"agg{f}")
                   for f in range(nF)]
            for bi, (t0, ntb, c0, bc, runs) in enumerate(blocks):
                qa = nc.sync if bi % 2 == 0 else nc.scalar
                qb = nc.scalar if bi % 2 == 0 else nc.sync
                ew_t = sp.tile([128, BCMAX], bf, tag="ew")
                if variant != "reduceonly":
                    qa.dma_start(ew_t[:, 0:bc], ew_d[bi][:])
                else:
                    qa.dma_start(ew_t[:, 0:4], ew_d[bi][:, 0:4])
                if variant == "dmaonly":
                    continue
                if mode == "deg":
                    _reduce_2stage(ew_t, bc, runs, agg[0], "s1a")
                elif mode == "layer1":
                    xs_t = []
                    for f in range(D):
                        qf = qb if f == 0 else qa
                        t_ = sp.tile([128, BCMAX], bf, tag=f"xs{f}")
                        if variant != "reduceonly":
                            qf.dma_start(t_[:, 0:bc], xs_d[f][bi][:])
                        else:
                            qf.dma_start(t_[:, 0:4], xs_d[f][bi][:, 0:4])
                        xs_t.append(t_)
                    m0 = wp.tile([128, BCMAX], bf, tag="m0")
                    nc.vector.tensor_mul(m0[:, 0:bc], ew_t[:, 0:bc],
                                         xs_t[0][:, 0:bc])
                    m1 = wp.tile([128, BCMAX], bf, tag="m1")
                    nc.vector.tensor_mul(m1[:, 0:bc], ew_t[:, 0:bc],
                                         xs_t[1][:, 0:bc])
                    _reduce_2stage(m0, bc, runs, agg[0], "s1a")
                    _reduce_2stage(m1, bc, runs, agg[1], "s1b")
                else:
                    vs_t = sp.tile([128, BCMAX], bf, tag="vs")
                    if variant != "reduceonly":
                        qb.dma_start(vs_t[:, 0:bc], vs_d[bi][:])
                    else:
                        qb.dma_start(vs_t[:, 0:4], vs_d[bi][:, 0:4])
                    m0 = wp.tile([128, BCMAX], bf, tag="m0")
                    nc.vector.tensor_mul(m0[:, 0:bc], ew_t[:, 0:bc],
                                         vs_t[:, 0:bc])
                    _reduce_2stage(m0, bc, runs, agg[0], "s1a")

            if variant in ("dmaonly", "reduceonly", "noepi"):
                return
            # ---- epilogue (self-loop slots make agg complete: deg sweep
            # yields deg+1; layer sweeps include the dinv*val self term) ----
            if mode == "deg":
                sq = wp.tile([128, TPC], f32, tag="sq")
                nc.scalar.activation(sq, agg[0],
                                     mybir.ActivationFunctionType.Sqrt)
                dinv = wp.tile([128, TPC], f32, tag="dinv")
                nc.vector.reciprocal(dinv, sq)
                nc.sync.dma_start(dinv_out[:], dinv[:])
                for f in range(D):
                    xt = wp.tile([128, TPC], f32, tag=f"xt{f}")
                    nc.vector.tensor_mul(xt, xN[f], dinv)
                    nc.sync.dma_start(xt_out[f][:], xt[:])
            elif mode == "layer1":
                zb = []
                for f in range(D):
                    z = wp.tile([128, TPC], f32, tag=f"z{f}")
                    nc.vector.tensor_mul(z, agg[f], dinvN)
                    zb_ = wp.tile([128, TPC], bf, tag=f"zb{f}")
                    nc.scalar.copy(zb_, z)
                    zb.append(zb_)
                # h layout [128, (j t)]: hidden-unit major; weights arrive
                # pre-materialized in the same layout so every product runs
                # in the 2x packed-bf16 DVE mode.
                hA = wp.tile([128, TPC * HID], bf, tag="hA")
                nc.vector.tensor_tensor(
                    hA.rearrange("p (j t) -> p j t", j=HID),
                    zb[0].unsqueeze(1).broadcast_to([128, HID, TPC]),
                    w1x[0].rearrange("p (j t) -> p j t", j=HID),
                    mybir.AluOpType.mult)
                hB = wp.tile([128, TPC * HID], bf, tag="hB")
                nc.vector.tensor_tensor(
                    hB.rearrange("p (j t) -> p j t", j=HID),
                    zb[1].unsqueeze(1).broadcast_to([128, HID, TPC]),
                    w1x[1].rearrange("p (j t) -> p j t", j=HID),
                    mybir.AluOpType.mult)
                nc.vector.tensor_add(hA, hA, hB)
                if not skip_b1:
                    nc.vector.tensor_tensor(
                        hA.rearrange("p (j t) -> p j t", j=HID),
                        hA.rearrange("p (j t) -> p j t", j=HID),
                        b1_sb.unsqueeze(2).broadcast_to([128, HID, TPC]),
                        mybir.AluOpType.add)
                nc.scalar.activation(hA, hA, mybir.ActivationFunctionType.Relu)
                hv = wp.tile([128, TPC * HID], bf, tag="hv")
                nc.vector.tensor_tensor(hv, hA, w2x_sb[:],
                                        mybir.AluOpType.mult)
                v = wp.tile([128, TPC], f32, tag="v")
                nc.vector.tensor_reduce(
                    v, hv.rearrange("p (j t) -> p t j", j=HID),
                    mybir.AxisListType.X, mybir.AluOpType.add)
                vt = wp.tile([128, TPC], f32, tag="vt")
                nc.vector.tensor_mul(vt, v, dinvN)
                nc.sync.dma_start(vt_out[:], vt[:])
            else:
                y = wp.tile([128, TPC], f32, tag="y")
                nc.vector.tensor_mul(y, agg[0], dinvN)
                if not skip_b2:
                    nc.vector.tensor_scalar(y, y, b2_sb[:, 0:1], None,
                                            mybir.AluOpType.add)
                nc.sync.dma_start(y_out[:], y[:])

        if reps > 1:
            assert reps % unroll == 0
            with tc.For_i(0, reps // unroll, 1):
                for _ in range(unroll):
                    body()
        else:
            body()

    _split_multi_waits(nc)
    return nc


def _rep_bf16(vec):
    return np.ascontiguousarray(
        np.tile(np.asarray(vec, np.float32).reshape(1, -1), (128, 1))
    ).astype(BF16)


def kernel(x, edge_index, edge_weight, W1, b1, W2, b2):
    x = np.asarray(x, np.float32)
    edge_index = np.asarray(edge_index)
    edge_weight = np.asarray(edge_weight, np.float32)
    W1 = np.asarray(W1, np.float32)
    b1 = np.asarray(b1, np.float32)
    W2 = np.asarray(W2, np.float32)
    b2 = np.asarray(b2, np.float32)
    skip_b1 = not np.any(b1 != 0)
    skip_b2 = not np.any(b2 != 0)

    pp = _preprocess(edge_index, edge_weight)
    order = pp["order"]

    ew_cs = _stream_blocks(pp, pp["ew"], "ew", BF16)

    xfull = np.zeros((NPAD, D), np.float32)
    xfull[:N] = x
    xnew = xfull[order]                          # newpos layout
    xn_cs = [_to_core_nodes(xnew[:, f], np.float32) for f in range(D)]

    # ---- NEFF 1: deg+1 -> dinv, x*dinv ----
    nc1 = _build_sweep("deg", pp)
    in1 = [dict(ew_cs[c], xn0=xn_cs[0][c], xn1=xn_cs[1][c])
           for c in range(NCORE)]
    r1 = run_bass_kernel_spmd(nc1, in1, core_ids=list(range(NCORE)))
    dinv_new = _from_core_nodes([r1.results[c]["dinvout"] for c in range(NCORE)])
    xt_new = [_from_core_nodes([r1.results[c][f"xtout{f}"] for c in range(NCORE)])
              for f in range(D)]

    # ---- host glue: per-edge (x*dinv)[src] streams ----
    xt_orig = np.empty((NPAD, D), np.float32)
    for f in range(D):
        xt_orig[order, f] = xt_new[f]
    xs_cs = [_stream_blocks(pp, xt_orig[pp["src"], f], f"xs{f}_", BF16)
             for f in range(D)]
    dinv_n = _to_core_nodes(dinv_new, np.float32)

    w1x = [_rep_bf16(np.repeat(W1[f], TPC)) for f in range(D)]
    b1b = _rep_bf16(b1)
    w2xb = _rep_bf16(np.repeat(W2[:, 0], TPC))
    b2b = np.full((128, 1), float(b2[0]), np.float32)

    # ---- NEFF 2: layer 1 -> v*dinv ----
    nc2 = _build_sweep("layer1", pp, skip_b1=skip_b1, skip_b2=skip_b2)
    in2 = [dict(ew_cs[c], **xs_cs[0][c], **xs_cs[1][c],
                dinvn=dinv_n[c], w1x0b=w1x[0], w1x1b=w1x[1], b1b=b1b,
                w2xb=w2xb) for c in range(NCORE)]
    r2 = run_bass_kernel_spmd(nc2, in2, core_ids=list(range(NCORE)))
    vt_new = _from_core_nodes([r2.results[c]["vtout"] for c in range(NCORE)])

    # ---- host glue: (v*dinv)[src] stream ----
    vt_orig = np.empty(NPAD, np.float32)
    vt_orig[order] = vt_new
    vs_cs = _stream_blocks(pp, vt_orig[pp["src"]], "vs", BF16)
    dinv_n2 = dinv_n

    # ---- NEFF 3: layer 2 -> output ----
    nc3 = _build_sweep("layer2", pp, skip_b1=skip_b1, skip_b2=skip_b2)
    in3 = [dict(ew_cs[c], **vs_cs[c], dinvn=dinv_n2[c], b2b=b2b)
           for c in range(NCORE)]
    r3 = run_bass_kernel_spmd(nc3, in3, core_ids=list(range(NCORE)))
    y_new = _from_core_nodes([r3.results[c]["yout"] for c in range(NCORE)])

    y_orig = np.empty(NPAD, np.float32)
    y_orig[order] = y_new
    return y_orig[:N, None].astype(np.float32)


# revision 18
# speedup vs baseline: 51.2161x; 1.1128x over previous
"""GCN (2-layer, PyG gcn_norm) on 8 Trainium2 NeuronCores via Bass.

Strategy (dst-partition-row sharding, no collectives, no PE):
  * Host appends self-loop edges (weight 1, as in gcn_norm), sorts nodes
    by in-degree and assigns each node one SBUF partition-row of G slots
    (G = per-stripe max degree, rounded up), so the per-node segment-sum
    is a plain free-axis tensor_reduce — no one-hot masks, no matmuls.
    Stripes of 1024 nodes (one 128-node tile per core) share a G schedule
    so all 8 SPMD cores run one program.
  * Per-edge streams are bf16 (tolerance is 2e-2); accumulations stay
    f32.  Streams are stored per DMA-block so every transfer is fully
    contiguous, issued alternately on the SP and Activation DGE queues.
  * Three sequential NEFF launches: (1) deg -> dinv, x*dinv, (2) layer-1
    aggregation -> h -> v*dinv, (3) layer-2 aggregation -> output.
    Between launches the host only gathers returned per-node arrays into
    per-edge streams (index-space data movement, no float math).
"""

import sys

sys.path.insert(0, "/opt/trn_rl_repo")

import numpy as np
import ml_dtypes

import concourse.bass as bass
import concourse.tile as tile
from concourse import mybir
from concourse.bass_utils import run_bass_kernel_spmd

BF16 = ml_dtypes.bfloat16

N = 100000
E = 3200000
D = 2
HID = 16
NCORE = 8
TPC = 98                      # stripes == node tiles per core
NPAD = TPC * 1024             # 100352
GMULT = 8                     # stripe slot width rounded up to this
BLK_COLS = 1024               # target stream columns per DMA block


def _split_multi_waits(nc):
    """This toolchain's walrus encodes at most one sync-wait per instruction.
    Hoist extra waits onto fresh single-wait NoOps placed just before."""
    ctr = 0
    for fn in nc.m.functions:
        for bb in fn.blocks:
            insts = list(bb.instructions)
            if not any(
                i.sync_info is not None and len(i.sync_info.on_wait or []) > 1
                for i in insts
            ):
                continue
            new = []
            for inst in insts:
                si = inst.sync_info
                if si is not None and len(si.on_wait or []) > 1:
                    waits = list(si.on_wait)
                    for w in waits[:-1]:
                        ctr += 1
                        new.append(
                            mybir.InstNoOp(
                                name=f"wsplit-{ctr}",
                                engine=inst.engine,
                                sync_info=mybir.SyncInfo(on_wait=[w], on_update=[]),
                                bass_nofuse=True,
                            )
                        )
                    si.on_wait = [waits[-1]]
                new.append(inst)
            bb.instructions = new
    return ctr


def _preprocess(edge_index, edge_weight):
    """Append self-loops, degree-sort nodes, assign each node a
    partition-row slot range, and scatter edge weight / src index into the
    per-core slot streams."""
    loop = np.arange(N, dtype=np.int64)
    dst = np.concatenate([edge_index[1].astype(np.int64), loop])
    src = np.concatenate([edge_index[0].astype(np.int64), loop])
    ew = np.concatenate([edge_weight.astype(np.float32),
                         np.ones(N, np.float32)])
    ne = len(dst)

    deg = np.bincount(dst, minlength=NPAD)
    order = np.argsort(deg, kind="stable")       # newpos -> orig id
    newpos = np.empty(NPAD, np.int64)
    newpos[order] = np.arange(NPAD)

    counts_new = deg[order]                      # per-newpos degree
    smax = counts_new.reshape(TPC, 1024).max(axis=1)
    G = np.maximum(GMULT, ((smax + GMULT - 1) // GMULT) * GMULT).astype(np.int64)
    offs = np.zeros(TPC + 1, np.int64)
    np.cumsum(G, out=offs[1:])
    CS = int(offs[-1])

    nd = newpos[dst]
    start = np.zeros(NPAD + 1, np.int64)
    np.cumsum(counts_new, out=start[1:])
    perm = np.argsort(nd, kind="stable")
    r = np.empty(ne, np.int64)
    r[perm] = np.arange(ne) - start[nd[perm]]    # rank of edge within its dst

    s = nd >> 10
    w = nd & 1023
    c = w >> 7
    p = w & 127
    flat = (c * 128 + p) * CS + offs[s] + r

    ew_flat = np.zeros(NCORE * 128 * CS, np.float32)
    src_flat = np.zeros(NCORE * 128 * CS, np.int64)
    ew_flat[flat] = ew
    src_flat[flat] = src

    # DMA blocks: consecutive stripes until >= BLK_COLS columns; per-block
    # runs of stripes sharing G (one tensor_reduce instruction per run).
    blocks = []
    t0, cols = 0, 0
    for t in range(TPC):
        cols += int(G[t])
        if cols >= BLK_COLS or t == TPC - 1:
            runs = []
            ro = 0
            for tt in range(t0, t + 1):
                g = int(G[tt])
                if runs and runs[-1][2] == g:
                    runs[-1] = (runs[-1][0], runs[-1][1] + 1, g, runs[-1][3])
                else:
                    runs.append((tt, 1, g, ro))
                ro += g
            blocks.append((t0, t + 1 - t0, int(offs[t0]), cols, runs))
            t0, cols = t + 1, 0

    return dict(G=G, offs=offs, CS=CS, blocks=blocks, order=order,
                ew=ew_flat, src=src_flat)


def _stream_blocks(sched, arrflat, prefix, dtype):
    """Per-core dicts of per-DMA-block contiguous stream arrays."""
    CS = sched["CS"]
    a = arrflat.reshape(NCORE, 128, CS)
    out = []
    for c in range(NCORE):
        d = {}
        for bi, (t0, ntb, c0, bc, runs) in enumerate(sched["blocks"]):
            d[f"{prefix}{bi}"] = np.ascontiguousarray(
                a[c, :, c0:c0 + bc]).astype(dtype)
        out.append(d)
    return out


def _to_core_nodes(val_new, dtype):
    """[NPAD] array in newpos space -> per-core [128, TPC]
    (newpos = s*1024 + c*128 + p)."""
    a = val_new.reshape(TPC, NCORE, 128)
    return [np.ascontiguousarray(a[:, c, :].T).astype(dtype) for c in range(NCORE)]


def _from_core_nodes(parts):
    full = np.empty((TPC, NCORE, 128), np.float32)
    for c in range(NCORE):
        full[:, c, :] = np.asarray(parts[c], np.float32).T
    return full.reshape(NPAD)


def _build_sweep(mode, sched, reps=1, variant=None, unroll=16,
                 skip_b1=True, skip_b2=True):
    """Build the Bass program for one sweep. mode in {deg, layer1, layer2}.
    reps>1 wraps `reps` copies of the (idempotent) body in a hardware For_i
    loop, `unroll` bodies per trip — used only for timing measurements.
    variant (timing experiments only): 'dmaonly' = stream DMA without
    compute, 'reduceonly' = compute without stream DMA."""
    from contextlib import ExitStack

    CS = sched["CS"]
    blocks = sched["blocks"]
    BCMAX = max(b[3] for b in blocks)
    f32 = mybir.dt.float32
    bf = mybir.dt.bfloat16

    nc = bass.Bass("TRN2", target_bir_lowering=False, debug=False,
                   num_devices=NCORE)

    def din(name, shape, dtype=f32):
        return nc.dram_tensor(name, shape, dtype, kind="ExternalInput").ap()

    def dout(name, shape, dtype=f32):
        return nc.dram_tensor(name, shape, dtype, kind="ExternalOutput").ap()

    def din_blocks(prefix):
        return [din(f"{prefix}{bi}", [128, b[3]], bf)
                for bi, b in enumerate(blocks)]

    ew_d = din_blocks("ew")
    if mode == "deg":
        xn_d = [din(f"xn{f}", [128, TPC]) for f in range(D)]
        dinv_out = dout("dinvout", [128, TPC])
        xt_out = [dout(f"xtout{f}", [128, TPC]) for f in range(D)]
    elif mode == "layer1":
        xs_d = [din_blocks(f"xs{f}_") for f in range(D)]
        dinv_d = din("dinvn", [128, TPC])
        w1x_d = [din(f"w1x{f}b", [128, HID * TPC], bf) for f in range(D)]
        b1_d = din("b1b", [128, HID], bf)
        w2x_d = din("w2xb", [128, HID * TPC], bf)
        vt_out = dout("vtout", [128, TPC])
    else:
        vs_d = din_blocks("vs")
        dinv_d = din("dinvn", [128, TPC])
        b2_d = din("b2b", [128, 1])
        y_out = dout("yout", [128, TPC])

    with tile.TileContext(nc) as tc, ExitStack() as ctx:
        const = ctx.enter_context(tc.tile_pool(name="const", bufs=1))
        sp = ctx.enter_context(tc.tile_pool(name="streams", bufs=4))
        wp = ctx.enter_context(tc.tile_pool(name="work", bufs=4))
        accp = ctx.enter_context(tc.tile_pool(name="acc", bufs=2))

        if mode == "deg":
            xN = []
            for f in range(D):
                t_ = const.tile([128, TPC], f32, tag=f"xn{f}")
                nc.sync.dma_start(t_[:], xn_d[f][:])
                xN.append(t_)
        elif mode == "layer1":
            dinvN = const.tile([128, TPC], f32)
            nc.sync.dma_start(dinvN[:], dinv_d[:])
            w1x = []
            for f in range(D):
                t_ = const.tile([128, HID * TPC], bf, tag=f"w1x{f}")
                nc.sync.dma_start(t_[:], w1x_d[f][:])
                w1x.append(t_)
            b1_sb = const.tile([128, HID], bf)
            nc.sync.dma_start(b1_sb[:], b1_d[:])
            w2x_sb = const.tile([128, HID * TPC], bf)
            nc.sync.dma_start(w2x_sb[:], w2x_d[:])
        else:
            dinvN = const.tile([128, TPC], f32)
            nc.sync.dma_start(dinvN[:], dinv_d[:])
            b2_sb = const.tile([128, 1], f32)
            nc.sync.dma_start(b2_sb[:], b2_d[:])

        nF = D if mode == "layer1" else 1

        def _reduce_2stage(m_t, bc, runs, dst_agg, s1tag):
            # stage 1: one bf16 2x-rate instruction per block summing 8-slot
            # sub-chunks; stage 2: f32 per-run reduce of the partials.
            s1 = wp.tile([128, BCMAX // 8], bf, tag=s1tag)
            with nc.allow_low_precision("stage-1 partial sums of 8 bf16 terms"):
                nc.vector.tensor_reduce(
                    s1[:, 0:bc // 8],
                    m_t[:, 0:bc].rearrange("p (q g) -> p q g", g=8),
                    mybir.AxisListType.X, mybir.AluOpType.add)
            for (tt, nt, g, ro) in runs:
                nc.vector.tensor_reduce(
                    dst_agg[:, tt:tt + nt],
                    s1[:, ro // 8:ro // 8 + nt * (g // 8)].rearrange(
                        "p (t q) -> p t q", q=g // 8),
                    mybir.AxisListType.X, mybir.AluOpType.add)

        def body():
            agg = [accp.tile([128, TPC], f32, tag=f"agg{f}", name=f"agg{f}")
                   for f in range(nF)]
            for bi, (t0, ntb, c0, bc, runs) in enumerate(blocks):
                qa = nc.sync if bi % 2 == 0 else nc.scalar
                qb = nc.scalar if bi % 2 == 0 else nc.sync
                ew_t = sp.tile([128, BCMAX], bf, tag="ew")
                if variant != "reduceonly":
                    qa.dma_start(ew_t[:, 0:bc], ew_d[bi][:])
                else:
                    qa.dma_start(ew_t[:, 0:4], ew_d[bi][:, 0:4])
                if variant == "dmaonly":
                    continue
                if mode == "deg":
                    _reduce_2stage(ew_t, bc, runs, agg[0], "s1a")
                elif mode == "layer1":
                    xs_t = []
                    for f in range(D):
                        qf = qb if f == 0 else qa
                        t_ = sp.tile([128, BCMAX], bf, tag=f"xs{f}")
                        if variant != "reduceonly":
                            qf.dma_start(t_[:, 0:bc], xs_d[f][bi][:])
                        else:
                            qf.dma_start(t_[:, 0:4], xs_d[f][bi][:, 0:4])
                        xs_t.append(t_)
                    m0 = wp.tile([128, BCMAX], bf, tag="m0")
                    nc.vector.tensor_mul(m0[:, 0:bc], ew_t[:, 0:bc],
                                         xs_t[0][:, 0:bc])
                    m1 = wp.tile([128, BCMAX], bf, tag="m1")
                    nc.vector.tensor_mul(m1[:, 0:bc], ew_t[:, 0:bc],
                                         xs_t[1][:, 0:bc])
                    _reduce_2stage(m0, bc, runs, agg[0], "s1a")
                    _reduce_2stage(m1, bc, runs, agg[1], "s1b")
                else:
                    vs_t = sp.tile([128, BCMAX], bf, tag="vs")
                    if variant != "reduceonly":
                        qb.dma_start(vs_t[:, 0:bc], vs_d[bi][:])
                    else:
                        qb.dma_start(vs_t[:, 0:4], vs_d[bi][:, 0:4])
                    m0 = wp.tile([128, BCMAX], bf, tag="m0")
                    nc.vector.tensor_mul(m0[:, 0:bc], ew_t[:, 0:bc],
                                         vs_t[:, 0:bc])
                    _reduce_2stage(m0, bc, runs, agg[0], "s1a")

            if variant in ("dmaonly", "reduceonly", "noepi"):
                return
            # ---- epilogue (self-loop slots make agg complete: deg sweep
            # yields deg+1; layer sweeps include the dinv*val self term) ----
            if mode == "deg":
                sq = wp.tile([128, TPC], f32, tag="sq")
                nc.scalar.activation(sq, agg[0],
                                     mybir.ActivationFunctionType.Sqrt)
                dinv = wp.tile([128, TPC], f32, tag="dinv")
                nc.vector.reciprocal(dinv, sq)
                nc.sync.dma_start(dinv_out[:], dinv[:])
                for f in range(D):
                    xt = wp.tile([128, TPC], f32, tag=f"xt{f}")
                    nc.vector.tensor_mul(xt, xN[f], dinv)
                    nc.sync.dma_start(xt_out[f][:], xt[:])
            elif mode == "layer1":
                zb = []
                for f in range(D):
                    z = wp.tile([128, TPC], f32, tag=f"z{f}")
                    nc.vector.tensor_mul(z, agg[f], dinvN)
                    zb_ = wp.tile([128, TPC], bf, tag=f"zb{f}")
                    nc.vector.tensor_copy(zb_, z)
                    zb.append(zb_)
                # h layout [128, (j t)]: hidden-unit major; weights arrive
                # pre-materialized in the same layout so every product runs
                # in the 2x packed-bf16 DVE mode.
                hA = wp.tile([128, TPC * HID], bf, tag="hA")
                nc.vector.tensor_tensor(
                    hA.rearrange("p (j t) -> p j t", j=HID),
                    zb[0].unsqueeze(1).broadcast_to([128, HID, TPC]),
                    w1x[0].rearrange("p (j t) -> p j t", j=HID),
                    mybir.AluOpType.mult)
                hB = wp.tile([128, TPC * HID], bf, tag="hB")
                nc.vector.tensor_tensor(
                    hB.rearrange("p (j t) -> p j t", j=HID),
                    zb[1].unsqueeze(1).broadcast_to([128, HID, TPC]),
                    w1x[1].rearrange("p (j t) -> p j t", j=HID),
                    mybir.AluOpType.mult)
                nc.vector.tensor_add(hA, hA, hB)
                if not skip_b1:
                    nc.vector.tensor_tensor(
                        hA.rearrange("p (j t) -> p j t", j=HID),
                        hA.rearrange("p (j t) -> p j t", j=HID),
                        b1_sb.unsqueeze(2).broadcast_to([128, HID, TPC]),
                        mybir.AluOpType.add)
                nc.vector.tensor_scalar_max(hA, hA, 0.0)
                hv = wp.tile([128, TPC * HID], bf, tag="hv")
                nc.vector.tensor_tensor(hv, hA, w2x_sb[:],
                                        mybir.AluOpType.mult)
                v = wp.tile([128, TPC], f32, tag="v")
                nc.vector.tensor_reduce(
                    v, hv.rearrange("p (j t) -> p t j", j=HID),
                    mybir.AxisListType.X, mybir.AluOpType.add)
                vt = wp.tile([128, TPC], f32, tag="vt")
                nc.vector.tensor_mul(vt, v, dinvN)
                nc.sync.dma_start(vt_out[:], vt[:])
            else:
                y = wp.tile([128, TPC], f32, tag="y")
                nc.vector.tensor_mul(y, agg[0], dinvN)
                if not skip_b2:
                    nc.vector.tensor_scalar(y, y, b2_sb[:, 0:1], None,
                                            mybir.AluOpType.add)
                nc.sync.dma_start(y_out[:], y[:])

        if reps > 1:
            assert reps % unroll == 0
            with tc.For_i(0, reps // unroll, 1):
                for _ in range(unroll):
                    body()
        else:
            body()

    _split_multi_waits(nc)
    return nc


def _rep_bf16(vec):
    return np.ascontiguousarray(
        np.tile(np.asarray(vec, np.float32).reshape(1, -1), (128, 1))
    ).astype(BF16)


def kernel(x, edge_index, edge_weight, W1, b1, W2, b2):
    x = np.asarray(x, np.float32)
    edge_index = np.asarray(edge_index)
    edge_weight = np.asarray(edge_weight, np.float32)
    W1 = np.asarray(W1, np.float32)
    b1 = np.asarray(b1, np.float32)
    W2 = np.asarray(W2, np.float32)
    b2 = np.asarray(b2, np.float32)
    skip_b1 = not np.any(b1 != 0)
    skip_b2 = not np.any(b2 != 0)

    pp = _preprocess(edge_index, edge_weight)
    order = pp["order"]

    ew_cs = _stream_blocks(pp, pp["ew"], "ew", BF16)

    xfull = np.zeros((NPAD, D), np.float32)
    xfull[:N] = x
    xnew = xfull[order]                          # newpos layout
    xn_cs = [_to_core_nodes(xnew[:, f], np.float32) for f in range(D)]

    # ---- NEFF 1: deg+1 -> dinv, x*dinv ----
    nc1 = _build_sweep("deg", pp)
    in1 = [dict(ew_cs[c], xn0=xn_cs[0][c], xn1=xn_cs[1][c])
           for c in range(NCORE)]
    r1 = run_bass_kernel_spmd(nc1, in1, core_ids=list(range(NCORE)))
    dinv_new = _from_core_nodes([r1.results[c]["dinvout"] for c in range(NCORE)])
    xt_new = [_from_core_nodes([r1.results[c][f"xtout{f}"] for c in range(NCORE)])
              for f in range(D)]

    # ---- host glue: per-edge (x*dinv)[src] streams ----
    xt_orig = np.empty((NPAD, D), np.float32)
    for f in range(D):
        xt_orig[order, f] = xt_new[f]
    xs_cs = [_stream_blocks(pp, xt_orig[pp["src"], f], f"xs{f}_", BF16)
             for f in range(D)]
    dinv_n = _to_core_nodes(dinv_new, np.float32)

    w1x = [_rep_bf16(np.repeat(W1[f], TPC)) for f in range(D)]
    b1b = _rep_bf16(b1)
    w2xb = _rep_bf16(np.repeat(W2[:, 0], TPC))
    b2b = np.full((128, 1), float(b2[0]), np.float32)

    # ---- NEFF 2: layer 1 -> v*dinv ----
    nc2 = _build_sweep("layer1", pp, skip_b1=skip_b1, skip_b2=skip_b2)
    in2 = [dict(ew_cs[c], **xs_cs[0][c], **xs_cs[1][c],
                dinvn=dinv_n[c], w1x0b=w1x[0], w1x1b=w1x[1], b1b=b1b,
                w2xb=w2xb) for c in range(NCORE)]
    r2 = run_bass_kernel_spmd(nc2, in2, core_ids=list(range(NCORE)))
    vt_new = _from_core_nodes([r2.results[c]["vtout"] for c in range(NCORE)])

    # ---- host glue: (v*dinv)[src] stream ----
    vt_orig = np.empty(NPAD, np.float32)
    vt_orig[order] = vt_new
    vs_cs = _stream_blocks(pp, vt_orig[pp["src"]], "vs", BF16)
    dinv_n2 = dinv_n

    # ---- NEFF 3: layer 2 -> output ----
    nc3 = _build_sweep("layer2", pp, skip_b1=skip_b1, skip_b2=skip_b2)
    in3 = [dict(ew_cs[c], **vs_cs[c], dinvn=dinv_n2[c], b2b=b2b)
           for c in range(NCORE)]
    r3 = run_bass_kernel_spmd(nc3, in3, core_ids=list(range(NCORE)))
    y_new = _from_core_nodes([r3.results[c]["yout"] for c in range(NCORE)])

    y_orig = np.empty(NPAD, np.float32)
    y_orig[order] = y_new
    return y_orig[:N, None].astype(np.float32)


# revision 19
# speedup vs baseline: 54.4535x; 1.0632x over previous
"""GCN (2-layer, PyG gcn_norm) on 8 Trainium2 NeuronCores via Bass.

Strategy (dst-partition-row sharding, no collectives, no PE):
  * Host appends self-loop edges (weight 1, as in gcn_norm), sorts nodes
    by in-degree and assigns each node one SBUF partition-row of G slots
    (G = per-stripe max degree, rounded up), so the per-node segment-sum
    is a plain free-axis tensor_reduce — no one-hot masks, no matmuls.
    Stripes of 1024 nodes (one 128-node tile per core) share a G schedule
    so all 8 SPMD cores run one program.
  * Per-edge streams are bf16 (tolerance is 2e-2); accumulations stay
    f32.  Streams are stored per DMA-block so every transfer is fully
    contiguous, issued alternately on the SP and Activation DGE queues.
  * Three sequential NEFF launches: (1) deg -> dinv, x*dinv, (2) layer-1
    aggregation -> h -> v*dinv, (3) layer-2 aggregation -> output.
    Between launches the host only gathers returned per-node arrays into
    per-edge streams (index-space data movement, no float math).
"""

import sys

sys.path.insert(0, "/opt/trn_rl_repo")

import numpy as np
import ml_dtypes

import concourse.bass as bass
import concourse.tile as tile
from concourse import mybir
from concourse.bass_utils import run_bass_kernel_spmd

BF16 = ml_dtypes.bfloat16

N = 100000
E = 3200000
D = 2
HID = 16
NCORE = 8
TPC = 98                      # stripes == node tiles per core
NPAD = TPC * 1024             # 100352
GMULT = 8                     # stripe slot width rounded up to this
BLK_COLS = 4096               # target stream columns per DMA block (>= CS: single block)


def _split_multi_waits(nc):
    """This toolchain's walrus encodes at most one sync-wait per instruction.
    Hoist extra waits onto fresh single-wait NoOps placed just before."""
    ctr = 0
    for fn in nc.m.functions:
        for bb in fn.blocks:
            insts = list(bb.instructions)
            if not any(
                i.sync_info is not None and len(i.sync_info.on_wait or []) > 1
                for i in insts
            ):
                continue
            new = []
            for inst in insts:
                si = inst.sync_info
                if si is not None and len(si.on_wait or []) > 1:
                    waits = list(si.on_wait)
                    for w in waits[:-1]:
                        ctr += 1
                        new.append(
                            mybir.InstNoOp(
                                name=f"wsplit-{ctr}",
                                engine=inst.engine,
                                sync_info=mybir.SyncInfo(on_wait=[w], on_update=[]),
                                bass_nofuse=True,
                            )
                        )
                    si.on_wait = [waits[-1]]
                new.append(inst)
            bb.instructions = new
    return ctr


def _preprocess(edge_index, edge_weight):
    """Append self-loops, degree-sort nodes, assign each node a
    partition-row slot range, and scatter edge weight / src index into the
    per-core slot streams."""
    loop = np.arange(N, dtype=np.int64)
    dst = np.concatenate([edge_index[1].astype(np.int64), loop])
    src = np.concatenate([edge_index[0].astype(np.int64), loop])
    ew = np.concatenate([edge_weight.astype(np.float32),
                         np.ones(N, np.float32)])
    ne = len(dst)

    deg = np.bincount(dst, minlength=NPAD)
    order = np.argsort(deg, kind="stable")       # newpos -> orig id
    newpos = np.empty(NPAD, np.int64)
    newpos[order] = np.arange(NPAD)

    counts_new = deg[order]                      # per-newpos degree
    smax = counts_new.reshape(TPC, 1024).max(axis=1)
    G = np.maximum(GMULT, ((smax + GMULT - 1) // GMULT) * GMULT).astype(np.int64)
    offs = np.zeros(TPC + 1, np.int64)
    np.cumsum(G, out=offs[1:])
    CS = int(offs[-1])

    nd = newpos[dst]
    start = np.zeros(NPAD + 1, np.int64)
    np.cumsum(counts_new, out=start[1:])
    perm = np.argsort(nd, kind="stable")
    r = np.empty(ne, np.int64)
    r[perm] = np.arange(ne) - start[nd[perm]]    # rank of edge within its dst

    s = nd >> 10
    w = nd & 1023
    c = w >> 7
    p = w & 127
    flat = (c * 128 + p) * CS + offs[s] + r

    ew_flat = np.zeros(NCORE * 128 * CS, np.float32)
    src_flat = np.zeros(NCORE * 128 * CS, np.int64)
    ew_flat[flat] = ew
    src_flat[flat] = src

    # DMA blocks: consecutive stripes until >= BLK_COLS columns; per-block
    # runs of stripes sharing G (one tensor_reduce instruction per run).
    blocks = []
    t0, cols = 0, 0
    for t in range(TPC):
        cols += int(G[t])
        if cols >= BLK_COLS or t == TPC - 1:
            runs = []
            ro = 0
            for tt in range(t0, t + 1):
                g = int(G[tt])
                if runs and runs[-1][2] == g:
                    runs[-1] = (runs[-1][0], runs[-1][1] + 1, g, runs[-1][3])
                else:
                    runs.append((tt, 1, g, ro))
                ro += g
            blocks.append((t0, t + 1 - t0, int(offs[t0]), cols, runs))
            t0, cols = t + 1, 0

    return dict(G=G, offs=offs, CS=CS, blocks=blocks, order=order,
                ew=ew_flat, src=src_flat)


def _stream_blocks(sched, arrflat, prefix, dtype):
    """Per-core dicts of per-DMA-block contiguous stream arrays."""
    CS = sched["CS"]
    a = arrflat.reshape(NCORE, 128, CS)
    out = []
    for c in range(NCORE):
        d = {}
        for bi, (t0, ntb, c0, bc, runs) in enumerate(sched["blocks"]):
            d[f"{prefix}{bi}"] = np.ascontiguousarray(
                a[c, :, c0:c0 + bc]).astype(dtype)
        out.append(d)
    return out


def _to_core_nodes(val_new, dtype):
    """[NPAD] array in newpos space -> per-core [128, TPC]
    (newpos = s*1024 + c*128 + p)."""
    a = val_new.reshape(TPC, NCORE, 128)
    return [np.ascontiguousarray(a[:, c, :].T).astype(dtype) for c in range(NCORE)]


def _from_core_nodes(parts):
    full = np.empty((TPC, NCORE, 128), np.float32)
    for c in range(NCORE):
        full[:, c, :] = np.asarray(parts[c], np.float32).T
    return full.reshape(NPAD)


def _build_sweep(mode, sched, reps=1, variant=None, unroll=16,
                 skip_b1=True, skip_b2=True):
    """Build the Bass program for one sweep. mode in {deg, layer1, layer2}.
    reps>1 wraps `reps` copies of the (idempotent) body in a hardware For_i
    loop, `unroll` bodies per trip — used only for timing measurements.
    variant (timing experiments only): 'dmaonly' = stream DMA without
    compute, 'reduceonly' = compute without stream DMA."""
    from contextlib import ExitStack

    CS = sched["CS"]
    blocks = sched["blocks"]
    BCMAX = max(b[3] for b in blocks)
    f32 = mybir.dt.float32
    bf = mybir.dt.bfloat16

    nc = bass.Bass("TRN2", target_bir_lowering=False, debug=False,
                   num_devices=NCORE)

    def din(name, shape, dtype=f32):
        return nc.dram_tensor(name, shape, dtype, kind="ExternalInput").ap()

    def dout(name, shape, dtype=f32):
        return nc.dram_tensor(name, shape, dtype, kind="ExternalOutput").ap()

    def din_blocks(prefix):
        return [din(f"{prefix}{bi}", [128, b[3]], bf)
                for bi, b in enumerate(blocks)]

    ew_d = din_blocks("ew")
    if mode == "deg":
        xn_d = [din(f"xn{f}", [128, TPC]) for f in range(D)]
        dinv_out = dout("dinvout", [128, TPC])
        xt_out = [dout(f"xtout{f}", [128, TPC]) for f in range(D)]
    elif mode == "layer1":
        xs_d = [din_blocks(f"xs{f}_") for f in range(D)]
        dinv_d = din("dinvn", [128, TPC])
        w1x_d = [din(f"w1x{f}b", [128, HID * TPC], bf) for f in range(D)]
        b1_d = din("b1b", [128, HID], bf)
        w2x_d = din("w2xb", [128, HID * TPC], bf)
        vt_out = dout("vtout", [128, TPC])
    else:
        vs_d = din_blocks("vs")
        dinv_d = din("dinvn", [128, TPC])
        b2_d = din("b2b", [128, 1])
        y_out = dout("yout", [128, TPC])

    with tile.TileContext(nc) as tc, ExitStack() as ctx:
        const = ctx.enter_context(tc.tile_pool(name="const", bufs=1))
        sp = ctx.enter_context(tc.tile_pool(name="streams", bufs=3))
        wp = ctx.enter_context(tc.tile_pool(name="work", bufs=3))
        accp = ctx.enter_context(tc.tile_pool(name="acc", bufs=2))

        if mode == "deg":
            xN = []
            for f in range(D):
                t_ = const.tile([128, TPC], f32, tag=f"xn{f}")
                nc.sync.dma_start(t_[:], xn_d[f][:])
                xN.append(t_)
        elif mode == "layer1":
            dinvN = const.tile([128, TPC], f32)
            nc.sync.dma_start(dinvN[:], dinv_d[:])
            w1x = []
            for f in range(D):
                t_ = const.tile([128, HID * TPC], bf, tag=f"w1x{f}")
                nc.sync.dma_start(t_[:], w1x_d[f][:])
                w1x.append(t_)
            b1_sb = const.tile([128, HID], bf)
            nc.sync.dma_start(b1_sb[:], b1_d[:])
            w2x_sb = const.tile([128, HID * TPC], bf)
            nc.sync.dma_start(w2x_sb[:], w2x_d[:])
        else:
            dinvN = const.tile([128, TPC], f32)
            nc.sync.dma_start(dinvN[:], dinv_d[:])
            b2_sb = const.tile([128, 1], f32)
            nc.sync.dma_start(b2_sb[:], b2_d[:])

        nF = D if mode == "layer1" else 1

        def _reduce_2stage(m_t, bc, runs, dst_agg, s1tag):
            # stage 1: one bf16 2x-rate instruction per block summing 8-slot
            # sub-chunks; stage 2: f32 per-run reduce of the partials.
            s1 = wp.tile([128, BCMAX // 8], bf, tag=s1tag)
            with nc.allow_low_precision("stage-1 partial sums of 8 bf16 terms"):
                nc.vector.tensor_reduce(
                    s1[:, 0:bc // 8],
                    m_t[:, 0:bc].rearrange("p (q g) -> p q g", g=8),
                    mybir.AxisListType.X, mybir.AluOpType.add)
            for (tt, nt, g, ro) in runs:
                nc.vector.tensor_reduce(
                    dst_agg[:, tt:tt + nt],
                    s1[:, ro // 8:ro // 8 + nt * (g // 8)].rearrange(
                        "p (t q) -> p t q", q=g // 8),
                    mybir.AxisListType.X, mybir.AluOpType.add)

        def body():
            agg = [accp.tile([128, TPC], f32, tag=f"agg{f}", name=f"agg{f}")
                   for f in range(nF)]
            for bi, (t0, ntb, c0, bc, runs) in enumerate(blocks):
                qa = nc.sync if bi % 2 == 0 else nc.scalar
                qb = nc.scalar if bi % 2 == 0 else nc.sync
                h1 = (bc // 2) & ~7
                ew_t = sp.tile([128, BCMAX], bf, tag="ew")
                if variant != "reduceonly":
                    qa.dma_start(ew_t[:, 0:h1], ew_d[bi][:, 0:h1])
                    qb.dma_start(ew_t[:, h1:bc], ew_d[bi][:, h1:bc])
                else:
                    qa.dma_start(ew_t[:, 0:4], ew_d[bi][:, 0:4])
                if variant == "dmaonly":
                    continue
                if mode == "deg":
                    _reduce_2stage(ew_t, bc, runs, agg[0], "s1a")
                elif mode == "layer1":
                    xs_t = []
                    for f in range(D):
                        t_ = sp.tile([128, BCMAX], bf, tag=f"xs{f}")
                        if variant != "reduceonly":
                            qb.dma_start(t_[:, 0:h1], xs_d[f][bi][:, 0:h1])
                            qa.dma_start(t_[:, h1:bc], xs_d[f][bi][:, h1:bc])
                        else:
                            qb.dma_start(t_[:, 0:4], xs_d[f][bi][:, 0:4])
                        xs_t.append(t_)
                    m0 = wp.tile([128, BCMAX], bf, tag="m0")
                    nc.vector.tensor_mul(m0[:, 0:bc], ew_t[:, 0:bc],
                                         xs_t[0][:, 0:bc])
                    m1 = wp.tile([128, BCMAX], bf, tag="m1")
                    nc.vector.tensor_mul(m1[:, 0:bc], ew_t[:, 0:bc],
                                         xs_t[1][:, 0:bc])
                    _reduce_2stage(m0, bc, runs, agg[0], "s1a")
                    _reduce_2stage(m1, bc, runs, agg[1], "s1b")
                else:
                    vs_t = sp.tile([128, BCMAX], bf, tag="vs")
                    if variant != "reduceonly":
                        qb.dma_start(vs_t[:, 0:h1], vs_d[bi][:, 0:h1])
                        qa.dma_start(vs_t[:, h1:bc], vs_d[bi][:, h1:bc])
                    else:
                        qb.dma_start(vs_t[:, 0:4], vs_d[bi][:, 0:4])
                    m0 = wp.tile([128, BCMAX], bf, tag="m0")
                    nc.vector.tensor_mul(m0[:, 0:bc], ew_t[:, 0:bc],
                                         vs_t[:, 0:bc])
                    _reduce_2stage(m0, bc, runs, agg[0], "s1a")

            if variant in ("dmaonly", "reduceonly", "noepi"):
                return
            # ---- epilogue (self-loop slots make agg complete: deg sweep
            # yields deg+1; layer sweeps include the dinv*val self term) ----
            if mode == "deg":
                sq = wp.tile([128, TPC], f32, tag="sq")
                nc.scalar.activation(sq, agg[0],
                                     mybir.ActivationFunctionType.Sqrt)
                dinv = wp.tile([128, TPC], f32, tag="dinv")
                nc.vector.reciprocal(dinv, sq)
                nc.sync.dma_start(dinv_out[:], dinv[:])
                for f in range(D):
                    xt = wp.tile([128, TPC], f32, tag=f"xt{f}")
                    nc.vector.tensor_mul(xt, xN[f], dinv)
                    nc.sync.dma_start(xt_out[f][:], xt[:])
            elif mode == "layer1":
                zb = []
                for f in range(D):
                    z = wp.tile([128, TPC], f32, tag=f"z{f}")
                    nc.vector.tensor_mul(z, agg[f], dinvN)
                    zb_ = wp.tile([128, TPC], bf, tag=f"zb{f}")
                    nc.vector.tensor_copy(zb_, z)
                    zb.append(zb_)
                # h layout [128, (j t)]: hidden-unit major; weights arrive
                # pre-materialized in the same layout so every product runs
                # in the 2x packed-bf16 DVE mode.
                hA = wp.tile([128, TPC * HID], bf, tag="hA")
                nc.vector.tensor_tensor(
                    hA.rearrange("p (j t) -> p j t", j=HID),
                    zb[0].unsqueeze(1).broadcast_to([128, HID, TPC]),
                    w1x[0].rearrange("p (j t) -> p j t", j=HID),
                    mybir.AluOpType.mult)
                hB = wp.tile([128, TPC * HID], bf, tag="hB")
                nc.vector.tensor_tensor(
                    hB.rearrange("p (j t) -> p j t", j=HID),
                    zb[1].unsqueeze(1).broadcast_to([128, HID, TPC]),
                    w1x[1].rearrange("p (j t) -> p j t", j=HID),
                    mybir.AluOpType.mult)
                nc.vector.tensor_add(hA, hA, hB)
                if not skip_b1:
                    nc.vector.tensor_tensor(
                        hA.rearrange("p (j t) -> p j t", j=HID),
                        hA.rearrange("p (j t) -> p j t", j=HID),
                        b1_sb.unsqueeze(2).broadcast_to([128, HID, TPC]),
                        mybir.AluOpType.add)
                nc.vector.tensor_scalar_max(hA, hA, 0.0)
                hv = wp.tile([128, TPC * HID], bf, tag="hv")
                nc.vector.tensor_tensor(hv, hA, w2x_sb[:],
                                        mybir.AluOpType.mult)
                v = wp.tile([128, TPC], f32, tag="v")
                nc.vector.tensor_reduce(
                    v, hv.rearrange("p (j t) -> p t j", j=HID),
                    mybir.AxisListType.X, mybir.AluOpType.add)
                vt = wp.tile([128, TPC], f32, tag="vt")
                nc.vector.tensor_mul(vt, v, dinvN)
                nc.sync.dma_start(vt_out[:], vt[:])
            else:
                y = wp.tile([128, TPC], f32, tag="y")
                nc.vector.tensor_mul(y, agg[0], dinvN)
                if not skip_b2:
                    nc.vector.tensor_scalar(y, y, b2_sb[:, 0:1], None,
                                            mybir.AluOpType.add)
                nc.sync.dma_start(y_out[:], y[:])

        if reps > 1:
            assert reps % unroll == 0
            with tc.For_i(0, reps // unroll, 1):
                for _ in range(unroll):
                    body()
        else:
            body()

    _split_multi_waits(nc)
    return nc


def _rep_bf16(vec):
    return np.ascontiguousarray(
        np.tile(np.asarray(vec, np.float32).reshape(1, -1), (128, 1))
    ).astype(BF16)


def kernel(x, edge_index, edge_weight, W1, b1, W2, b2):
    x = np.asarray(x, np.float32)
    edge_index = np.asarray(edge_index)
    edge_weight = np.asarray(edge_weight, np.float32)
    W1 = np.asarray(W1, np.float32)
    b1 = np.asarray(b1, np.float32)
    W2 = np.asarray(W2, np.float32)
    b2 = np.asarray(b2, np.float32)
    skip_b1 = not np.any(b1 != 0)
    skip_b2 = not np.any(b2 != 0)

    pp = _preprocess(edge_index, edge_weight)
    order = pp["order"]

    ew_cs = _stream_blocks(pp, pp["ew"], "ew", BF16)

    xfull = np.zeros((NPAD, D), np.float32)
    xfull[:N] = x
    xnew = xfull[order]                          # newpos layout
    xn_cs = [_to_core_nodes(xnew[:, f], np.float32) for f in range(D)]

    # ---- NEFF 1: deg+1 -> dinv, x*dinv ----
    nc1 = _build_sweep("deg", pp)
    in1 = [dict(ew_cs[c], xn0=xn_cs[0][c], xn1=xn_cs[1][c])
           for c in range(NCORE)]
    r1 = run_bass_kernel_spmd(nc1, in1, core_ids=list(range(NCORE)))
    dinv_new = _from_core_nodes([r1.results[c]["dinvout"] for c in range(NCORE)])
    xt_new = [_from_core_nodes([r1.results[c][f"xtout{f}"] for c in range(NCORE)])
              for f in range(D)]

    # ---- host glue: per-edge (x*dinv)[src] streams ----
    xt_orig = np.empty((NPAD, D), np.float32)
    for f in range(D):
        xt_orig[order, f] = xt_new[f]
    xs_cs = [_stream_blocks(pp, xt_orig[pp["src"], f], f"xs{f}_", BF16)
             for f in range(D)]
    dinv_n = _to_core_nodes(dinv_new, np.float32)

    w1x = [_rep_bf16(np.repeat(W1[f], TPC)) for f in range(D)]
    b1b = _rep_bf16(b1)
    w2xb = _rep_bf16(np.repeat(W2[:, 0], TPC))
    b2b = np.full((128, 1), float(b2[0]), np.float32)

    # ---- NEFF 2: layer 1 -> v*dinv ----
    nc2 = _build_sweep("layer1", pp, skip_b1=skip_b1, skip_b2=skip_b2)
    in2 = [dict(ew_cs[c], **xs_cs[0][c], **xs_cs[1][c],
                dinvn=dinv_n[c], w1x0b=w1x[0], w1x1b=w1x[1], b1b=b1b,
                w2xb=w2xb) for c in range(NCORE)]
    r2 = run_bass_kernel_spmd(nc2, in2, core_ids=list(range(NCORE)))
    vt_new = _from_core_nodes([r2.results[c]["vtout"] for c in range(NCORE)])

    # ---- host glue: (v*dinv)[src] stream ----
    vt_orig = np.empty(NPAD, np.float32)
    vt_orig[order] = vt_new
    vs_cs = _stream_blocks(pp, vt_orig[pp["src"]], "vs", BF16)
    dinv_n2 = dinv_n

    # ---- NEFF 3: layer 2 -> output ----
    nc3 = _build_sweep("layer2", pp, skip_b1=skip_b1, skip_b2=skip_b2)
    in3 = [dict(ew_cs[c], **vs_cs[c], dinvn=dinv_n2[c], b2b=b2b)
           for c in range(NCORE)]
    r3 = run_bass_kernel_spmd(nc3, in3, core_ids=list(range(NCORE)))
    y_new = _from_core_nodes([r3.results[c]["yout"] for c in range(NCORE)])

    y_orig = np.empty(NPAD, np.float32)
    y_orig[order] = y_new
    return y_orig[:N, None].astype(np.float32)


# revision 20
# speedup vs baseline: 64.1240x; 1.1776x over previous
"""GCN (2-layer, PyG gcn_norm) on 8 Trainium2 NeuronCores via Bass.

Strategy (dst-partition-row sharding, no collectives, no PE):
  * Host appends self-loop edges (weight 1, as in gcn_norm), sorts nodes
    by in-degree and assigns each node one SBUF partition-row of G slots
    (G = per-stripe max degree, rounded up), so the per-node segment-sum
    is a plain free-axis tensor_reduce — no one-hot masks, no matmuls.
    Stripes of 1024 nodes (one 128-node tile per core) share a G schedule
    so all 8 SPMD cores run one program.
  * Per-edge streams are bf16 (tolerance is 2e-2); accumulations stay
    f32.  Streams are stored per DMA-block so every transfer is fully
    contiguous, issued alternately on the SP and Activation DGE queues.
  * Three sequential NEFF launches: (1) deg -> dinv, x*dinv, (2) layer-1
    aggregation -> h -> v*dinv, (3) layer-2 aggregation -> output.
    Between launches the host only gathers returned per-node arrays into
    per-edge streams (index-space data movement, no float math).
"""

import sys

sys.path.insert(0, "/opt/trn_rl_repo")

import numpy as np
import ml_dtypes

import concourse.bass as bass
import concourse.tile as tile
from concourse import mybir
from concourse.bass_utils import run_bass_kernel_spmd

BF16 = ml_dtypes.bfloat16

N = 100000
E = 3200000
D = 2
HID = 16
NCORE = 8
TPC = 98                      # stripes == node tiles per core
NPAD = TPC * 1024             # 100352
GMULT = 8                     # stripe slot width rounded up to this
BLK_COLS = 4096               # target stream columns per DMA block (>= CS: single block)


def _split_multi_waits(nc):
    """This toolchain's walrus encodes at most one sync-wait per instruction.
    Hoist extra waits onto fresh single-wait NoOps placed just before."""
    ctr = 0
    for fn in nc.m.functions:
        for bb in fn.blocks:
            insts = list(bb.instructions)
            if not any(
                i.sync_info is not None and len(i.sync_info.on_wait or []) > 1
                for i in insts
            ):
                continue
            new = []
            for inst in insts:
                si = inst.sync_info
                if si is not None and len(si.on_wait or []) > 1:
                    waits = list(si.on_wait)
                    for w in waits[:-1]:
                        ctr += 1
                        new.append(
                            mybir.InstNoOp(
                                name=f"wsplit-{ctr}",
                                engine=inst.engine,
                                sync_info=mybir.SyncInfo(on_wait=[w], on_update=[]),
                                bass_nofuse=True,
                            )
                        )
                    si.on_wait = [waits[-1]]
                new.append(inst)
            bb.instructions = new
    return ctr


def _preprocess(edge_index, edge_weight):
    """Append self-loops, degree-sort nodes, assign each node a
    partition-row slot range, and scatter edge weight / src index into the
    per-core slot streams."""
    loop = np.arange(N, dtype=np.int64)
    dst = np.concatenate([edge_index[1].astype(np.int64), loop])
    src = np.concatenate([edge_index[0].astype(np.int64), loop])
    ew = np.concatenate([edge_weight.astype(np.float32),
                         np.ones(N, np.float32)])
    ne = len(dst)

    deg = np.bincount(dst, minlength=NPAD)
    order = np.argsort(deg, kind="stable")       # newpos -> orig id
    newpos = np.empty(NPAD, np.int64)
    newpos[order] = np.arange(NPAD)

    counts_new = deg[order]                      # per-newpos degree
    smax = counts_new.reshape(TPC, 1024).max(axis=1)
    G = np.maximum(GMULT, ((smax + GMULT - 1) // GMULT) * GMULT).astype(np.int64)
    offs = np.zeros(TPC + 1, np.int64)
    np.cumsum(G, out=offs[1:])
    CS = int(offs[-1])

    nd = newpos[dst]
    start = np.zeros(NPAD + 1, np.int64)
    np.cumsum(counts_new, out=start[1:])
    perm = np.argsort(nd, kind="stable")
    r = np.empty(ne, np.int64)
    r[perm] = np.arange(ne) - start[nd[perm]]    # rank of edge within its dst

    s = nd >> 10
    w = nd & 1023
    c = w >> 7
    p = w & 127
    flat = (c * 128 + p) * CS + offs[s] + r

    ew_flat = np.zeros(NCORE * 128 * CS, np.float32)
    src_flat = np.zeros(NCORE * 128 * CS, np.int64)
    ew_flat[flat] = ew
    src_flat[flat] = src

    # DMA blocks: consecutive stripes until >= BLK_COLS columns; per-block
    # runs of stripes sharing G (one tensor_reduce instruction per run).
    blocks = []
    t0, cols = 0, 0
    for t in range(TPC):
        cols += int(G[t])
        if cols >= BLK_COLS or t == TPC - 1:
            runs = []
            ro = 0
            for tt in range(t0, t + 1):
                g = int(G[tt])
                if runs and runs[-1][2] == g:
                    runs[-1] = (runs[-1][0], runs[-1][1] + 1, g, runs[-1][3])
                else:
                    runs.append((tt, 1, g, ro))
                ro += g
            blocks.append((t0, t + 1 - t0, int(offs[t0]), cols, runs))
            t0, cols = t + 1, 0

    return dict(G=G, offs=offs, CS=CS, blocks=blocks, order=order,
                ew=ew_flat, src=src_flat)


def _stream_blocks(sched, arrflat, prefix, dtype):
    """Per-core dicts of per-DMA-block contiguous stream arrays."""
    CS = sched["CS"]
    a = arrflat.reshape(NCORE, 128, CS)
    out = []
    for c in range(NCORE):
        d = {}
        for bi, (t0, ntb, c0, bc, runs) in enumerate(sched["blocks"]):
            d[f"{prefix}{bi}"] = np.ascontiguousarray(
                a[c, :, c0:c0 + bc]).astype(dtype)
        out.append(d)
    return out


def _to_core_nodes(val_new, dtype):
    """[NPAD] array in newpos space -> per-core [128, TPC]
    (newpos = s*1024 + c*128 + p)."""
    a = val_new.reshape(TPC, NCORE, 128)
    return [np.ascontiguousarray(a[:, c, :].T).astype(dtype) for c in range(NCORE)]


def _from_core_nodes(parts):
    full = np.empty((TPC, NCORE, 128), np.float32)
    for c in range(NCORE):
        full[:, c, :] = np.asarray(parts[c], np.float32).T
    return full.reshape(NPAD)


def _build_sweep(mode, sched, reps=1, variant=None, unroll=16,
                 skip_b1=True, skip_b2=True):
    """Build the Bass program for one sweep. mode in {deg, layer1, layer2}.
    reps>1 wraps `reps` copies of the (idempotent) body in a hardware For_i
    loop, `unroll` bodies per trip — used only for timing measurements.
    variant (timing experiments only): 'dmaonly' = stream DMA without
    compute, 'reduceonly' = compute without stream DMA."""
    from contextlib import ExitStack

    CS = sched["CS"]
    blocks = sched["blocks"]
    BCMAX = max(b[3] for b in blocks)
    f32 = mybir.dt.float32
    bf = mybir.dt.bfloat16

    nc = bass.Bass("TRN2", target_bir_lowering=False, debug=False,
                   num_devices=NCORE)

    def din(name, shape, dtype=f32):
        return nc.dram_tensor(name, shape, dtype, kind="ExternalInput").ap()

    def dout(name, shape, dtype=f32):
        return nc.dram_tensor(name, shape, dtype, kind="ExternalOutput").ap()

    def din_blocks(prefix):
        return [din(f"{prefix}{bi}", [128, b[3]], bf)
                for bi, b in enumerate(blocks)]

    ew_d = din_blocks("ew")
    if mode == "deg":
        xn_d = [din(f"xn{f}", [128, TPC]) for f in range(D)]
        dinv_out = dout("dinvout", [128, TPC])
        xt_out = [dout(f"xtout{f}", [128, TPC]) for f in range(D)]
    elif mode == "layer1":
        xs_d = [din_blocks(f"xs{f}_") for f in range(D)]
        dinv_d = din("dinvn", [128, TPC])
        w1x_d = [din(f"w1x{f}b", [128, HID * TPC], bf) for f in range(D)]
        b1_d = din("b1b", [128, HID], bf)
        w2x_d = din("w2xb", [128, HID * TPC], bf)
        vt_out = dout("vtout", [128, TPC])
    else:
        vs_d = din_blocks("vs")
        dinv_d = din("dinvn", [128, TPC])
        b2_d = din("b2b", [128, 1])
        y_out = dout("yout", [128, TPC])

    with tile.TileContext(nc) as tc, ExitStack() as ctx:
        const = ctx.enter_context(tc.tile_pool(name="const", bufs=1))
        sp = ctx.enter_context(tc.tile_pool(name="streams", bufs=3))
        wp = ctx.enter_context(tc.tile_pool(name="work", bufs=3))
        accp = ctx.enter_context(tc.tile_pool(name="acc", bufs=2))

        if mode == "deg":
            xN = []
            for f in range(D):
                t_ = const.tile([128, TPC], f32, tag=f"xn{f}")
                nc.sync.dma_start(t_[:], xn_d[f][:])
                xN.append(t_)
        elif mode == "layer1":
            dinvN = const.tile([128, TPC], f32)
            nc.sync.dma_start(dinvN[:], dinv_d[:])
            w1x = []
            for f in range(D):
                t_ = const.tile([128, HID * TPC], bf, tag=f"w1x{f}")
                nc.sync.dma_start(t_[:], w1x_d[f][:])
                w1x.append(t_)
            b1_sb = const.tile([128, HID], bf)
            nc.sync.dma_start(b1_sb[:], b1_d[:])
            w2x_sb = const.tile([128, HID * TPC], bf)
            nc.sync.dma_start(w2x_sb[:], w2x_d[:])
        else:
            dinvN = const.tile([128, TPC], f32)
            nc.sync.dma_start(dinvN[:], dinv_d[:])
            b2_sb = const.tile([128, 1], f32)
            nc.sync.dma_start(b2_sb[:], b2_d[:])

        nF = D if mode == "layer1" else 1

        def _reduce_2stage(m_t, bc, runs, dst_agg, s1tag):
            # Pair-add tree: TensorReduce has no fast DVE mode (1x), but
            # TensorTensor adds on packed bf16 sub-slices run at 2x.  Two
            # tree levels collapse each 8-slot group to 2 partials; a final
            # narrow f32 TensorReduce finishes per G-run.
            u = wp.tile([128, BCMAX // 2], bf, tag=s1tag + "u")
            mv = m_t[:, 0:bc].rearrange("p (q g) -> p q g", g=8)
            nc.vector.tensor_tensor(
                u[:, 0:bc // 2].rearrange("p (q h) -> p q h", h=4),
                mv[:, :, 0:4], mv[:, :, 4:8], mybir.AluOpType.add)
            w = wp.tile([128, BCMAX // 4], bf, tag=s1tag + "w")
            uv = u[:, 0:bc // 2].rearrange("p (q h) -> p q h", h=4)
            nc.vector.tensor_tensor(
                w[:, 0:bc // 4].rearrange("p (q h) -> p q h", h=2),
                uv[:, :, 0:2], uv[:, :, 2:4], mybir.AluOpType.add)
            for (tt, nt, g, ro) in runs:
                nc.vector.tensor_reduce(
                    dst_agg[:, tt:tt + nt],
                    w[:, ro // 4:ro // 4 + nt * (g // 4)].rearrange(
                        "p (t q) -> p t q", q=g // 4),
                    mybir.AxisListType.X, mybir.AluOpType.add)

        def body():
            agg = [accp.tile([128, TPC], f32, tag=f"agg{f}", name=f"agg{f}")
                   for f in range(nF)]
            for bi, (t0, ntb, c0, bc, runs) in enumerate(blocks):
                qa = nc.sync if bi % 2 == 0 else nc.scalar
                qb = nc.scalar if bi % 2 == 0 else nc.sync
                h1 = (bc // 2) & ~7
                ew_t = sp.tile([128, BCMAX], bf, tag="ew")
                if variant != "reduceonly":
                    qa.dma_start(ew_t[:, 0:h1], ew_d[bi][:, 0:h1])
                    qb.dma_start(ew_t[:, h1:bc], ew_d[bi][:, h1:bc])
                else:
                    qa.dma_start(ew_t[:, 0:4], ew_d[bi][:, 0:4])
                if variant == "dmaonly":
                    continue
                if mode == "deg":
                    _reduce_2stage(ew_t, bc, runs, agg[0], "s1a")
                elif mode == "layer1":
                    xs_t = []
                    for f in range(D):
                        t_ = sp.tile([128, BCMAX], bf, tag=f"xs{f}")
                        if variant != "reduceonly":
                            qb.dma_start(t_[:, 0:h1], xs_d[f][bi][:, 0:h1])
                            qa.dma_start(t_[:, h1:bc], xs_d[f][bi][:, h1:bc])
                        else:
                            qb.dma_start(t_[:, 0:4], xs_d[f][bi][:, 0:4])
                        xs_t.append(t_)
                    m0 = wp.tile([128, BCMAX], bf, tag="m0")
                    nc.vector.tensor_mul(m0[:, 0:bc], ew_t[:, 0:bc],
                                         xs_t[0][:, 0:bc])
                    m1 = wp.tile([128, BCMAX], bf, tag="m1")
                    nc.vector.tensor_mul(m1[:, 0:bc], ew_t[:, 0:bc],
                                         xs_t[1][:, 0:bc])
                    _reduce_2stage(m0, bc, runs, agg[0], "s1a")
                    _reduce_2stage(m1, bc, runs, agg[1], "s1b")
                else:
                    vs_t = sp.tile([128, BCMAX], bf, tag="vs")
                    if variant != "reduceonly":
                        qb.dma_start(vs_t[:, 0:h1], vs_d[bi][:, 0:h1])
                        qa.dma_start(vs_t[:, h1:bc], vs_d[bi][:, h1:bc])
                    else:
                        qb.dma_start(vs_t[:, 0:4], vs_d[bi][:, 0:4])
                    m0 = wp.tile([128, BCMAX], bf, tag="m0")
                    nc.vector.tensor_mul(m0[:, 0:bc], ew_t[:, 0:bc],
                                         vs_t[:, 0:bc])
                    _reduce_2stage(m0, bc, runs, agg[0], "s1a")

            if variant in ("dmaonly", "reduceonly", "noepi"):
                return
            # ---- epilogue (self-loop slots make agg complete: deg sweep
            # yields deg+1; layer sweeps include the dinv*val self term) ----
            if mode == "deg":
                sq = wp.tile([128, TPC], f32, tag="sq")
                nc.scalar.activation(sq, agg[0],
                                     mybir.ActivationFunctionType.Sqrt)
                dinv = wp.tile([128, TPC], f32, tag="dinv")
                nc.vector.reciprocal(dinv, sq)
                nc.sync.dma_start(dinv_out[:], dinv[:])
                for f in range(D):
                    xt = wp.tile([128, TPC], f32, tag=f"xt{f}")
                    nc.vector.tensor_mul(xt, xN[f], dinv)
                    nc.sync.dma_start(xt_out[f][:], xt[:])
            elif mode == "layer1":
                zb = []
                for f in range(D):
                    z = wp.tile([128, TPC], f32, tag=f"z{f}")
                    nc.vector.tensor_mul(z, agg[f], dinvN)
                    zb_ = wp.tile([128, TPC], bf, tag=f"zb{f}")
                    nc.vector.tensor_copy(zb_, z)
                    zb.append(zb_)
                # h layout [128, (j t)]: hidden-unit major; weights arrive
                # pre-materialized in the same layout so every product runs
                # in the 2x packed-bf16 DVE mode.
                hA = wp.tile([128, TPC * HID], bf, tag="hA")
                nc.vector.tensor_tensor(
                    hA.rearrange("p (j t) -> p j t", j=HID),
                    zb[0].unsqueeze(1).broadcast_to([128, HID, TPC]),
                    w1x[0].rearrange("p (j t) -> p j t", j=HID),
                    mybir.AluOpType.mult)
                hB = wp.tile([128, TPC * HID], bf, tag="hB")
                nc.vector.tensor_tensor(
                    hB.rearrange("p (j t) -> p j t", j=HID),
                    zb[1].unsqueeze(1).broadcast_to([128, HID, TPC]),
                    w1x[1].rearrange("p (j t) -> p j t", j=HID),
                    mybir.AluOpType.mult)
                nc.vector.tensor_add(hA, hA, hB)
                if not skip_b1:
                    nc.vector.tensor_tensor(
                        hA.rearrange("p (j t) -> p j t", j=HID),
                        hA.rearrange("p (j t) -> p j t", j=HID),
                        b1_sb.unsqueeze(2).broadcast_to([128, HID, TPC]),
                        mybir.AluOpType.add)
                nc.vector.tensor_scalar_max(hA, hA, 0.0)
                hv = wp.tile([128, TPC * HID], bf, tag="hv")
                nc.vector.tensor_tensor(hv, hA, w2x_sb[:],
                                        mybir.AluOpType.mult)
                T8 = 8 * TPC
                va = wp.tile([128, T8], bf, tag="va")
                nc.vector.tensor_add(va, hv[:, 0:T8], hv[:, T8:2 * T8])
                vb = wp.tile([128, T8 // 2], bf, tag="vb")
                nc.vector.tensor_add(vb, va[:, 0:T8 // 2], va[:, T8 // 2:T8])
                vc = wp.tile([128, T8 // 4], bf, tag="vc")
                nc.vector.tensor_add(vc, vb[:, 0:T8 // 4],
                                     vb[:, T8 // 4:T8 // 2])
                vd = wp.tile([128, TPC], f32, tag="vd")
                nc.vector.tensor_add(vd, vc[:, 0:TPC], vc[:, TPC:2 * TPC])
                vt = wp.tile([128, TPC], f32, tag="vt")
                nc.vector.tensor_mul(vt, vd, dinvN)
                nc.sync.dma_start(vt_out[:], vt[:])
            else:
                y = wp.tile([128, TPC], f32, tag="y")
                nc.vector.tensor_mul(y, agg[0], dinvN)
                if not skip_b2:
                    nc.vector.tensor_scalar(y, y, b2_sb[:, 0:1], None,
                                            mybir.AluOpType.add)
                nc.sync.dma_start(y_out[:], y[:])

        if reps > 1:
            assert reps % unroll == 0
            with tc.For_i(0, reps // unroll, 1):
                for _ in range(unroll):
                    body()
        else:
            body()

    _split_multi_waits(nc)
    return nc


def _rep_bf16(vec):
    return np.ascontiguousarray(
        np.tile(np.asarray(vec, np.float32).reshape(1, -1), (128, 1))
    ).astype(BF16)


def kernel(x, edge_index, edge_weight, W1, b1, W2, b2):
    x = np.asarray(x, np.float32)
    edge_index = np.asarray(edge_index)
    edge_weight = np.asarray(edge_weight, np.float32)
    W1 = np.asarray(W1, np.float32)
    b1 = np.asarray(b1, np.float32)
    W2 = np.asarray(W2, np.float32)
    b2 = np.asarray(b2, np.float32)
    skip_b1 = not np.any(b1 != 0)
    skip_b2 = not np.any(b2 != 0)

    pp = _preprocess(edge_index, edge_weight)
    order = pp["order"]

    ew_cs = _stream_blocks(pp, pp["ew"], "ew", BF16)

    xfull = np.zeros((NPAD, D), np.float32)
    xfull[:N] = x
    xnew = xfull[order]                          # newpos layout
    xn_cs = [_to_core_nodes(xnew[:, f], np.float32) for f in range(D)]

    # ---- NEFF 1: deg+1 -> dinv, x*dinv ----
    nc1 = _build_sweep("deg", pp)
    in1 = [dict(ew_cs[c], xn0=xn_cs[0][c], xn1=xn_cs[1][c])
           for c in range(NCORE)]
    r1 = run_bass_kernel_spmd(nc1, in1, core_ids=list(range(NCORE)))
    dinv_new = _from_core_nodes([r1.results[c]["dinvout"] for c in range(NCORE)])
    xt_new = [_from_core_nodes([r1.results[c][f"xtout{f}"] for c in range(NCORE)])
              for f in range(D)]

    # ---- host glue: per-edge (x*dinv)[src] streams ----
    xt_orig = np.empty((NPAD, D), np.float32)
    for f in range(D):
        xt_orig[order, f] = xt_new[f]
    xs_cs = [_stream_blocks(pp, xt_orig[pp["src"], f], f"xs{f}_", BF16)
             for f in range(D)]
    dinv_n = _to_core_nodes(dinv_new, np.float32)

    w1x = [_rep_bf16(np.repeat(W1[f], TPC)) for f in range(D)]
    b1b = _rep_bf16(b1)
    w2xb = _rep_bf16(np.repeat(W2[:, 0], TPC))
    b2b = np.full((128, 1), float(b2[0]), np.float32)

    # ---- NEFF 2: layer 1 -> v*dinv ----
    nc2 = _build_sweep("layer1", pp, skip_b1=skip_b1, skip_b2=skip_b2)
    in2 = [dict(ew_cs[c], **xs_cs[0][c], **xs_cs[1][c],
                dinvn=dinv_n[c], w1x0b=w1x[0], w1x1b=w1x[1], b1b=b1b,
                w2xb=w2xb) for c in range(NCORE)]
    r2 = run_bass_kernel_spmd(nc2, in2, core_ids=list(range(NCORE)))
    vt_new = _from_core_nodes([r2.results[c]["vtout"] for c in range(NCORE)])

    # ---- host glue: (v*dinv)[src] stream ----
    vt_orig = np.empty(NPAD, np.float32)
    vt_orig[order] = vt_new
    vs_cs = _stream_blocks(pp, vt_orig[pp["src"]], "vs", BF16)
    dinv_n2 = dinv_n

    # ---- NEFF 3: layer 2 -> output ----
    nc3 = _build_sweep("layer2", pp, skip_b1=skip_b1, skip_b2=skip_b2)
    in3 = [dict(ew_cs[c], **vs_cs[c], dinvn=dinv_n2[c], b2b=b2b)
           for c in range(NCORE)]
    r3 = run_bass_kernel_spmd(nc3, in3, core_ids=list(range(NCORE)))
    y_new = _from_core_nodes([r3.results[c]["yout"] for c in range(NCORE)])

    y_orig = np.empty(NPAD, np.float32)
    y_orig[order] = y_new
    return y_orig[:N, None].astype(np.float32)


# revision 21
# speedup vs baseline: 65.5423x; 1.0221x over previous
"""GCN (2-layer, PyG gcn_norm) on 8 Trainium2 NeuronCores via Bass.

Strategy (dst-partition-row sharding, no collectives, no PE):
  * Host appends self-loop edges (weight 1, as in gcn_norm), sorts nodes
    by in-degree and assigns each node one SBUF partition-row of G slots
    (G = per-stripe max degree, rounded up), so the per-node segment-sum
    is a plain free-axis tensor_reduce — no one-hot masks, no matmuls.
    Stripes of 1024 nodes (one 128-node tile per core) share a G schedule
    so all 8 SPMD cores run one program.
  * Per-edge streams are bf16 (tolerance is 2e-2); accumulations stay
    f32.  Streams are stored per DMA-block so every transfer is fully
    contiguous, issued alternately on the SP and Activation DGE queues.
  * Three sequential NEFF launches: (1) deg -> dinv, x*dinv, (2) layer-1
    aggregation -> h -> v*dinv, (3) layer-2 aggregation -> output.
    Between launches the host only gathers returned per-node arrays into
    per-edge streams (index-space data movement, no float math).
"""

import sys

sys.path.insert(0, "/opt/trn_rl_repo")

import numpy as np
import ml_dtypes

import concourse.bass as bass
import concourse.tile as tile
from concourse import mybir
from concourse.bass_utils import run_bass_kernel_spmd

BF16 = ml_dtypes.bfloat16

N = 100000
E = 3200000
D = 2
HID = 16
NCORE = 8
TPC = 98                      # stripes == node tiles per core
NPAD = TPC * 1024             # 100352
GMULT = 8                     # stripe slot width rounded up to this
BLK_COLS = 4096               # target stream columns per DMA block (>= CS: single block)


def _split_multi_waits(nc):
    """This toolchain's walrus encodes at most one sync-wait per instruction.
    Hoist extra waits onto fresh single-wait NoOps placed just before."""
    ctr = 0
    for fn in nc.m.functions:
        for bb in fn.blocks:
            insts = list(bb.instructions)
            if not any(
                i.sync_info is not None and len(i.sync_info.on_wait or []) > 1
                for i in insts
            ):
                continue
            new = []
            for inst in insts:
                si = inst.sync_info
                if si is not None and len(si.on_wait or []) > 1:
                    waits = list(si.on_wait)
                    for w in waits[:-1]:
                        ctr += 1
                        new.append(
                            mybir.InstNoOp(
                                name=f"wsplit-{ctr}",
                                engine=inst.engine,
                                sync_info=mybir.SyncInfo(on_wait=[w], on_update=[]),
                                bass_nofuse=True,
                            )
                        )
                    si.on_wait = [waits[-1]]
                new.append(inst)
            bb.instructions = new
    return ctr


def _preprocess(edge_index, edge_weight):
    """Append self-loops, degree-sort nodes, assign each node a
    partition-row slot range, and scatter edge weight / src index into the
    per-core slot streams."""
    loop = np.arange(N, dtype=np.int64)
    dst = np.concatenate([edge_index[1].astype(np.int64), loop])
    src = np.concatenate([edge_index[0].astype(np.int64), loop])
    ew = np.concatenate([edge_weight.astype(np.float32),
                         np.ones(N, np.float32)])
    ne = len(dst)

    deg = np.bincount(dst, minlength=NPAD)
    order = np.argsort(deg, kind="stable")       # newpos -> orig id
    newpos = np.empty(NPAD, np.int64)
    newpos[order] = np.arange(NPAD)

    counts_new = deg[order]                      # per-newpos degree
    smax = counts_new.reshape(TPC, 1024).max(axis=1)
    G = np.maximum(GMULT, ((smax + GMULT - 1) // GMULT) * GMULT).astype(np.int64)
    offs = np.zeros(TPC + 1, np.int64)
    np.cumsum(G, out=offs[1:])
    CS = int(offs[-1])

    nd = newpos[dst]
    start = np.zeros(NPAD + 1, np.int64)
    np.cumsum(counts_new, out=start[1:])
    perm = np.argsort(nd, kind="stable")
    r = np.empty(ne, np.int64)
    r[perm] = np.arange(ne) - start[nd[perm]]    # rank of edge within its dst

    s = nd >> 10
    w = nd & 1023
    c = w >> 7
    p = w & 127
    flat = (c * 128 + p) * CS + offs[s] + r

    ew_flat = np.zeros(NCORE * 128 * CS, np.float32)
    src_flat = np.zeros(NCORE * 128 * CS, np.int64)
    ew_flat[flat] = ew
    src_flat[flat] = src

    # DMA blocks: consecutive stripes until >= BLK_COLS columns; per-block
    # runs of stripes sharing G (one tensor_reduce instruction per run).
    blocks = []
    t0, cols = 0, 0
    for t in range(TPC):
        cols += int(G[t])
        if cols >= BLK_COLS or t == TPC - 1:
            runs = []
            ro = 0
            for tt in range(t0, t + 1):
                g = int(G[tt])
                if runs and runs[-1][2] == g:
                    runs[-1] = (runs[-1][0], runs[-1][1] + 1, g, runs[-1][3])
                else:
                    runs.append((tt, 1, g, ro))
                ro += g
            blocks.append((t0, t + 1 - t0, int(offs[t0]), cols, runs))
            t0, cols = t + 1, 0

    return dict(G=G, offs=offs, CS=CS, blocks=blocks, order=order,
                ew=ew_flat, src=src_flat)


def _stream_blocks(sched, arrflat, prefix, dtype):
    """Per-core dicts of per-DMA-block contiguous stream arrays."""
    CS = sched["CS"]
    a = arrflat.reshape(NCORE, 128, CS)
    out = []
    for c in range(NCORE):
        d = {}
        for bi, (t0, ntb, c0, bc, runs) in enumerate(sched["blocks"]):
            d[f"{prefix}{bi}"] = np.ascontiguousarray(
                a[c, :, c0:c0 + bc]).astype(dtype)
        out.append(d)
    return out


def _to_core_nodes(val_new, dtype):
    """[NPAD] array in newpos space -> per-core [128, TPC]
    (newpos = s*1024 + c*128 + p)."""
    a = val_new.reshape(TPC, NCORE, 128)
    return [np.ascontiguousarray(a[:, c, :].T).astype(dtype) for c in range(NCORE)]


def _from_core_nodes(parts):
    full = np.empty((TPC, NCORE, 128), np.float32)
    for c in range(NCORE):
        full[:, c, :] = np.asarray(parts[c], np.float32).T
    return full.reshape(NPAD)


def _build_sweep(mode, sched, reps=1, variant=None, unroll=16,
                 skip_b1=True, skip_b2=True):
    """Build the Bass program for one sweep. mode in {deg, layer1, layer2}.
    reps>1 wraps `reps` copies of the (idempotent) body in a hardware For_i
    loop, `unroll` bodies per trip — used only for timing measurements.
    variant (timing experiments only): 'dmaonly' = stream DMA without
    compute, 'reduceonly' = compute without stream DMA."""
    from contextlib import ExitStack

    CS = sched["CS"]
    blocks = sched["blocks"]
    BCMAX = max(b[3] for b in blocks)
    f32 = mybir.dt.float32
    bf = mybir.dt.bfloat16

    nc = bass.Bass("TRN2", target_bir_lowering=False, debug=False,
                   num_devices=NCORE)

    def din(name, shape, dtype=f32):
        return nc.dram_tensor(name, shape, dtype, kind="ExternalInput").ap()

    def dout(name, shape, dtype=f32):
        return nc.dram_tensor(name, shape, dtype, kind="ExternalOutput").ap()

    def din_blocks(prefix):
        return [din(f"{prefix}{bi}", [128, b[3]], bf)
                for bi, b in enumerate(blocks)]

    ew_d = din_blocks("ew")
    if mode == "deg":
        xn_d = [din(f"xn{f}", [128, TPC]) for f in range(D)]
        deg_out = dout("degout", [128, (D + 1) * TPC])
    elif mode == "layer1":
        xs_d = [din_blocks(f"xs{f}_") for f in range(D)]
        dinv_d = din("dinvn", [128, TPC])
        w1x_d = [din(f"w1x{f}b", [128, HID * TPC], bf) for f in range(D)]
        b1_d = din("b1b", [128, HID], bf)
        w2x_d = din("w2xb", [128, HID * TPC], bf)
        vt_out = dout("vtout", [128, TPC])
    else:
        vs_d = din_blocks("vs")
        dinv_d = din("dinvn", [128, TPC])
        b2_d = din("b2b", [128, 1])
        y_out = dout("yout", [128, TPC])

    with tile.TileContext(nc) as tc, ExitStack() as ctx:
        nbuf = 3 if mode == "layer1" else 4
        const = ctx.enter_context(tc.tile_pool(name="const", bufs=1))
        sp = ctx.enter_context(tc.tile_pool(name="streams", bufs=nbuf))
        wp = ctx.enter_context(tc.tile_pool(name="work", bufs=nbuf))
        accp = ctx.enter_context(tc.tile_pool(name="acc", bufs=2))

        if mode == "deg":
            xN = []
            for f in range(D):
                t_ = const.tile([128, TPC], f32, tag=f"xn{f}")
                nc.sync.dma_start(t_[:], xn_d[f][:])
                xN.append(t_)
        elif mode == "layer1":
            dinvN = const.tile([128, TPC], f32)
            nc.sync.dma_start(dinvN[:], dinv_d[:])
            w1x = []
            for f in range(D):
                t_ = const.tile([128, HID * TPC], bf, tag=f"w1x{f}")
                nc.sync.dma_start(t_[:], w1x_d[f][:])
                w1x.append(t_)
            b1_sb = const.tile([128, HID], bf)
            nc.sync.dma_start(b1_sb[:], b1_d[:])
            w2x_sb = const.tile([128, HID * TPC], bf)
            nc.sync.dma_start(w2x_sb[:], w2x_d[:])
        else:
            dinvN = const.tile([128, TPC], f32)
            nc.sync.dma_start(dinvN[:], dinv_d[:])
            b2_sb = const.tile([128, 1], f32)
            nc.sync.dma_start(b2_sb[:], b2_d[:])

        nF = D if mode == "layer1" else 1

        def _reduce_2stage(m_t, bc, runs, dst_agg, s1tag):
            # Pair-add tree: TensorReduce has no fast DVE mode (1x), but
            # TensorTensor adds on packed bf16 sub-slices run at 2x.  Two
            # tree levels collapse each 8-slot group to 2 partials; a final
            # narrow f32 TensorReduce finishes per G-run.
            u = wp.tile([128, BCMAX // 2], bf, tag=s1tag + "u")
            mv = m_t[:, 0:bc].rearrange("p (q g) -> p q g", g=8)
            nc.vector.tensor_tensor(
                u[:, 0:bc // 2].rearrange("p (q h) -> p q h", h=4),
                mv[:, :, 0:4], mv[:, :, 4:8], mybir.AluOpType.add)
            w = wp.tile([128, BCMAX // 4], bf, tag=s1tag + "w")
            uv = u[:, 0:bc // 2].rearrange("p (q h) -> p q h", h=4)
            nc.vector.tensor_tensor(
                w[:, 0:bc // 4].rearrange("p (q h) -> p q h", h=2),
                uv[:, :, 0:2], uv[:, :, 2:4], mybir.AluOpType.add)
            for (tt, nt, g, ro) in runs:
                nc.vector.tensor_reduce(
                    dst_agg[:, tt:tt + nt],
                    w[:, ro // 4:ro // 4 + nt * (g // 4)].rearrange(
                        "p (t q) -> p t q", q=g // 4),
                    mybir.AxisListType.X, mybir.AluOpType.add)

        def body():
            agg = [accp.tile([128, TPC], f32, tag=f"agg{f}", name=f"agg{f}")
                   for f in range(nF)]
            for bi, (t0, ntb, c0, bc, runs) in enumerate(blocks):
                qa = nc.sync if bi % 2 == 0 else nc.scalar
                qb = nc.scalar if bi % 2 == 0 else nc.sync
                h1 = (bc // 2) & ~7
                ew_t = sp.tile([128, BCMAX], bf, tag="ew")
                if variant != "reduceonly":
                    qa.dma_start(ew_t[:, 0:h1], ew_d[bi][:, 0:h1])
                    qb.dma_start(ew_t[:, h1:bc], ew_d[bi][:, h1:bc])
                else:
                    qa.dma_start(ew_t[:, 0:4], ew_d[bi][:, 0:4])
                if variant == "dmaonly":
                    continue
                if mode == "deg":
                    _reduce_2stage(ew_t, bc, runs, agg[0], "s1a")
                elif mode == "layer1":
                    xs_t = []
                    for f in range(D):
                        t_ = sp.tile([128, BCMAX], bf, tag=f"xs{f}")
                        if variant != "reduceonly":
                            qb.dma_start(t_[:, 0:h1], xs_d[f][bi][:, 0:h1])
                            qa.dma_start(t_[:, h1:bc], xs_d[f][bi][:, h1:bc])
                        else:
                            qb.dma_start(t_[:, 0:4], xs_d[f][bi][:, 0:4])
                        xs_t.append(t_)
                    m0 = wp.tile([128, BCMAX], bf, tag="m0")
                    nc.vector.tensor_mul(m0[:, 0:bc], ew_t[:, 0:bc],
                                         xs_t[0][:, 0:bc])
                    m1 = wp.tile([128, BCMAX], bf, tag="m1")
                    nc.vector.tensor_mul(m1[:, 0:bc], ew_t[:, 0:bc],
                                         xs_t[1][:, 0:bc])
                    _reduce_2stage(m0, bc, runs, agg[0], "s1a")
                    _reduce_2stage(m1, bc, runs, agg[1], "s1b")
                else:
                    vs_t = sp.tile([128, BCMAX], bf, tag="vs")
                    if variant != "reduceonly":
                        qb.dma_start(vs_t[:, 0:h1], vs_d[bi][:, 0:h1])
                        qa.dma_start(vs_t[:, h1:bc], vs_d[bi][:, h1:bc])
                    else:
                        qb.dma_start(vs_t[:, 0:4], vs_d[bi][:, 0:4])
                    m0 = wp.tile([128, BCMAX], bf, tag="m0")
                    nc.vector.tensor_mul(m0[:, 0:bc], ew_t[:, 0:bc],
                                         vs_t[:, 0:bc])
                    _reduce_2stage(m0, bc, runs, agg[0], "s1a")

            if variant in ("dmaonly", "reduceonly", "noepi"):
                return
            # ---- epilogue (self-loop slots make agg complete: deg sweep
            # yields deg+1; layer sweeps include the dinv*val self term) ----
            if mode == "deg":
                sq = wp.tile([128, TPC], f32, tag="sq")
                nc.scalar.activation(sq, agg[0],
                                     mybir.ActivationFunctionType.Sqrt)
                pk = wp.tile([128, (D + 1) * TPC], f32, tag="pk")
                nc.vector.reciprocal(pk[:, 0:TPC], sq)
                for f in range(D):
                    nc.vector.tensor_mul(pk[:, (1 + f) * TPC:(2 + f) * TPC],
                                         xN[f], pk[:, 0:TPC])
                nc.sync.dma_start(deg_out[:], pk[:])
            elif mode == "layer1":
                zb = []
                for f in range(D):
                    zb_ = wp.tile([128, TPC], bf, tag=f"zb{f}")
                    nc.vector.tensor_mul(zb_, agg[f], dinvN)
                    zb.append(zb_)
                # h layout [128, (j t)]: hidden-unit major; weights arrive
                # pre-materialized in the same layout so every product runs
                # in the 2x packed-bf16 DVE mode.
                hA = wp.tile([128, TPC * HID], bf, tag="hA")
                nc.vector.tensor_tensor(
                    hA.rearrange("p (j t) -> p j t", j=HID),
                    zb[0].unsqueeze(1).broadcast_to([128, HID, TPC]),
                    w1x[0].rearrange("p (j t) -> p j t", j=HID),
                    mybir.AluOpType.mult)
                hB = wp.tile([128, TPC * HID], bf, tag="hB")
                nc.vector.tensor_tensor(
                    hB.rearrange("p (j t) -> p j t", j=HID),
                    zb[1].unsqueeze(1).broadcast_to([128, HID, TPC]),
                    w1x[1].rearrange("p (j t) -> p j t", j=HID),
                    mybir.AluOpType.mult)
                nc.vector.tensor_add(hA, hA, hB)
                if not skip_b1:
                    nc.vector.tensor_tensor(
                        hA.rearrange("p (j t) -> p j t", j=HID),
                        hA.rearrange("p (j t) -> p j t", j=HID),
                        b1_sb.unsqueeze(2).broadcast_to([128, HID, TPC]),
                        mybir.AluOpType.add)
                nc.vector.tensor_scalar_max(hA, hA, 0.0)
                hv = wp.tile([128, TPC * HID], bf, tag="hv")
                nc.vector.tensor_tensor(hv, hA, w2x_sb[:],
                                        mybir.AluOpType.mult)
                T8 = 8 * TPC
                va = wp.tile([128, T8], bf, tag="va")
                nc.vector.tensor_add(va, hv[:, 0:T8], hv[:, T8:2 * T8])
                vb = wp.tile([128, T8 // 2], bf, tag="vb")
                nc.vector.tensor_add(vb, va[:, 0:T8 // 2], va[:, T8 // 2:T8])
                vc = wp.tile([128, T8 // 4], bf, tag="vc")
                nc.vector.tensor_add(vc, vb[:, 0:T8 // 4],
                                     vb[:, T8 // 4:T8 // 2])
                vd = wp.tile([128, TPC], f32, tag="vd")
                nc.vector.tensor_add(vd, vc[:, 0:TPC], vc[:, TPC:2 * TPC])
                vt = wp.tile([128, TPC], f32, tag="vt")
                nc.vector.tensor_mul(vt, vd, dinvN)
                nc.sync.dma_start(vt_out[:], vt[:])
            else:
                y = wp.tile([128, TPC], f32, tag="y")
                nc.vector.tensor_mul(y, agg[0], dinvN)
                if not skip_b2:
                    nc.vector.tensor_scalar(y, y, b2_sb[:, 0:1], None,
                                            mybir.AluOpType.add)
                nc.sync.dma_start(y_out[:], y[:])

        if reps > 1:
            assert reps % unroll == 0
            with tc.For_i(0, reps // unroll, 1):
                for _ in range(unroll):
                    body()
        else:
            body()

    _split_multi_waits(nc)
    return nc


def _rep_bf16(vec):
    return np.ascontiguousarray(
        np.tile(np.asarray(vec, np.float32).reshape(1, -1), (128, 1))
    ).astype(BF16)


def kernel(x, edge_index, edge_weight, W1, b1, W2, b2):
    x = np.asarray(x, np.float32)
    edge_index = np.asarray(edge_index)
    edge_weight = np.asarray(edge_weight, np.float32)
    W1 = np.asarray(W1, np.float32)
    b1 = np.asarray(b1, np.float32)
    W2 = np.asarray(W2, np.float32)
    b2 = np.asarray(b2, np.float32)
    skip_b1 = not np.any(b1 != 0)
    skip_b2 = not np.any(b2 != 0)

    pp = _preprocess(edge_index, edge_weight)
    order = pp["order"]

    ew_cs = _stream_blocks(pp, pp["ew"], "ew", BF16)

    xfull = np.zeros((NPAD, D), np.float32)
    xfull[:N] = x
    xnew = xfull[order]                          # newpos layout
    xn_cs = [_to_core_nodes(xnew[:, f], np.float32) for f in range(D)]

    # ---- NEFF 1: deg+1 -> dinv, x*dinv ----
    nc1 = _build_sweep("deg", pp)
    in1 = [dict(ew_cs[c], xn0=xn_cs[0][c], xn1=xn_cs[1][c])
           for c in range(NCORE)]
    r1 = run_bass_kernel_spmd(nc1, in1, core_ids=list(range(NCORE)))
    pk = [np.asarray(r1.results[c]["degout"], np.float32) for c in range(NCORE)]
    dinv_new = _from_core_nodes([p[:, 0:TPC] for p in pk])
    xt_new = [_from_core_nodes([p[:, (1 + f) * TPC:(2 + f) * TPC] for p in pk])
              for f in range(D)]

    # ---- host glue: per-edge (x*dinv)[src] streams ----
    xt_orig = np.empty((NPAD, D), np.float32)
    for f in range(D):
        xt_orig[order, f] = xt_new[f]
    xs_cs = [_stream_blocks(pp, xt_orig[pp["src"], f], f"xs{f}_", BF16)
             for f in range(D)]
    dinv_n = _to_core_nodes(dinv_new, np.float32)

    w1x = [_rep_bf16(np.repeat(W1[f], TPC)) for f in range(D)]
    b1b = _rep_bf16(b1)
    w2xb = _rep_bf16(np.repeat(W2[:, 0], TPC))
    b2b = np.full((128, 1), float(b2[0]), np.float32)

    # ---- NEFF 2: layer 1 -> v*dinv ----
    nc2 = _build_sweep("layer1", pp, skip_b1=skip_b1, skip_b2=skip_b2)
    in2 = [dict(ew_cs[c], **xs_cs[0][c], **xs_cs[1][c],
                dinvn=dinv_n[c], w1x0b=w1x[0], w1x1b=w1x[1], b1b=b1b,
                w2xb=w2xb) for c in range(NCORE)]
    r2 = run_bass_kernel_spmd(nc2, in2, core_ids=list(range(NCORE)))
    vt_new = _from_core_nodes([r2.results[c]["vtout"] for c in range(NCORE)])

    # ---- host glue: (v*dinv)[src] stream ----
    vt_orig = np.empty(NPAD, np.float32)
    vt_orig[order] = vt_new
    vs_cs = _stream_blocks(pp, vt_orig[pp["src"]], "vs", BF16)
    dinv_n2 = dinv_n

    # ---- NEFF 3: layer 2 -> output ----
    nc3 = _build_sweep("layer2", pp, skip_b1=skip_b1, skip_b2=skip_b2)
    in3 = [dict(ew_cs[c], **vs_cs[c], dinvn=dinv_n2[c], b2b=b2b)
           for c in range(NCORE)]
    r3 = run_bass_kernel_spmd(nc3, in3, core_ids=list(range(NCORE)))
    y_new = _from_core_nodes([r3.results[c]["yout"] for c in range(NCORE)])

    y_orig = np.empty(NPAD, np.float32)
    y_orig[order] = y_new
    return y_orig[:N, None].astype(np.float32)


# revision 23
# speedup vs baseline: 66.8172x; 1.0195x over previous
"""GCN (2-layer, PyG gcn_norm) on 8 Trainium2 NeuronCores via Bass.

Strategy (dst-partition-row sharding, no collectives, no PE):
  * Host appends self-loop edges (weight 1, as in gcn_norm), sorts nodes
    by in-degree and assigns each node one SBUF partition-row of G slots
    (G = per-stripe max degree rounded up to 8; ~12% padding), so the
    per-node segment-sum needs no one-hot masks or matmuls.  Stripes of
    1024 nodes (one 128-node tile per core) share a G schedule so all 8
    SPMD cores run one program.
  * Per-edge streams are bf16 (tolerance is 2e-2).  The segment-sum runs
    as a packed-bf16 pair-add tree on DVE (tensor_tensor at the 2x rate;
    TensorReduce itself has no fast mode) with a final narrow f32
    tensor_reduce per equal-G run.  The layer-1 node epilogue
    (z->W1->relu->W2) uses weight tiles pre-materialized in (hidden,
    tile) layout so every product is a packed 2x tensor_tensor.
  * Streams transfer as whole contiguous blocks split across the two
    hardware DGE queues (SP + Activation); the Activation engine is kept
    compute-free so its queue can prefetch ahead of the epilogue.
  * Three sequential NEFF launches: (1) deg -> dinv, x*dinv, (2) layer-1
    aggregation -> h -> v*dinv, (3) layer-2 aggregation -> output.
    Between launches the host only gathers returned per-node arrays into
    per-edge streams (index-space data movement, no float math).
"""

import sys

sys.path.insert(0, "/opt/trn_rl_repo")

import numpy as np
import ml_dtypes

import concourse.bass as bass
import concourse.tile as tile
from concourse import mybir
from concourse.bass_utils import run_bass_kernel_spmd

BF16 = ml_dtypes.bfloat16

N = 100000
E = 3200000
D = 2
HID = 16
NCORE = 8
TPC = 98                      # stripes == node tiles per core
NPAD = TPC * 1024             # 100352
GMULT = 8                     # stripe slot width rounded up to this
BLK_COLS = 4096               # target stream columns per DMA block (>= CS: single block)


def _split_multi_waits(nc):
    """This toolchain's walrus encodes at most one sync-wait per instruction.
    Hoist extra waits onto fresh single-wait NoOps placed just before."""
    ctr = 0
    for fn in nc.m.functions:
        for bb in fn.blocks:
            insts = list(bb.instructions)
            if not any(
                i.sync_info is not None and len(i.sync_info.on_wait or []) > 1
                for i in insts
            ):
                continue
            new = []
            for inst in insts:
                si = inst.sync_info
                if si is not None and len(si.on_wait or []) > 1:
                    waits = list(si.on_wait)
                    for w in waits[:-1]:
                        ctr += 1
                        new.append(
                            mybir.InstNoOp(
                                name=f"wsplit-{ctr}",
                                engine=inst.engine,
                                sync_info=mybir.SyncInfo(on_wait=[w], on_update=[]),
                                bass_nofuse=True,
                            )
                        )
                    si.on_wait = [waits[-1]]
                new.append(inst)
            bb.instructions = new
    return ctr


def _preprocess(edge_index, edge_weight):
    """Append self-loops, degree-sort nodes, assign each node a
    partition-row slot range, and scatter edge weight / src index into the
    per-core slot streams."""
    loop = np.arange(N, dtype=np.int64)
    dst = np.concatenate([edge_index[1].astype(np.int64), loop])
    src = np.concatenate([edge_index[0].astype(np.int64), loop])
    ew = np.concatenate([edge_weight.astype(np.float32),
                         np.ones(N, np.float32)])
    ne = len(dst)

    deg = np.bincount(dst, minlength=NPAD)
    order = np.argsort(deg, kind="stable")       # newpos -> orig id
    newpos = np.empty(NPAD, np.int64)
    newpos[order] = np.arange(NPAD)

    counts_new = deg[order]                      # per-newpos degree
    smax = counts_new.reshape(TPC, 1024).max(axis=1)
    G = np.maximum(GMULT, ((smax + GMULT - 1) // GMULT) * GMULT).astype(np.int64)
    offs = np.zeros(TPC + 1, np.int64)
    np.cumsum(G, out=offs[1:])
    CS = int(offs[-1])

    nd = newpos[dst]
    start = np.zeros(NPAD + 1, np.int64)
    np.cumsum(counts_new, out=start[1:])
    perm = np.argsort(nd, kind="stable")
    r = np.empty(ne, np.int64)
    r[perm] = np.arange(ne) - start[nd[perm]]    # rank of edge within its dst

    s = nd >> 10
    w = nd & 1023
    c = w >> 7
    p = w & 127
    flat = (c * 128 + p) * CS + offs[s] + r

    ew_flat = np.zeros(NCORE * 128 * CS, np.float32)
    src_flat = np.zeros(NCORE * 128 * CS, np.int64)
    ew_flat[flat] = ew
    src_flat[flat] = src

    # DMA blocks: consecutive stripes until >= BLK_COLS columns; per-block
    # runs of stripes sharing G (one tensor_reduce instruction per run).
    blocks = []
    t0, cols = 0, 0
    for t in range(TPC):
        cols += int(G[t])
        if cols >= BLK_COLS or t == TPC - 1:
            runs = []
            ro = 0
            for tt in range(t0, t + 1):
                g = int(G[tt])
                if runs and runs[-1][2] == g:
                    runs[-1] = (runs[-1][0], runs[-1][1] + 1, g, runs[-1][3])
                else:
                    runs.append((tt, 1, g, ro))
                ro += g
            blocks.append((t0, t + 1 - t0, int(offs[t0]), cols, runs))
            t0, cols = t + 1, 0

    return dict(G=G, offs=offs, CS=CS, blocks=blocks, order=order,
                ew=ew_flat, src=src_flat)


def _stream_blocks(sched, arrflat, prefix, dtype):
    """Per-core dicts of per-DMA-block contiguous stream arrays."""
    CS = sched["CS"]
    a = arrflat.reshape(NCORE, 128, CS)
    out = []
    for c in range(NCORE):
        d = {}
        for bi, (t0, ntb, c0, bc, runs) in enumerate(sched["blocks"]):
            d[f"{prefix}{bi}"] = np.ascontiguousarray(
                a[c, :, c0:c0 + bc]).astype(dtype)
        out.append(d)
    return out


def _to_core_nodes(val_new, dtype):
    """[NPAD] array in newpos space -> per-core [128, TPC]
    (newpos = s*1024 + c*128 + p)."""
    a = val_new.reshape(TPC, NCORE, 128)
    return [np.ascontiguousarray(a[:, c, :].T).astype(dtype) for c in range(NCORE)]


def _from_core_nodes(parts):
    full = np.empty((TPC, NCORE, 128), np.float32)
    for c in range(NCORE):
        full[:, c, :] = np.asarray(parts[c], np.float32).T
    return full.reshape(NPAD)


def _build_sweep(mode, sched, reps=1, variant=None, unroll=16,
                 skip_b1=True, skip_b2=True):
    """Build the Bass program for one sweep. mode in {deg, layer1, layer2}.
    reps>1 wraps `reps` copies of the (idempotent) body in a hardware For_i
    loop, `unroll` bodies per trip — used only for timing measurements.
    variant (timing experiments only): 'dmaonly' = stream DMA without
    compute, 'reduceonly' = compute without stream DMA."""
    from contextlib import ExitStack

    CS = sched["CS"]
    blocks = sched["blocks"]
    BCMAX = max(b[3] for b in blocks)
    f32 = mybir.dt.float32
    bf = mybir.dt.bfloat16

    nc = bass.Bass("TRN2", target_bir_lowering=False, debug=False,
                   num_devices=NCORE)

    def din(name, shape, dtype=f32):
        return nc.dram_tensor(name, shape, dtype, kind="ExternalInput").ap()

    def dout(name, shape, dtype=f32):
        return nc.dram_tensor(name, shape, dtype, kind="ExternalOutput").ap()

    def din_blocks(prefix):
        return [din(f"{prefix}{bi}", [128, b[3]], bf)
                for bi, b in enumerate(blocks)]

    ew_d = din_blocks("ew")
    if mode == "deg":
        xn_d = [din(f"xn{f}", [128, TPC]) for f in range(D)]
        deg_out = dout("degout", [128, (D + 1) * TPC])
    elif mode == "layer1":
        xs_d = [din_blocks(f"xs{f}_") for f in range(D)]
        dinv_d = din("dinvn", [128, TPC])
        w1x_d = [din(f"w1x{f}b", [128, HID * TPC], bf) for f in range(D)]
        b1_d = din("b1b", [128, HID], bf)
        w2x_d = din("w2xb", [128, HID * TPC], bf)
        vt_out = dout("vtout", [128, TPC])
    else:
        vs_d = din_blocks("vs")
        dinv_d = din("dinvn", [128, TPC])
        b2_d = din("b2b", [128, 1])
        y_out = dout("yout", [128, TPC])

    with tile.TileContext(nc) as tc, ExitStack() as ctx:
        nbuf = 3 if mode == "layer1" else 4
        const = ctx.enter_context(tc.tile_pool(name="const", bufs=1))
        sp = ctx.enter_context(tc.tile_pool(name="streams", bufs=nbuf))
        wp = ctx.enter_context(tc.tile_pool(name="work", bufs=nbuf))
        accp = ctx.enter_context(tc.tile_pool(name="acc", bufs=2))

        if mode == "deg":
            xN = []
            for f in range(D):
                t_ = const.tile([128, TPC], f32, tag=f"xn{f}")
                nc.sync.dma_start(t_[:], xn_d[f][:])
                xN.append(t_)
        elif mode == "layer1":
            dinvN = const.tile([128, TPC], f32)
            nc.sync.dma_start(dinvN[:], dinv_d[:])
            w1x = []
            for f in range(D):
                t_ = const.tile([128, HID * TPC], bf, tag=f"w1x{f}")
                nc.sync.dma_start(t_[:], w1x_d[f][:])
                w1x.append(t_)
            b1_sb = const.tile([128, HID], bf)
            nc.sync.dma_start(b1_sb[:], b1_d[:])
            w2x_sb = const.tile([128, HID * TPC], bf)
            nc.sync.dma_start(w2x_sb[:], w2x_d[:])
        else:
            dinvN = const.tile([128, TPC], f32)
            nc.sync.dma_start(dinvN[:], dinv_d[:])
            b2_sb = const.tile([128, 1], f32)
            nc.sync.dma_start(b2_sb[:], b2_d[:])

        nF = D if mode == "layer1" else 1

        def _reduce_2stage(m_t, bc, runs, dst_agg, s1tag):
            # Pair-add tree: TensorReduce has no fast DVE mode (1x), but
            # TensorTensor adds on packed bf16 sub-slices run at 2x.  Two
            # tree levels collapse each 8-slot group to 2 partials; a final
            # narrow f32 TensorReduce finishes per G-run.
            u = wp.tile([128, BCMAX // 2], bf, tag=s1tag + "u")
            mv = m_t[:, 0:bc].rearrange("p (q g) -> p q g", g=8)
            nc.vector.tensor_tensor(
                u[:, 0:bc // 2].rearrange("p (q h) -> p q h", h=4),
                mv[:, :, 0:4], mv[:, :, 4:8], mybir.AluOpType.add)
            w = wp.tile([128, BCMAX // 4], bf, tag=s1tag + "w")
            uv = u[:, 0:bc // 2].rearrange("p (q h) -> p q h", h=4)
            nc.vector.tensor_tensor(
                w[:, 0:bc // 4].rearrange("p (q h) -> p q h", h=2),
                uv[:, :, 0:2], uv[:, :, 2:4], mybir.AluOpType.add)
            for (tt, nt, g, ro) in runs:
                nc.vector.tensor_reduce(
                    dst_agg[:, tt:tt + nt],
                    w[:, ro // 4:ro // 4 + nt * (g // 4)].rearrange(
                        "p (t q) -> p t q", q=g // 4),
                    mybir.AxisListType.X, mybir.AluOpType.add)

        def body():
            agg = [accp.tile([128, TPC], f32, tag=f"agg{f}", name=f"agg{f}")
                   for f in range(nF)]
            for bi, (t0, ntb, c0, bc, runs) in enumerate(blocks):
                qa = nc.sync if bi % 2 == 0 else nc.scalar
                qb = nc.scalar if bi % 2 == 0 else nc.sync
                h1 = (bc // 2) & ~7
                ew_t = sp.tile([128, BCMAX], bf, tag="ew")
                if variant == "reduceonly":
                    qa.dma_start(ew_t[:, 0:4], ew_d[bi][:, 0:4])
                elif mode == "layer2":
                    qa.dma_start(ew_t[:, 0:bc], ew_d[bi][:])
                else:
                    qa.dma_start(ew_t[:, 0:h1], ew_d[bi][:, 0:h1])
                    qb.dma_start(ew_t[:, h1:bc], ew_d[bi][:, h1:bc])
                if variant == "dmaonly":
                    continue
                if mode == "deg":
                    _reduce_2stage(ew_t, bc, runs, agg[0], "s1a")
                elif mode == "layer1":
                    xs_t = []
                    for f in range(D):
                        t_ = sp.tile([128, BCMAX], bf, tag=f"xs{f}")
                        if variant != "reduceonly":
                            qb.dma_start(t_[:, 0:h1], xs_d[f][bi][:, 0:h1])
                            qa.dma_start(t_[:, h1:bc], xs_d[f][bi][:, h1:bc])
                        else:
                            qb.dma_start(t_[:, 0:4], xs_d[f][bi][:, 0:4])
                        xs_t.append(t_)
                    m0 = wp.tile([128, BCMAX], bf, tag="m0")
                    nc.vector.tensor_mul(m0[:, 0:bc], ew_t[:, 0:bc],
                                         xs_t[0][:, 0:bc])
                    m1 = wp.tile([128, BCMAX], bf, tag="m1")
                    nc.vector.tensor_mul(m1[:, 0:bc], ew_t[:, 0:bc],
                                         xs_t[1][:, 0:bc])
                    _reduce_2stage(m0, bc, runs, agg[0], "s1a")
                    _reduce_2stage(m1, bc, runs, agg[1], "s1b")
                else:
                    vs_t = sp.tile([128, BCMAX], bf, tag="vs")
                    if variant != "reduceonly":
                        qb.dma_start(vs_t[:, 0:bc], vs_d[bi][:])
                    else:
                        qb.dma_start(vs_t[:, 0:4], vs_d[bi][:, 0:4])
                    m0 = wp.tile([128, BCMAX], bf, tag="m0")
                    nc.vector.tensor_mul(m0[:, 0:bc], ew_t[:, 0:bc],
                                         vs_t[:, 0:bc])
                    _reduce_2stage(m0, bc, runs, agg[0], "s1a")

            if variant in ("dmaonly", "reduceonly", "noepi"):
                return
            # ---- epilogue (self-loop slots make agg complete: deg sweep
            # yields deg+1; layer sweeps include the dinv*val self term) ----
            if mode == "deg":
                sq = wp.tile([128, TPC], f32, tag="sq")
                nc.scalar.activation(sq, agg[0],
                                     mybir.ActivationFunctionType.Sqrt)
                pk = wp.tile([128, (D + 1) * TPC], f32, tag="pk")
                nc.vector.reciprocal(pk[:, 0:TPC], sq)
                for f in range(D):
                    nc.vector.tensor_mul(pk[:, (1 + f) * TPC:(2 + f) * TPC],
                                         xN[f], pk[:, 0:TPC])
                nc.sync.dma_start(deg_out[:], pk[:])
            elif mode == "layer1":
                zb = []
                for f in range(D):
                    zb_ = wp.tile([128, TPC], bf, tag=f"zb{f}")
                    nc.vector.tensor_mul(zb_, agg[f], dinvN)
                    zb.append(zb_)
                # h layout [128, (j t)]: hidden-unit major; weights arrive
                # pre-materialized in the same layout so every product runs
                # in the 2x packed-bf16 DVE mode.
                hA = wp.tile([128, TPC * HID], bf, tag="hA")
                nc.vector.tensor_tensor(
                    hA.rearrange("p (j t) -> p j t", j=HID),
                    zb[0].unsqueeze(1).broadcast_to([128, HID, TPC]),
                    w1x[0].rearrange("p (j t) -> p j t", j=HID),
                    mybir.AluOpType.mult)
                hB = wp.tile([128, TPC * HID], bf, tag="hB")
                nc.vector.tensor_tensor(
                    hB.rearrange("p (j t) -> p j t", j=HID),
                    zb[1].unsqueeze(1).broadcast_to([128, HID, TPC]),
                    w1x[1].rearrange("p (j t) -> p j t", j=HID),
                    mybir.AluOpType.mult)
                nc.vector.tensor_add(hA, hA, hB)
                if not skip_b1:
                    nc.vector.tensor_tensor(
                        hA.rearrange("p (j t) -> p j t", j=HID),
                        hA.rearrange("p (j t) -> p j t", j=HID),
                        b1_sb.unsqueeze(2).broadcast_to([128, HID, TPC]),
                        mybir.AluOpType.add)
                nc.vector.tensor_scalar_max(hA, hA, 0.0)
                hv = wp.tile([128, TPC * HID], bf, tag="hv")
                nc.vector.tensor_tensor(hv, hA, w2x_sb[:],
                                        mybir.AluOpType.mult)
                T8 = 8 * TPC
                va = wp.tile([128, T8], bf, tag="va")
                nc.vector.tensor_add(va, hv[:, 0:T8], hv[:, T8:2 * T8])
                vb = wp.tile([128, T8 // 2], bf, tag="vb")
                nc.vector.tensor_add(vb, va[:, 0:T8 // 2], va[:, T8 // 2:T8])
                vc = wp.tile([128, T8 // 4], bf, tag="vc")
                nc.vector.tensor_add(vc, vb[:, 0:T8 // 4],
                                     vb[:, T8 // 4:T8 // 2])
                vd = wp.tile([128, TPC], f32, tag="vd")
                nc.vector.tensor_add(vd, vc[:, 0:TPC], vc[:, TPC:2 * TPC])
                vt = wp.tile([128, TPC], f32, tag="vt")
                nc.vector.tensor_mul(vt, vd, dinvN)
                nc.sync.dma_start(vt_out[:], vt[:])
            else:
                y = wp.tile([128, TPC], f32, tag="y")
                nc.vector.tensor_mul(y, agg[0], dinvN)
                if not skip_b2:
                    nc.vector.tensor_scalar(y, y, b2_sb[:, 0:1], None,
                                            mybir.AluOpType.add)
                nc.sync.dma_start(y_out[:], y[:])

        if reps > 1:
            assert reps % unroll == 0
            with tc.For_i(0, reps // unroll, 1):
                for _ in range(unroll):
                    body()
        else:
            body()

    _split_multi_waits(nc)
    return nc


def _rep_bf16(vec):
    return np.ascontiguousarray(
        np.tile(np.asarray(vec, np.float32).reshape(1, -1), (128, 1))
    ).astype(BF16)


def kernel(x, edge_index, edge_weight, W1, b1, W2, b2):
    x = np.asarray(x, np.float32)
    edge_index = np.asarray(edge_index)
    edge_weight = np.asarray(edge_weight, np.float32)
    W1 = np.asarray(W1, np.float32)
    b1 = np.asarray(b1, np.float32)
    W2 = np.asarray(W2, np.float32)
    b2 = np.asarray(b2, np.float32)
    skip_b1 = not np.any(b1 != 0)
    skip_b2 = not np.any(b2 != 0)

    pp = _preprocess(edge_index, edge_weight)
    order = pp["order"]

    ew_cs = _stream_blocks(pp, pp["ew"], "ew", BF16)

    xfull = np.zeros((NPAD, D), np.float32)
    xfull[:N] = x
    xnew = xfull[order]                          # newpos layout
    xn_cs = [_to_core_nodes(xnew[:, f], np.float32) for f in range(D)]

    # ---- NEFF 1: deg+1 -> dinv, x*dinv ----
    nc1 = _build_sweep("deg", pp)
    in1 = [dict(ew_cs[c], xn0=xn_cs[0][c], xn1=xn_cs[1][c])
           for c in range(NCORE)]
    r1 = run_bass_kernel_spmd(nc1, in1, core_ids=list(range(NCORE)))
    pk = [np.asarray(r1.results[c]["degout"], np.float32) for c in range(NCORE)]
    dinv_new = _from_core_nodes([p[:, 0:TPC] for p in pk])
    xt_new = [_from_core_nodes([p[:, (1 + f) * TPC:(2 + f) * TPC] for p in pk])
              for f in range(D)]

    # ---- host glue: per-edge (x*dinv)[src] streams ----
    xt_orig = np.empty((NPAD, D), np.float32)
    for f in range(D):
        xt_orig[order, f] = xt_new[f]
    xs_cs = [_stream_blocks(pp, xt_orig[pp["src"], f], f"xs{f}_", BF16)
             for f in range(D)]
    dinv_n = _to_core_nodes(dinv_new, np.float32)

    w1x = [_rep_bf16(np.repeat(W1[f], TPC)) for f in range(D)]
    b1b = _rep_bf16(b1)
    w2xb = _rep_bf16(np.repeat(W2[:, 0], TPC))
    b2b = np.full((128, 1), float(b2[0]), np.float32)

    # ---- NEFF 2: layer 1 -> v*dinv ----
    nc2 = _build_sweep("layer1", pp, skip_b1=skip_b1, skip_b2=skip_b2)
    in2 = [dict(ew_cs[c], **xs_cs[0][c], **xs_cs[1][c],
                dinvn=dinv_n[c], w1x0b=w1x[0], w1x1b=w1x[1], b1b=b1b,
                w2xb=w2xb) for c in range(NCORE)]
    r2 = run_bass_kernel_spmd(nc2, in2, core_ids=list(range(NCORE)))
    vt_new = _from_core_nodes([r2.results[c]["vtout"] for c in range(NCORE)])

    # ---- host glue: (v*dinv)[src] stream ----
    vt_orig = np.empty(NPAD, np.float32)
    vt_orig[order] = vt_new
    vs_cs = _stream_blocks(pp, vt_orig[pp["src"]], "vs", BF16)
    dinv_n2 = dinv_n

    # ---- NEFF 3: layer 2 -> output ----
    nc3 = _build_sweep("layer2", pp, skip_b1=skip_b1, skip_b2=skip_b2)
    in3 = [dict(ew_cs[c], **vs_cs[c], dinvn=dinv_n2[c], b2b=b2b)
           for c in range(NCORE)]
    r3 = run_bass_kernel_spmd(nc3, in3, core_ids=list(range(NCORE)))
    y_new = _from_core_nodes([r3.results[c]["yout"] for c in range(NCORE)])

    y_orig = np.empty(NPAD, np.float32)
    y_orig[order] = y_new
    return y_orig[:N, None].astype(np.float32)


# revision 24
# speedup vs baseline: 76.1377x; 1.1395x over previous
"""GCN (2-layer, PyG gcn_norm) on 8 Trainium2 NeuronCores via Bass.

Strategy (dst-partition-row sharding, no collectives, no PE):
  * Host appends self-loop edges (weight 1, as in gcn_norm), sorts nodes
    by in-degree and assigns each node one SBUF partition-row of G slots
    (G = per-stripe max degree rounded up to 8; ~12% padding), so the
    per-node segment-sum needs no one-hot masks or matmuls.  Stripes of
    1024 nodes (one 128-node tile per core) share a G schedule so all 8
    SPMD cores run one program.
  * Per-edge streams are bf16 (tolerance is 2e-2).  The segment-sum runs
    as a packed-bf16 pair-add tree on DVE (tensor_tensor at the 2x rate;
    TensorReduce itself has no fast mode) with a final narrow f32
    tensor_reduce per equal-G run.  The layer-1 node epilogue
    (z->W1->relu->W2) uses weight tiles pre-materialized in (hidden,
    tile) layout so every product is a packed 2x tensor_tensor.
  * Streams transfer as whole contiguous blocks split across the two
    hardware DGE queues (SP + Activation); the Activation engine is kept
    compute-free so its queue can prefetch ahead of the epilogue.
  * Three sequential NEFF launches: (1) deg -> dinv, x*dinv, (2) layer-1
    aggregation -> h -> v*dinv, (3) layer-2 aggregation -> output.
    Between launches the host only gathers returned per-node arrays into
    per-edge streams (index-space data movement, no float math).
"""

import sys

sys.path.insert(0, "/opt/trn_rl_repo")

import numpy as np
import ml_dtypes

import concourse.bass as bass
import concourse.tile as tile
from concourse import mybir
from concourse.bass_utils import run_bass_kernel_spmd

BF16 = ml_dtypes.bfloat16

N = 100000
E = 3200000
D = 2
HID = 16
NCORE = 8
TPC = 98                      # stripes == node tiles per core
NPAD = TPC * 1024             # 100352
GMULT = 8                     # stripe slot width rounded up to this
BLK_COLS = 4096               # target stream columns per DMA block (>= CS: single block)


def _split_multi_waits(nc):
    """This toolchain's walrus encodes at most one sync-wait per instruction.
    Hoist extra waits onto fresh single-wait NoOps placed just before."""
    ctr = 0
    for fn in nc.m.functions:
        for bb in fn.blocks:
            insts = list(bb.instructions)
            if not any(
                i.sync_info is not None and len(i.sync_info.on_wait or []) > 1
                for i in insts
            ):
                continue
            new = []
            for inst in insts:
                si = inst.sync_info
                if si is not None and len(si.on_wait or []) > 1:
                    waits = list(si.on_wait)
                    for w in waits[:-1]:
                        ctr += 1
                        new.append(
                            mybir.InstNoOp(
                                name=f"wsplit-{ctr}",
                                engine=inst.engine,
                                sync_info=mybir.SyncInfo(on_wait=[w], on_update=[]),
                                bass_nofuse=True,
                            )
                        )
                    si.on_wait = [waits[-1]]
                new.append(inst)
            bb.instructions = new
    return ctr


def _preprocess(edge_index, edge_weight):
    """Append self-loops, degree-sort nodes, assign each node a
    partition-row slot range, and scatter edge weight / src index into the
    per-core slot streams."""
    loop = np.arange(N, dtype=np.int64)
    dst = np.concatenate([edge_index[1].astype(np.int64), loop])
    src = np.concatenate([edge_index[0].astype(np.int64), loop])
    ew = np.concatenate([edge_weight.astype(np.float32),
                         np.ones(N, np.float32)])
    ne = len(dst)

    deg = np.bincount(dst, minlength=NPAD)
    order = np.argsort(deg, kind="stable")       # newpos -> orig id
    newpos = np.empty(NPAD, np.int64)
    newpos[order] = np.arange(NPAD)

    counts_new = deg[order]                      # per-newpos degree
    smax = counts_new.reshape(TPC, 1024).max(axis=1)
    G = np.maximum(GMULT, ((smax + GMULT - 1) // GMULT) * GMULT).astype(np.int64)
    offs = np.zeros(TPC + 1, np.int64)
    np.cumsum(G, out=offs[1:])
    CS = int(offs[-1])

    nd = newpos[dst]
    start = np.zeros(NPAD + 1, np.int64)
    np.cumsum(counts_new, out=start[1:])
    perm = np.argsort(nd, kind="stable")
    r = np.empty(ne, np.int64)
    r[perm] = np.arange(ne) - start[nd[perm]]    # rank of edge within its dst

    s = nd >> 10
    w = nd & 1023
    c = w >> 7
    p = w & 127
    flat = (c * 128 + p) * CS + offs[s] + r

    ew_flat = np.zeros(NCORE * 128 * CS, np.float32)
    src_flat = np.zeros(NCORE * 128 * CS, np.int64)
    ew_flat[flat] = ew
    src_flat[flat] = src

    # DMA blocks: consecutive stripes until >= BLK_COLS columns; per-block
    # runs of stripes sharing G (one tensor_reduce instruction per run).
    blocks = []
    t0, cols = 0, 0
    for t in range(TPC):
        cols += int(G[t])
        if cols >= BLK_COLS or t == TPC - 1:
            runs = []
            ro = 0
            for tt in range(t0, t + 1):
                g = int(G[tt])
                if runs and runs[-1][2] == g:
                    runs[-1] = (runs[-1][0], runs[-1][1] + 1, g, runs[-1][3])
                else:
                    runs.append((tt, 1, g, ro))
                ro += g
            blocks.append((t0, t + 1 - t0, int(offs[t0]), cols, runs))
            t0, cols = t + 1, 0

    return dict(G=G, offs=offs, CS=CS, blocks=blocks, order=order,
                ew=ew_flat, src=src_flat)


def _stream_blocks(sched, arrflat, prefix, dtype):
    """Per-core dicts of per-DMA-block contiguous stream arrays."""
    CS = sched["CS"]
    a = arrflat.reshape(NCORE, 128, CS)
    out = []
    for c in range(NCORE):
        d = {}
        for bi, (t0, ntb, c0, bc, runs) in enumerate(sched["blocks"]):
            d[f"{prefix}{bi}"] = np.ascontiguousarray(
                a[c, :, c0:c0 + bc]).astype(dtype)
        out.append(d)
    return out


def _to_core_nodes(val_new, dtype):
    """[NPAD] array in newpos space -> per-core [128, TPC]
    (newpos = s*1024 + c*128 + p)."""
    a = val_new.reshape(TPC, NCORE, 128)
    return [np.ascontiguousarray(a[:, c, :].T).astype(dtype) for c in range(NCORE)]


def _from_core_nodes(parts):
    full = np.empty((TPC, NCORE, 128), np.float32)
    for c in range(NCORE):
        full[:, c, :] = np.asarray(parts[c], np.float32).T
    return full.reshape(NPAD)


def _build_sweep(mode, sched, reps=1, variant=None, unroll=16,
                 skip_b1=True, skip_b2=True):
    """Build the Bass program for one sweep. mode in {deg, layer1, layer2}.
    reps>1 wraps `reps` copies of the (idempotent) body in a hardware For_i
    loop, `unroll` bodies per trip — used only for timing measurements.
    variant (timing experiments only): 'dmaonly' = stream DMA without
    compute, 'reduceonly' = compute without stream DMA."""
    from contextlib import ExitStack

    CS = sched["CS"]
    blocks = sched["blocks"]
    BCMAX = max(b[3] for b in blocks)
    f32 = mybir.dt.float32
    bf = mybir.dt.bfloat16

    nc = bass.Bass("TRN2", target_bir_lowering=False, debug=False,
                   num_devices=NCORE)

    def din(name, shape, dtype=f32):
        return nc.dram_tensor(name, shape, dtype, kind="ExternalInput").ap()

    def dout(name, shape, dtype=f32):
        return nc.dram_tensor(name, shape, dtype, kind="ExternalOutput").ap()

    def din_blocks(prefix):
        return [din(f"{prefix}{bi}", [128, b[3]], bf)
                for bi, b in enumerate(blocks)]

    ew_d = din_blocks("ew")
    if mode == "deg":
        xn_d = [din(f"xn{f}", [128, TPC]) for f in range(D)]
        deg_out = dout("degout", [128, (D + 1) * TPC])
    elif mode == "layer1":
        xs_d = [din_blocks(f"xs{f}_") for f in range(D)]
        dinv_d = din("dinvn", [128, TPC])
        w1x_d = [din(f"w1x{f}b", [128, HID * TPC], bf) for f in range(D)]
        b1_d = din("b1b", [128, HID], bf)
        w2x_d = din("w2xb", [128, HID * TPC], bf)
        vt_out = dout("vtout", [128, TPC])
    else:
        vs_d = din_blocks("vs")
        dinv_d = din("dinvn", [128, TPC])
        b2_d = din("b2b", [128, 1])
        y_out = dout("yout", [128, TPC])

    with tile.TileContext(nc) as tc, ExitStack() as ctx:
        nbuf = 3 if mode == "layer1" else 6
        const = ctx.enter_context(tc.tile_pool(name="const", bufs=1))
        sp = ctx.enter_context(tc.tile_pool(name="streams", bufs=nbuf))
        wp = ctx.enter_context(tc.tile_pool(name="work", bufs=nbuf))
        accp = ctx.enter_context(tc.tile_pool(name="acc", bufs=2))

        if mode == "deg":
            xN = []
            for f in range(D):
                t_ = const.tile([128, TPC], f32, tag=f"xn{f}")
                nc.sync.dma_start(t_[:], xn_d[f][:])
                xN.append(t_)
        elif mode == "layer1":
            dinvN = const.tile([128, TPC], f32)
            nc.sync.dma_start(dinvN[:], dinv_d[:])
            w1x = []
            for f in range(D):
                t_ = const.tile([128, HID * TPC], bf, tag=f"w1x{f}")
                nc.sync.dma_start(t_[:], w1x_d[f][:])
                w1x.append(t_)
            b1_sb = const.tile([128, HID], bf)
            nc.sync.dma_start(b1_sb[:], b1_d[:])
            w2x_sb = const.tile([128, HID * TPC], bf)
            nc.sync.dma_start(w2x_sb[:], w2x_d[:])
        else:
            dinvN = const.tile([128, TPC], f32)
            nc.sync.dma_start(dinvN[:], dinv_d[:])
            b2_sb = const.tile([128, 1], f32)
            nc.sync.dma_start(b2_sb[:], b2_d[:])

        nF = D if mode == "layer1" else 1

        def _reduce_2stage(m_t, bc, runs, dst_agg, s1tag):
            # Pair-add tree: TensorReduce has no fast DVE mode (1x), but
            # TensorTensor adds on packed bf16 sub-slices run at 2x.  Two
            # tree levels collapse each 8-slot group to 2 partials; a final
            # narrow f32 TensorReduce finishes per G-run.
            u = wp.tile([128, BCMAX // 2], bf, tag=s1tag + "u")
            mv = m_t[:, 0:bc].rearrange("p (q g) -> p q g", g=8)
            nc.vector.tensor_tensor(
                u[:, 0:bc // 2].rearrange("p (q h) -> p q h", h=4),
                mv[:, :, 0:4], mv[:, :, 4:8], mybir.AluOpType.add)
            w = wp.tile([128, BCMAX // 4], bf, tag=s1tag + "w")
            uv = u[:, 0:bc // 2].rearrange("p (q h) -> p q h", h=4)
            nc.vector.tensor_tensor(
                w[:, 0:bc // 4].rearrange("p (q h) -> p q h", h=2),
                uv[:, :, 0:2], uv[:, :, 2:4], mybir.AluOpType.add)
            for (tt, nt, g, ro) in runs:
                nc.vector.tensor_reduce(
                    dst_agg[:, tt:tt + nt],
                    w[:, ro // 4:ro // 4 + nt * (g // 4)].rearrange(
                        "p (t q) -> p t q", q=g // 4),
                    mybir.AxisListType.X, mybir.AluOpType.add)

        def body():
            agg = [accp.tile([128, TPC], f32, tag=f"agg{f}", name=f"agg{f}")
                   for f in range(nF)]
            for bi, (t0, ntb, c0, bc, runs) in enumerate(blocks):
                qa = nc.sync if bi % 2 == 0 else nc.scalar
                qb = nc.scalar if bi % 2 == 0 else nc.sync
                h1 = (bc // 2) & ~7
                ew_t = sp.tile([128, BCMAX], bf, tag="ew")
                if variant == "reduceonly":
                    qa.dma_start(ew_t[:, 0:4], ew_d[bi][:, 0:4])
                elif mode == "layer2":
                    qa.dma_start(ew_t[:, 0:bc], ew_d[bi][:])
                else:
                    qa.dma_start(ew_t[:, 0:h1], ew_d[bi][:, 0:h1])
                    qb.dma_start(ew_t[:, h1:bc], ew_d[bi][:, h1:bc])
                if variant == "dmaonly":
                    continue
                if mode == "deg":
                    _reduce_2stage(ew_t, bc, runs, agg[0], "s1a")
                elif mode == "layer1":
                    xs_t = []
                    for f in range(D):
                        t_ = sp.tile([128, BCMAX], bf, tag=f"xs{f}")
                        if variant != "reduceonly":
                            qb.dma_start(t_[:, 0:h1], xs_d[f][bi][:, 0:h1])
                            qa.dma_start(t_[:, h1:bc], xs_d[f][bi][:, h1:bc])
                        else:
                            qb.dma_start(t_[:, 0:4], xs_d[f][bi][:, 0:4])
                        xs_t.append(t_)
                    m0 = wp.tile([128, BCMAX], bf, tag="m0")
                    nc.vector.tensor_mul(m0[:, 0:bc], ew_t[:, 0:bc],
                                         xs_t[0][:, 0:bc])
                    m1 = wp.tile([128, BCMAX], bf, tag="m1")
                    nc.vector.tensor_mul(m1[:, 0:bc], ew_t[:, 0:bc],
                                         xs_t[1][:, 0:bc])
                    _reduce_2stage(m0, bc, runs, agg[0], "s1a")
                    _reduce_2stage(m1, bc, runs, agg[1], "s1b")
                else:
                    vs_t = sp.tile([128, BCMAX], bf, tag="vs")
                    if variant != "reduceonly":
                        qb.dma_start(vs_t[:, 0:bc], vs_d[bi][:])
                    else:
                        qb.dma_start(vs_t[:, 0:4], vs_d[bi][:, 0:4])
                    m0 = wp.tile([128, BCMAX], bf, tag="m0")
                    nc.vector.tensor_mul(m0[:, 0:bc], ew_t[:, 0:bc],
                                         vs_t[:, 0:bc])
                    _reduce_2stage(m0, bc, runs, agg[0], "s1a")

            if variant in ("dmaonly", "reduceonly", "noepi"):
                return
            # ---- epilogue (self-loop slots make agg complete: deg sweep
            # yields deg+1; layer sweeps include the dinv*val self term) ----
            if mode == "deg":
                sq = wp.tile([128, TPC], f32, tag="sq")
                nc.scalar.activation(sq, agg[0],
                                     mybir.ActivationFunctionType.Sqrt)
                pk = wp.tile([128, (D + 1) * TPC], f32, tag="pk")
                nc.vector.reciprocal(pk[:, 0:TPC], sq)
                for f in range(D):
                    nc.vector.tensor_mul(pk[:, (1 + f) * TPC:(2 + f) * TPC],
                                         xN[f], pk[:, 0:TPC])
                nc.sync.dma_start(deg_out[:], pk[:])
            elif mode == "layer1":
                zb = []
                for f in range(D):
                    zb_ = wp.tile([128, TPC], bf, tag=f"zb{f}")
                    nc.vector.tensor_mul(zb_, agg[f], dinvN)
                    zb.append(zb_)
                # h layout [128, (j t)]: hidden-unit major; weights arrive
                # pre-materialized in the same layout so every product runs
                # in the 2x packed-bf16 DVE mode.
                hA = wp.tile([128, TPC * HID], bf, tag="hA")
                nc.vector.tensor_tensor(
                    hA.rearrange("p (j t) -> p j t", j=HID),
                    zb[0].unsqueeze(1).broadcast_to([128, HID, TPC]),
                    w1x[0].rearrange("p (j t) -> p j t", j=HID),
                    mybir.AluOpType.mult)
                hB = wp.tile([128, TPC * HID], bf, tag="hB")
                nc.vector.tensor_tensor(
                    hB.rearrange("p (j t) -> p j t", j=HID),
                    zb[1].unsqueeze(1).broadcast_to([128, HID, TPC]),
                    w1x[1].rearrange("p (j t) -> p j t", j=HID),
                    mybir.AluOpType.mult)
                nc.vector.tensor_add(hA, hA, hB)
                if not skip_b1:
                    nc.vector.tensor_tensor(
                        hA.rearrange("p (j t) -> p j t", j=HID),
                        hA.rearrange("p (j t) -> p j t", j=HID),
                        b1_sb.unsqueeze(2).broadcast_to([128, HID, TPC]),
                        mybir.AluOpType.add)
                nc.vector.tensor_scalar_max(hA, hA, 0.0)
                hv = wp.tile([128, TPC * HID], bf, tag="hv")
                nc.vector.tensor_tensor(hv, hA, w2x_sb[:],
                                        mybir.AluOpType.mult)
                T8 = 8 * TPC
                va = wp.tile([128, T8], bf, tag="va")
                nc.vector.tensor_add(va, hv[:, 0:T8], hv[:, T8:2 * T8])
                vb = wp.tile([128, T8 // 2], bf, tag="vb")
                nc.vector.tensor_add(vb, va[:, 0:T8 // 2], va[:, T8 // 2:T8])
                vc = wp.tile([128, T8 // 4], bf, tag="vc")
                nc.vector.tensor_add(vc, vb[:, 0:T8 // 4],
                                     vb[:, T8 // 4:T8 // 2])
                vd = wp.tile([128, TPC], f32, tag="vd")
                nc.vector.tensor_add(vd, vc[:, 0:TPC], vc[:, TPC:2 * TPC])
                vt = wp.tile([128, TPC], f32, tag="vt")
                nc.vector.tensor_mul(vt, vd, dinvN)
                nc.sync.dma_start(vt_out[:], vt[:])
            else:
                y = wp.tile([128, TPC], f32, tag="y")
                nc.vector.tensor_mul(y, agg[0], dinvN)
                if not skip_b2:
                    nc.vector.tensor_scalar(y, y, b2_sb[:, 0:1], None,
                                            mybir.AluOpType.add)
                nc.sync.dma_start(y_out[:], y[:])

        if reps > 1:
            assert reps % unroll == 0
            with tc.For_i(0, reps // unroll, 1):
                for _ in range(unroll):
                    body()
        else:
            body()

    _split_multi_waits(nc)
    return nc


def _rep_bf16(vec):
    return np.ascontiguousarray(
        np.tile(np.asarray(vec, np.float32).reshape(1, -1), (128, 1))
    ).astype(BF16)


def kernel(x, edge_index, edge_weight, W1, b1, W2, b2):
    x = np.asarray(x, np.float32)
    edge_index = np.asarray(edge_index)
    edge_weight = np.asarray(edge_weight, np.float32)
    W1 = np.asarray(W1, np.float32)
    b1 = np.asarray(b1, np.float32)
    W2 = np.asarray(W2, np.float32)
    b2 = np.asarray(b2, np.float32)
    skip_b1 = not np.any(b1 != 0)
    skip_b2 = not np.any(b2 != 0)

    pp = _preprocess(edge_index, edge_weight)
    order = pp["order"]

    ew_cs = _stream_blocks(pp, pp["ew"], "ew", BF16)

    xfull = np.zeros((NPAD, D), np.float32)
    xfull[:N] = x
    xnew = xfull[order]                          # newpos layout
    xn_cs = [_to_core_nodes(xnew[:, f], np.float32) for f in range(D)]

    # ---- NEFF 1: deg+1 -> dinv, x*dinv ----
    nc1 = _build_sweep("deg", pp)
    in1 = [dict(ew_cs[c], xn0=xn_cs[0][c], xn1=xn_cs[1][c])
           for c in range(NCORE)]
    r1 = run_bass_kernel_spmd(nc1, in1, core_ids=list(range(NCORE)))
    pk = [np.asarray(r1.results[c]["degout"], np.float32) for c in range(NCORE)]
    dinv_new = _from_core_nodes([p[:, 0:TPC] for p in pk])
    xt_new = [_from_core_nodes([p[:, (1 + f) * TPC:(2 + f) * TPC] for p in pk])
              for f in range(D)]

    # ---- host glue: per-edge (x*dinv)[src] streams ----
    xt_orig = np.empty((NPAD, D), np.float32)
    for f in range(D):
        xt_orig[order, f] = xt_new[f]
    xs_cs = [_stream_blocks(pp, xt_orig[pp["src"], f], f"xs{f}_", BF16)
             for f in range(D)]
    dinv_n = _to_core_nodes(dinv_new, np.float32)

    w1x = [_rep_bf16(np.repeat(W1[f], TPC)) for f in range(D)]
    b1b = _rep_bf16(b1)
    w2xb = _rep_bf16(np.repeat(W2[:, 0], TPC))
    b2b = np.full((128, 1), float(b2[0]), np.float32)

    # ---- NEFF 2: layer 1 -> v*dinv ----
    nc2 = _build_sweep("layer1", pp, skip_b1=skip_b1, skip_b2=skip_b2)
    in2 = [dict(ew_cs[c], **xs_cs[0][c], **xs_cs[1][c],
                dinvn=dinv_n[c], w1x0b=w1x[0], w1x1b=w1x[1], b1b=b1b,
                w2xb=w2xb) for c in range(NCORE)]
    r2 = run_bass_kernel_spmd(nc2, in2, core_ids=list(range(NCORE)))
    vt_new = _from_core_nodes([r2.results[c]["vtout"] for c in range(NCORE)])

    # ---- host glue: (v*dinv)[src] stream ----
    vt_orig = np.empty(NPAD, np.float32)
    vt_orig[order] = vt_new
    vs_cs = _stream_blocks(pp, vt_orig[pp["src"]], "vs", BF16)
    dinv_n2 = dinv_n

    # ---- NEFF 3: layer 2 -> output ----
    nc3 = _build_sweep("layer2", pp, skip_b1=skip_b1, skip_b2=skip_b2)
    in3 = [dict(ew_cs[c], **vs_cs[c], dinvn=dinv_n2[c], b2b=b2b)
           for c in range(NCORE)]
    r3 = run_bass_kernel_spmd(nc3, in3, core_ids=list(range(NCORE)))
    y_new = _from_core_nodes([r3.results[c]["yout"] for c in range(NCORE)])

    y_orig = np.empty(NPAD, np.float32)
    y_orig[order] = y_new
    return y_orig[:N, None].astype(np.float32)
